# revision 1
# baseline (speedup 1.0000x reference)
"""Trainium2 Bass kernel for nn_EnhancedTransformerBlock_80169859548047.

Sharding: 8 cores = (batch b, parity par). Core c handles batch b=c//2 and the
even (par=0) or odd (par=1) 128-token chunks of that batch's 2048-token
sequence. Interleaving chunks balances causal attention work; padding slot s's
key extent to 256*(s+1) tokens makes the instruction stream identical on all
cores — per-core differences live entirely in host-provided data (token
slices and additive score masks).

Dtypes: attention path (LN1 out, w_qkv, Q/K/V, P) in bf16; out-proj, experts,
router and gate matmuls in float32r; residual stream and LN math in fp32.
Softmax denominators come from a ones column appended per head to V (exact
PSUM accumulation). LN1/LN2 affines are folded into the following weights on
the host; the final LN affine is applied on the host after gathering.
"""

import numpy as np
import ml_dtypes

B, S, H, E, NH, HD = 4, 2048, 1024, 8, 16, 64
N_CORES = 8
EPS = 1e-12
SCALE = HD ** -0.5
MASKVAL = -80.0          # added after scale; exp(-80) ~ 1.8e-35 ~ 0
NSLOT = 8                # 128-token chunks per core
OWN = NSLOT * 128        # own tokens per core
HT = H // 128            # 8 H-tiles

_prog_cache = {}


def _build_program():
    import concourse.bacc as bacc
    import concourse.tile as tile
    import concourse.mybir as mybir
    from concourse.masks import make_identity
    from concourse.alu_op_type import AluOpType
    from contextlib import ExitStack

    F32 = mybir.dt.float32
    F32R = mybir.dt.float32r
    BF16 = mybir.dt.bfloat16
    AF = mybir.ActivationFunctionType

    nc = bacc.Bacc("TRN2", target_bir_lowering=False, debug=False, num_devices=1)

    def din(name, shape, dt):
        return nc.dram_tensor(name, list(shape), dt, kind="ExternalInput").ap()

    x_kv_d = din("x_kv", (S, H), F32)
    x_ownr_d = din("x_ownr", (OWN, H), F32)   # raw inputs, own tokens, slot order
    x_own_d = din("x_own", (OWN, H), F32)     # inputs + b_out, own tokens
    wq_d = din("wq", (H, 3 * H), BF16)
    bqkv_d = din("bqkv", (128, 16), F32)
    wout_d = din("wout", (H, H), F32R)
    wrout_d = din("wrout", (H, E), F32R)
    brout_d = din("brout", (128, E), F32)
    wexp_d = din("wexp", (E, H, H), F32R)
    bexp_d = din("bexp", (E, H), F32R)
    wal1_d = din("wal1", (H, 256), F32R)
    bal1_d = din("bal1", (128, 2), F32)
    wal2_d = din("wal2", (256, 1), F32R)
    masks_d = din("masks", (128, 16 * 512), BF16)
    thresh_d = din("thresh", (128, 1), F32)   # 0.8 - b_al2, replicated
    out_d = nc.dram_tensor("out", [OWN, H], F32, kind="ExternalOutput").ap()

    KT = [2 * s + 2 for s in range(NSLOT)]    # padded ktile count per slot

    with tile.TileContext(nc) as tc, ExitStack() as st:
        # manually-managed pools (non-LIFO lifetimes)
        small_cm = tc.tile_pool(name="small", bufs=1)
        small = small_cm.__enter__()
        id_bf = small.tile([128, 128], BF16)
        id_f = small.tile([128, 128], F32)
        bqkv_sb = small.tile([128, 16], F32)
        thresh_sb = small.tile([128, 1], F32)
        eps_sb = small.tile([128, 1], F32)
        nc.gpsimd.memset(eps_sb[:], EPS)
        rw_sb = small.tile([128, NSLOT, E], F32)
        rwT_cat = small.tile([1, E * OWN], F32)
        rwT_r = small.tile([8, OWN], F32R)
        maskrow = small.tile([1, OWN], F32)
        mask_pp = small.tile([128, NSLOT], F32)
        nc.sync.dma_start(bqkv_sb[:], bqkv_d[:])
        nc.sync.dma_start(thresh_sb[:], thresh_d[:])
        id_r_t = small.tile([128, 128], F32R)
        ones_f = small.tile([1, 64], F32)
        ones_r = small.tile([1, 64], F32R)
        make_identity(nc, id_bf[:])
        make_identity(nc, id_f[:])
        nc.vector.tensor_copy(id_r_t[:], id_f[:])
        nc.gpsimd.memset(ones_f[:], 1.0)
        nc.vector.tensor_copy(ones_r[:], ones_f[:])
        id_r = id_r_t[:]

        kvq_cm = tc.tile_pool(name="kvq", bufs=1)
        kvq = kvq_cm.__enter__()
        KTb = kvq.tile([128, HT, S], BF16)            # K^T [kcol, tok]
        Vb = kvq.tile([128, 16, NH * 65], BF16)       # V token-major + ones col
        QTb = kvq.tile([128, HT, OWN], BF16)          # Q^T [qcol, own tok]

        # =========== Phase A: LN1 + transpose + QKV ===========
        def layer_norm_apply(pool, src_ap, out_ap, out_is_act=True):
            stats = pool.tile([128, 2, 6], F32, tag="st")
            nc.vector.bn_stats(stats[:, 0, :], src_ap[:, 0:512])
            nc.vector.bn_stats(stats[:, 1, :], src_ap[:, 512:1024])
            mv = pool.tile([128, 2], F32, tag="mv")
            nc.vector.bn_aggr(mv[:], stats[:])
            sd = pool.tile([128, 1], F32, tag="sd")
            nc.scalar.activation(sd[:], mv[:, 1:2], AF.Sqrt, bias=eps_sb[:])
            rstd = pool.tile([128, 1], F32, tag="rs")
            nc.vector.reciprocal(rstd[:], sd[:])
            nbias = pool.tile([128, 1], F32, tag="nb")
            nc.vector.scalar_tensor_tensor(
                nbias[:], mv[:, 0:1], -1.0, rstd[:],
                AluOpType.mult, AluOpType.mult)
            nc.scalar.activation(out_ap, src_ap, AF.Identity,
                                 bias=nbias[:], scale=rstd[:])
            return mv, rstd

        with (
            tc.tile_pool(name="xln_pool", bufs=1) as xlnp,
            tc.tile_pool(name="a_io", bufs=2) as aio,
            tc.tile_pool(name="a_w", bufs=2) as aw,
            tc.tile_pool(name="a_pst", bufs=2, space="PSUM") as apst,
            tc.tile_pool(name="a_psq", bufs=2, space="PSUM") as apsq,
            tc.tile_pool(name="a_ps2", bufs=2, space="PSUM") as aps2,
        ):
            xlnT = xlnp.tile([128, HT, S], BF16)
            xownT = xlnp.tile([128, HT, OWN], BF16)
            for tt in range(16):
                xt = aio.tile([128, H], F32, tag="xt")
                nc.sync.dma_start(xt[:], x_kv_d[tt * 128:(tt + 1) * 128, :])
                xl = aio.tile([128, H], BF16, tag="xl")
                layer_norm_apply(aio, xt[:], xl[:])
                for kt in range(HT):
                    tp = apst.tile([128, 128], BF16, tag="tp")
                    nc.tensor.transpose(tp[:], xl[:, kt * 128:(kt + 1) * 128], id_bf[:])
                    nc.vector.tensor_copy(xlnT[:, kt, tt * 128:(tt + 1) * 128], tp[:])
            for tt in range(NSLOT):
                xt = aio.tile([128, H], F32, tag="xt")
                nc.sync.dma_start(xt[:], x_ownr_d[tt * 128:(tt + 1) * 128, :])
                xl = aio.tile([128, H], BF16, tag="xl")
                layer_norm_apply(aio, xt[:], xl[:])
                for kt in range(HT):
                    tp = apst.tile([128, 128], BF16, tag="tp")
                    nc.tensor.transpose(tp[:], xl[:, kt * 128:(kt + 1) * 128], id_bf[:])
                    nc.vector.tensor_copy(xownT[:, kt, tt * 128:(tt + 1) * 128], tp[:])

            # Q^T (own tokens) / K^T (all tokens): weight-stationary
            for qc in range(8):
                wcol = aw.tile([128, HT, 128], BF16, tag="wcol")
                nc.sync.dma_start(
                    wcol[:], wq_d[:, qc * 128:(qc + 1) * 128]
                    .rearrange("(kt p) c -> p kt c", p=128))
                for half in range(2):
                    ps = apsq.tile([128, 512], F32, tag="qps")
                    for kt in range(HT):
                        nc.tensor.matmul(ps[:], wcol[:, kt, :],
                                         xownT[:, kt, half * 512:(half + 1) * 512],
                                         start=(kt == 0), stop=(kt == HT - 1))
                    nc.scalar.activation(QTb[:, qc, half * 512:(half + 1) * 512],
                                         ps[:], AF.Identity,
                                         bias=bqkv_sb[:, qc:qc + 1])
            for kc in range(8):
                wcol = aw.tile([128, HT, 128], BF16, tag="wcol")
                nc.sync.dma_start(
                    wcol[:], wq_d[:, H + kc * 128:H + (kc + 1) * 128]
                    .rearrange("(kt p) c -> p kt c", p=128))
                for n in range(4):
                    ps = aps2.tile([128, 512], F32, tag="big")
                    for kt in range(HT):
                        nc.tensor.matmul(ps[:], wcol[:, kt, :],
                                         xlnT[:, kt, n * 512:(n + 1) * 512],
                                         start=(kt == 0), stop=(kt == HT - 1))
                    nc.scalar.activation(KTb[:, kc, n * 512:(n + 1) * 512], ps[:],
                                         AF.Identity, bias=bqkv_sb[:, 8 + kc:9 + kc])
            # V token-major (activation-stationary)
            for vh in range(2):
                wv = aw.tile([128, HT, 512], BF16, tag="wv")
                nc.sync.dma_start(
                    wv[:], wq_d[:, 2 * H + vh * 512:2 * H + (vh + 1) * 512]
                    .rearrange("(kt p) c -> p kt c", p=128))
                for tt in range(16):
                    ps = aps2.tile([128, 512], F32, tag="big")
                    for kt in range(HT):
                        nc.tensor.matmul(ps[:], xlnT[:, kt, tt * 128:(tt + 1) * 128],
                                         wv[:, kt, :],
                                         start=(kt == 0), stop=(kt == HT - 1))
                    for h8 in range(8):
                        hh = vh * 8 + h8
                        nc.vector.tensor_copy(Vb[:, tt, hh * 65:hh * 65 + 64],
                                              ps[:, h8 * 64:(h8 + 1) * 64])
            for tt in range(16):
                nc.gpsimd.memset(Vb[:, tt, 64:NH * 65:65], 1.0)

        # =========== Phase B: attention ===========
        attn_cm = tc.tile_pool(name="attn_p", bufs=1, side="right")
        attn_p = attn_cm.__enter__()
        attnT = attn_p.tile([128, HT, OWN], F32R)
        with (
            tc.tile_pool(name="maskp", bufs=1) as maskp,
            tc.tile_pool(name="b_p", bufs=4) as bp,
            tc.tile_pool(name="b_sc", bufs=4) as bsc,
            tc.tile_pool(name="b_ps", bufs=2, space="PSUM") as bps,
            tc.tile_pool(name="b_pv", bufs=2, space="PSUM") as bpv,
            tc.tile_pool(name="b_pn", bufs=1, space="PSUM") as bpn,
        ):
            masks_sb = maskp.tile([128, 16 * 512], BF16)
            nc.sync.dma_start(masks_sb[:], masks_d[:])
            for pr in range(4):
                nkt = 4 * pr + 4
                for hp in range(8):
                    # pass 1: scores + exp, P tiles parked in SBUF
                    p_ts = []
                    for kt in range(nkt):
                        spsA = bps.tile([128, 256], F32, tag="spsA", bufs=2)
                        spsB = bps.tile([128, 256], F32, tag="spsB", bufs=1)
                        nc.tensor.matmul(
                            spsA[:],
                            KTb[0:64, hp, kt * 128:(kt + 1) * 128],
                            QTb[0:64, hp, pr * 256:(pr + 1) * 256],
                            start=True, stop=True)
                        nc.tensor.matmul(
                            spsB[:],
                            KTb[64:128, hp, kt * 128:(kt + 1) * 128],
                            QTb[64:128, hp, pr * 256:(pr + 1) * 256],
                            start=True, stop=True)
                        p_t = bp.tile([128, 512], BF16, tag="p_t", bufs=20)
                        if kt >= nkt - 4:
                            j = kt - (nkt - 4)
                            mc = (pr * 4 + j) * 512
                            sm = bsc.tile([128, 512], F32, tag="sm")
                            nc.vector.scalar_tensor_tensor(
                                sm[:, 0:256], spsA[:], SCALE,
                                masks_sb[:, mc:mc + 256],
                                AluOpType.mult, AluOpType.add)
                            nc.vector.scalar_tensor_tensor(
                                sm[:, 256:512], spsB[:], SCALE,
                                masks_sb[:, mc + 256:mc + 512],
                                AluOpType.mult, AluOpType.add)
                            nc.scalar.activation(p_t[:], sm[:], AF.Exp)
                        else:
                            nc.scalar.activation(p_t[:, 0:256], spsA[:], AF.Exp,
                                                 scale=SCALE)
                            nc.scalar.activation(p_t[:, 256:512], spsB[:], AF.Exp,
                                                 scale=SCALE)
                        p_ts.append(p_t)
                    # pass 2: dense PV runs
                    pvA = bpv.tile([65, 256], F32, tag="pvA")
                    pvB = bpv.tile([65, 256], F32, tag="pvB")
                    for kt in range(nkt):
                        nc.tensor.matmul(pvA[:],
                                         Vb[:, kt, (2 * hp) * 65:(2 * hp + 1) * 65],
                                         p_ts[kt][:, 0:256],
                                         start=(kt == 0), stop=(kt == nkt - 1))
                        nc.tensor.matmul(pvB[:],
                                         Vb[:, kt, (2 * hp + 1) * 65:(2 * hp + 2) * 65],
                                         p_ts[kt][:, 256:512],
                                         start=(kt == 0), stop=(kt == nkt - 1))
                    for hi, pv in ((0, pvA), (1, pvB)):
                        hb = 64 * hi
                        rcr = bsc.tile([1, 256], F32R, tag="rcr")
                        with nc.allow_low_precision(reason="f32r feed for bcast"):
                            nc.vector.reciprocal(rcr[:], pv[64:65, :])
                        bcp = bpn.tile([64, 256], F32, tag="bcp")
                        nc.tensor.matmul(bcp[:], ones_r[:], rcr[:],
                                         start=True, stop=True)
                        bcs = bsc.tile([64, 256], F32, tag="bcs")
                        nc.vector.tensor_copy(bcs[:], bcp[:])
                        nc.vector.tensor_tensor(
                            attnT[hb:hb + 64, hp, pr * 256:(pr + 1) * 256],
                            pv[0:64, :], bcs[:], AluOpType.mult)

        # =========== Phase C: out-proj + residual + LN2 + y^T ===========
        kvq_cm.__exit__(None, None, None)
        hy_cm = tc.tile_pool(name="hy", bufs=1)
        hy = hy_cm.__enter__()
        h_sb = hy.tile([128, NSLOT, H], F32)
        yT_sb = hy.tile([128, HT, OWN], F32R)
        with (
            tc.tile_pool(name="c_w", bufs=2) as cw,
            tc.tile_pool(name="c_io", bufs=3) as cio,
            tc.tile_pool(name="c_ps", bufs=2, space="PSUM") as cps,
            tc.tile_pool(name="c_pst", bufs=4, space="PSUM") as cpst,
        ):
            for n in range(2):
                won = cw.tile([128, HT, 512], F32R, tag="won")
                nc.sync.dma_start(
                    won[:], wout_d[:, n * 512:(n + 1) * 512]
                    .rearrange("(kt p) c -> p kt c", p=128))
                for tt in range(NSLOT):
                    ps = cps.tile([128, 512], F32, tag="ops")
                    for kt in range(HT):
                        nc.tensor.matmul(ps[:], attnT[:, kt, tt * 128:(tt + 1) * 128],
                                         won[:, kt, :],
                                         start=(kt == 0), stop=(kt == HT - 1))
                    xo = cio.tile([128, 512], F32, tag="xo")
                    nc.sync.dma_start(
                        xo[:], x_own_d[tt * 128:(tt + 1) * 128, n * 512:(n + 1) * 512])
                    nc.vector.tensor_tensor(h_sb[:, tt, n * 512:(n + 1) * 512],
                                            ps[:], xo[:], AluOpType.add)
            for tt in range(NSLOT):
                yt = cio.tile([128, H], F32R, tag="yt")
                layer_norm_apply(cio, h_sb[:, tt, :], yt[:])
                for kt in range(HT):
                    tp = cpst.tile([128, 128], F32R, tag="tp2")
                    nc.tensor.transpose(tp[:], yt[:, kt * 128:(kt + 1) * 128], id_r)
                    nc.vector.tensor_copy(yT_sb[:, kt, tt * 128:(tt + 1) * 128], tp[:])

        # =========== Phase D: router + experts ===========
        attn_cm.__exit__(None, None, None)
        h2p_cm = tc.tile_pool(name="h2p", bufs=1, side="right")
        h2p = h2p_cm.__enter__()
        h2_sb = h2p.tile([128, NSLOT, H], F32)
        with (
            tc.tile_pool(name="d_sc", bufs=2) as dsc,
            tc.tile_pool(name="d_ps", bufs=2, space="PSUM") as dps,
            tc.tile_pool(name="d_pst", bufs=2, space="PSUM") as dpst,
        ):
            wr = dsc.tile([128, HT, E], F32R, tag="wr")
            nc.sync.dma_start(wr[:], wrout_d.rearrange("(kt p) e -> p kt e", p=128))
            br = dsc.tile([128, E], F32, tag="br")
            nc.sync.dma_start(br[:], brout_d[:])
            for tt in range(NSLOT):
                ps = dps.tile([128, E], F32, tag="rps")
                for kt in range(HT):
                    nc.tensor.matmul(ps[:], yT_sb[:, kt, tt * 128:(tt + 1) * 128],
                                     wr[:, kt, :],
                                     start=(kt == 0), stop=(kt == HT - 1))
                lg = dsc.tile([128, E], F32, tag="lg")
                nc.vector.tensor_tensor(lg[:], ps[:], br[:], AluOpType.add)
                ex = dsc.tile([128, E], F32, tag="ex")
                rsum = dsc.tile([128, 1], F32, tag="rsum")
                nc.scalar.activation(ex[:], lg[:], AF.Exp, accum_out=rsum[:])
                rrec = dsc.tile([128, 1], F32, tag="rrec")
                nc.vector.reciprocal(rrec[:], rsum[:])
                nc.vector.tensor_scalar(rw_sb[:, tt, :], ex[:], rrec[:], None,
                                        AluOpType.mult)
                tp = dpst.tile([8, 128], F32, tag="rtp")
                nc.tensor.transpose(tp[:], rw_sb[:, tt, :], id_f[:])
                nc.vector.tensor_copy(rwT_r[:, tt * 128:(tt + 1) * 128], tp[:])
            for e in range(E):
                # gather rw[:, :, e] into a partition-0 row (token order tt*128+p)
                for tt in range(NSLOT):
                    nc.sync.dma_start(
                        rwT_cat[0:1, e * OWN + tt * 128:e * OWN + (tt + 1) * 128],
                        rw_sb[:, tt, e:e + 1])

        with (
            tc.tile_pool(name="d_y", bufs=2) as dy,
            tc.tile_pool(name="d_w", bufs=4) as dw,
            tc.tile_pool(name="d_b", bufs=1) as db,
            tc.tile_pool(name="d_moe", bufs=1, space="PSUM") as dmoe,
        ):
            bexp_sb = db.tile([8, H], F32R)
            nc.sync.dma_start(bexp_sb[:], bexp_d[:])
            for fh in range(2):
                mps = [dmoe.tile([128, 512], F32, tag=f"mps{tt}",
                                 name=f"mps_{fh}_{tt}")
                       for tt in range(NSLOT)]
                for e in range(E):
                    bcr = dy.tile([128, OWN], F32, tag="bcr")
                    nc.gpsimd.partition_broadcast(bcr[:], rwT_cat[0:1, e * OWN:(e + 1) * OWN])
                    for kt in range(HT):
                        ye = dy.tile([128, OWN], F32R, tag="ye", bufs=4)
                        nc.vector.tensor_tensor(ye[:], yT_sb[:, kt, :],
                                                bcr[:], AluOpType.mult)
                        we = dw.tile([128, 512], F32R, tag="we")
                        nc.sync.dma_start(
                            we[:],
                            wexp_d[e, kt * 128:(kt + 1) * 128,
                                   fh * 512:(fh + 1) * 512])
                        for tt in range(NSLOT):
                            nc.tensor.matmul(
                                mps[tt][:], ye[:, tt * 128:(tt + 1) * 128],
                                we[:],
                                start=(e == 0 and kt == 0), stop=False)
                for tt in range(NSLOT):
                    nc.tensor.matmul(mps[tt][:],
                                     rwT_r[:, tt * 128:(tt + 1) * 128],
                                     bexp_sb[:, fh * 512:(fh + 1) * 512],
                                     start=False, stop=True)
                    nc.vector.tensor_tensor(
                        h2_sb[:, tt, fh * 512:(fh + 1) * 512], mps[tt][:],
                        h_sb[:, tt, fh * 512:(fh + 1) * 512], AluOpType.add)

        # =========== Phase E: gate + LNf + output ===========
        hy_cm.__exit__(None, None, None)
        with (
            tc.tile_pool(name="e_sc", bufs=1) as esc,
            tc.tile_pool(name="e_tmp", bufs=2) as etmp,
            tc.tile_pool(name="e_pst", bufs=3, space="PSUM") as epst,
            tc.tile_pool(name="e_psg", bufs=2, space="PSUM") as epsg,
            tc.tile_pool(name="e_psal", bufs=1, space="PSUM") as epsal,
            tc.tile_pool(name="e_out", bufs=3) as eout,
        ):
            h2T = esc.tile([128, HT, OWN], F32R)
            for tt in range(NSLOT):
                for kt in range(HT):
                    tp = epst.tile([128, 128], F32, tag="tp3")
                    nc.tensor.transpose(
                        tp[:], h2_sb[:, tt, kt * 128:(kt + 1) * 128], id_f[:])
                    nc.vector.tensor_copy(h2T[:, kt, tt * 128:(tt + 1) * 128], tp[:])
            wal1 = esc.tile([128, HT, 256], F32R)
            nc.sync.dma_start(wal1[:], wal1_d.rearrange("(kt p) c -> p kt c", p=128))
            bal1 = esc.tile([128, 2], F32)
            nc.sync.dma_start(bal1[:], bal1_d[:])
            wal2 = esc.tile([128, 2, 1], F32R)
            nc.sync.dma_start(wal2[:], wal2_d.rearrange("(m p) c -> p m c", p=128))
            gT = esc.tile([128, 2, OWN], F32R)
            for m2 in range(2):
                for n in range(2):
                    ps = epsg.tile([128, 512], F32, tag="gps")
                    for kt in range(HT):
                        nc.tensor.matmul(ps[:],
                                         wal1[:, kt, m2 * 128:(m2 + 1) * 128],
                                         h2T[:, kt, n * 512:(n + 1) * 512],
                                         start=(kt == 0), stop=(kt == HT - 1))
                    nc.scalar.activation(gT[:, m2, n * 512:(n + 1) * 512], ps[:],
                                         AF.Gelu, bias=bal1[:, m2:m2 + 1])
            for n in range(2):
                ps = epsal.tile([1, 512], F32, tag="alps")
                for m2 in range(2):
                    nc.tensor.matmul(ps[:], wal2[:, m2, :],
                                     gT[:, m2, n * 512:(n + 1) * 512],
                                     start=(m2 == 0), stop=(m2 == 1))
                nc.vector.tensor_scalar(maskrow[:, n * 512:(n + 1) * 512], ps[:],
                                        thresh_sb[0:1, 0:1], None, AluOpType.is_gt)
            for tt in range(NSLOT):
                nc.sync.dma_start(
                    mask_pp[:, tt:tt + 1],
                    maskrow[0:1, tt * 128:(tt + 1) * 128])
            for tt in range(NSLOT):
                stats = eout.tile([128, 2, 6], F32, tag="st3")
                nc.vector.bn_stats(stats[:, 0, :], h2_sb[:, tt, 0:512])
                nc.vector.bn_stats(stats[:, 1, :], h2_sb[:, tt, 512:1024])
                mv = eout.tile([128, 2], F32, tag="mv3")
                nc.vector.bn_aggr(mv[:], stats[:])
                sd = eout.tile([128, 1], F32, tag="sd3")
                nc.scalar.activation(sd[:], mv[:, 1:2], AF.Sqrt, bias=eps_sb[:])
                rstd = eout.tile([128, 1], F32, tag="rs3")
                nc.vector.reciprocal(rstd[:], sd[:])
                seff = eout.tile([128, 1], F32, tag="se3")
                nc.vector.tensor_tensor(seff[:], rstd[:], mask_pp[:, tt:tt + 1],
                                        AluOpType.mult)
                beff = eout.tile([128, 1], F32, tag="be3")
                nc.vector.scalar_tensor_tensor(beff[:], mv[:, 0:1], -1.0, seff[:],
                                               AluOpType.mult, AluOpType.mult)
                ot = eout.tile([128, H], F32, tag="ot")
                nc.scalar.activation(ot[:], h2_sb[:, tt, :], AF.Identity,
                                     bias=beff[:], scale=seff[:])
                nc.sync.dma_start(out_d[tt * 128:(tt + 1) * 128, :], ot[:])

        h2p_cm.__exit__(None, None, None)
        small_cm.__exit__(None, None, None)

    nc.compile()
    return nc


def _prep_host(inputs):
    f32 = np.float32
    bf16 = ml_dtypes.bfloat16
    x = np.asarray(inputs["inputs"], f32)
    ln1_g = np.asarray(inputs["ln1_g"], f32); ln1_b = np.asarray(inputs["ln1_b"], f32)
    w_qkv = np.asarray(inputs["w_qkv"], f32); b_qkv = np.asarray(inputs["b_qkv"], f32)
    w_out = np.asarray(inputs["w_out"], f32); b_out = np.asarray(inputs["b_out"], f32)
    ln2_g = np.asarray(inputs["ln2_g"], f32); ln2_b = np.asarray(inputs["ln2_b"], f32)
    w_router = np.asarray(inputs["w_router"], f32)
    b_router = np.asarray(inputs["b_router"], f32)
    w_exp = np.asarray(inputs["w_exp"], f32); b_exp = np.asarray(inputs["b_exp"], f32)
    w_al1 = np.asarray(inputs["w_al1"], f32); b_al1 = np.asarray(inputs["b_al1"], f32)
    w_al2 = np.asarray(inputs["w_al2"], f32); b_al2 = np.asarray(inputs["b_al2"], f32)

    wq_f = (ln1_g[:, None] * w_qkv).astype(bf16)
    bq_f = b_qkv + ln1_b @ w_qkv
    assert np.all(bq_f[2 * H:] == 0.0), "nonzero V bias not supported"
    bqkv_t = np.zeros((128, 16), f32)
    for j in range(16):
        bqkv_t[:, j] = bq_f[j * 128:(j + 1) * 128]
    wr_f = ln2_g[:, None] * w_router
    br_f = b_router + ln2_b @ w_router
    we_f = ln2_g[None, :, None] * w_exp
    be_f = b_exp + np.einsum("h,ehf->ef", ln2_b, w_exp)
    x_pb = x + b_out[None, None, :]

    def mk_masks(par):
        # S^T [k, q] masks per (slot-pair pr, kt-rel j in last 4), cols
        # [headA 256 | headB 256]; each 256 spans slots (2pr, 2pr+1).
        m = np.zeros((128, 4, 4, 2, 2, 128), f32)
        for pr in range(4):
            nkt = 4 * pr + 4
            for j in range(4):
                kt = nkt - 4 + j
                for sl_i, sl in enumerate((2 * pr, 2 * pr + 1)):
                    g = 2 * sl + par           # global 128-chunk of this slot
                    kk = np.arange(128)[:, None] + kt * 128
                    qq = np.arange(128)[None, :] + g * 128
                    m[:, pr, j, :, sl_i, :] = np.where(kk > qq, MASKVAL, 0.0)[:, None, :]
        return m.reshape(128, 4 * 4 * 2 * 2 * 128).astype(bf16)
    masks = [mk_masks(0), mk_masks(1)]

    thresh = np.full((128, 1), 0.8 - float(b_al2[0]), f32)
    bal1_t = np.zeros((128, 2), f32)
    bal1_t[:, 0] = b_al1[0:128]
    bal1_t[:, 1] = b_al1[128:256]

    shared = dict(
        wq=np.ascontiguousarray(wq_f),
        bqkv=bqkv_t,
        wout=np.ascontiguousarray(w_out),
        wrout=np.ascontiguousarray(wr_f.astype(f32)),
        brout=np.tile(br_f[None, :], (128, 1)).astype(f32),
        wexp=np.ascontiguousarray(we_f.astype(f32)),
        bexp=np.ascontiguousarray(be_f.astype(f32)),
        wal1=np.ascontiguousarray(w_al1),
        bal1=bal1_t,
        wal2=np.ascontiguousarray(w_al2),
        thresh=thresh,
    )
    per_core = []
    for c in range(N_CORES):
        b, par = c // 2, c % 2
        own_idx = np.concatenate(
            [np.arange(128) + (2 * s + par) * 128 for s in range(NSLOT)])
        m = dict(shared)
        m["x_kv"] = np.ascontiguousarray(x[b])
        m["x_ownr"] = np.ascontiguousarray(x[b][own_idx])
        m["x_own"] = np.ascontiguousarray(x_pb[b][own_idx])
        m["masks"] = masks[par]
        per_core.append(m)
    return per_core


def kernel(**inputs):
    from concourse.bass_utils import run_bass_kernel_spmd

    if "prog" not in _prog_cache:
        _prog_cache["prog"] = _build_program()
    nc = _prog_cache["prog"]

    per_core = _prep_host(inputs)
    trace = bool(globals().get("TRACE", False))
    res = run_bass_kernel_spmd(nc, per_core, core_ids=list(range(N_CORES)),
                               trace=trace)
    _prog_cache["last_result"] = res

    lnf_g = np.asarray(inputs["lnf_g"], np.float32)
    lnf_b = np.asarray(inputs["lnf_b"], np.float32)
    out = np.zeros((B, S, H), np.float32)
    for c in range(N_CORES):
        b, par = c // 2, c % 2
        o = res.results[c]["out"]
        for s in range(NSLOT):
            g0 = (2 * s + par) * 128
            out[b, g0:g0 + 128, :] = o[s * 128:(s + 1) * 128, :]
    return out * lnf_g[None, None, :] + lnf_b[None, None, :]



# revision 11
# speedup vs baseline: 1.1585x; 1.1585x over previous
"""Trainium2 Bass kernel for nn_EnhancedTransformerBlock_80169859548047.

Sharding: 8 cores = (batch b, parity par). Core c handles batch b=c//2 and the
even (par=0) or odd (par=1) 128-token chunks of that batch's 2048-token
sequence. The attention schedule is parity-uniform: for key chunk kt the query
suffix starts at slot fs(kt)=kt//2; the first suffix block's causal mask
(triangular / all-ones / all-zeros, depending on parity) is host data, so the
instruction stream is identical on all cores.

Dtypes: attention path (LN1 out, w_qkv, Q/K/V, P) in bf16; out-proj, experts,
router and gate matmuls in float32r; residual stream and LN math in fp32.
Softmax denominators come from a ones column appended per head to V (exact
PSUM accumulation). LN1/LN2 affines are folded into the following weights on
the host; the final LN affine is applied on the host after gathering.
"""

import numpy as np
import ml_dtypes

B, S, H, E, NH, HD = 4, 2048, 1024, 8, 16, 64
N_CORES = 8
EPS = 1e-12
SCALE = HD ** -0.5
NSLOT = 8                # 128-token chunks per core
OWN = NSLOT * 128        # own tokens per core
HT = H // 128            # 8 H-tiles
FS = [kt // 2 for kt in range(16)]          # suffix first slot per key chunk
# attention schedule: key chunks 0..7 processed singly (suffix > 512 wide),
# 8..15 in pairs (same fs, suffix <= 512)
SCHED = [(0,), (1,), (2,), (3,), (4,), (5,), (6,), (7,),
         (8, 9), (10, 11), (12, 13), (14, 15)]

_prog_cache = {}


def _build_program():
    import concourse.bacc as bacc
    import concourse.tile as tile
    import concourse.mybir as mybir
    from concourse.masks import make_identity
    from concourse.alu_op_type import AluOpType
    from contextlib import ExitStack

    F32 = mybir.dt.float32
    F32R = mybir.dt.float32r
    BF16 = mybir.dt.bfloat16
    AF = mybir.ActivationFunctionType

    nc = bacc.Bacc("TRN2", target_bir_lowering=False, debug=False, num_devices=1)

    def din(name, shape, dt):
        return nc.dram_tensor(name, list(shape), dt, kind="ExternalInput").ap()

    x_kv_d = din("x_kv", (S, H), F32)
    x_ownr_d = din("x_ownr", (OWN, H), F32)   # raw inputs, own tokens, slot order
    x_own_d = din("x_own", (OWN, H), F32)     # inputs + b_out, own tokens
    wq_d = din("wq", (H, 3 * H), BF16)
    bqkv_d = din("bqkv", (128, 16), F32)
    wout_d = din("wout", (H, H), F32R)
    wrout_d = din("wrout", (H, E), F32R)
    brout_d = din("brout", (128, E), F32)
    wexp_d = din("wexp", (E, H, H), F32R)
    bexp_d = din("bexp", (E, H), F32R)
    wal1_d = din("wal1", (H, 256), F32R)
    bal1_d = din("bal1", (128, 2), F32)
    wal2_d = din("wal2", (256, 1), F32R)
    bmask_d = din("bmask", (128, 16 * 128), BF16)  # per-kt first-block masks
    thresh_d = din("thresh", (128, 1), F32)   # 0.8 - b_al2, replicated
    out_d = nc.dram_tensor("out", [OWN, H], F32, kind="ExternalOutput").ap()

    with tile.TileContext(nc) as tc, ExitStack() as st:
        # manually-managed pools (non-LIFO lifetimes)
        small_cm = tc.tile_pool(name="small", bufs=1)
        small = small_cm.__enter__()
        id_bf = small.tile([128, 128], BF16)
        id_f = small.tile([128, 128], F32)
        bqkv_sb = small.tile([128, 16], F32)
        thresh_sb = small.tile([128, 1], F32)
        eps_sb = small.tile([128, 1], F32)
        nc.gpsimd.memset(eps_sb[:], EPS)
        nc.sync.dma_start(bqkv_sb[:], bqkv_d[:])
        nc.sync.dma_start(thresh_sb[:], thresh_d[:])
        id_r_t = small.tile([128, 128], F32R)
        make_identity(nc, id_bf[:])
        make_identity(nc, id_f[:])
        nc.vector.tensor_copy(id_r_t[:], id_f[:])
        id_r = id_r_t[:]

        kvq_cm = tc.tile_pool(name="kvq", bufs=1)
        kvq = kvq_cm.__enter__()
        KTb = kvq.tile([128, HT, S], BF16)            # K^T [kcol, tok]
        Vb = kvq.tile([128, 16, NH, 65], BF16)        # V token-major + ones col
        QTb = kvq.tile([128, HT, OWN], BF16)          # Q^T [qcol, own tok]

        # =========== Phase A: LN1 + transpose + QKV ===========
        def layer_norm_apply(pool, src_ap, out_ap):
            stats = pool.tile([128, 2, 6], F32, tag="st")
            nc.vector.bn_stats(stats[:, 0, :], src_ap[:, 0:512])
            nc.vector.bn_stats(stats[:, 1, :], src_ap[:, 512:1024])
            mv = pool.tile([128, 2], F32, tag="mv")
            nc.vector.bn_aggr(mv[:], stats[:])
            sd = pool.tile([128, 1], F32, tag="sd")
            nc.scalar.activation(sd[:], mv[:, 1:2], AF.Sqrt, bias=eps_sb[:])
            rstd = pool.tile([128, 1], F32, tag="rs")
            nc.vector.reciprocal(rstd[:], sd[:])
            nbias = pool.tile([128, 1], F32, tag="nb")
            nc.vector.scalar_tensor_tensor(
                nbias[:], mv[:, 0:1], -1.0, rstd[:],
                AluOpType.mult, AluOpType.mult)
            nc.scalar.activation(out_ap, src_ap, AF.Identity,
                                 bias=nbias[:], scale=rstd[:])
            return mv, rstd

        with (
            tc.tile_pool(name="xln_pool", bufs=1) as xlnp,
            tc.tile_pool(name="a_io", bufs=3) as aio,
            tc.tile_pool(name="a_pst", bufs=3, space="PSUM") as apst,
            tc.tile_pool(name="a_psk", bufs=2, space="PSUM") as apsk,
            tc.tile_pool(name="a_psv", bufs=2, space="PSUM") as apsv,
        ):
            with nc.named_scope("phaseA"):
                # xlnT: [h-part, kt, global token chunk, col]
                xlnT = xlnp.tile([128, HT, 16, 128], BF16)
                xownT = xlnp.tile([128, HT, NSLOT, 128], BF16)
                aw_cm = tc.tile_pool(name="a_w", bufs=1)
                aw = aw_cm.__enter__()
                wk = aw.tile([128, HT, H], BF16)
                wv = aw.tile([128, HT, H], BF16)
                nc.sync.dma_start(
                    wk[:], wq_d[:, H:2 * H].rearrange("(kt p) c -> p kt c", p=128))
                nc.sync.dma_start(
                    wv[:], wq_d[:, 2 * H:3 * H].rearrange("(kt p) c -> p kt c", p=128))
                for tt in range(16):
                    nc.gpsimd.memset(Vb[:, tt, :, 64:65], 1.0)

                def ln_transpose(tt_src, dst_tile, dst_tt):
                    xt = aio.tile([128, H], F32, tag="xt")
                    nc.sync.dma_start(xt[:], tt_src)
                    xl = aio.tile([128, H], BF16, tag="xl")
                    layer_norm_apply(aio, xt[:], xl[:])
                    for kt2 in range(2):
                        tp = apst.tile([128, 4, 128], BF16, tag="tp")
                        for q in range(4):
                            kt = kt2 * 4 + q
                            nc.tensor.transpose(
                                tp[:, q, :], xl[:, kt * 128:(kt + 1) * 128], id_bf[:])
                        nc.vector.tensor_copy(
                            dst_tile[:, kt2 * 4:(kt2 + 1) * 4, dst_tt, :], tp[:])

                for n in range(4):
                    for j in range(4):
                        tt = 4 * n + j
                        ln_transpose(x_kv_d[tt * 128:(tt + 1) * 128, :],
                                     xlnT, tt)
                    # K matmuls for this 512-token block
                    for kc in range(8):
                        ps = apsk.tile([128, 512], F32, tag="kps")
                        for kt in range(HT):
                            nc.tensor.matmul(
                                ps[:], wk[:, kt, kc * 128:(kc + 1) * 128],
                                xlnT[:, kt, 4 * n:4 * n + 4, :],
                                start=(kt == 0), stop=(kt == HT - 1))
                        nc.scalar.activation(
                            KTb[:, kc, n * 512:(n + 1) * 512], ps[:],
                            AF.Identity, bias=bqkv_sb[:, 8 + kc:9 + kc])
                    # V matmuls for this block (token-major)
                    for j in range(4):
                        tt = 4 * n + j
                        for vh in range(2):
                            ps = apsv.tile([128, 512], F32, tag="vps")
                            for kt in range(HT):
                                nc.tensor.matmul(
                                    ps[:], xlnT[:, kt, tt, :],
                                    wv[:, kt, vh * 512:(vh + 1) * 512],
                                    start=(kt == 0), stop=(kt == HT - 1))
                            nc.vector.tensor_copy(
                                Vb[:, tt, vh * 8:(vh + 1) * 8, 0:64],
                                ps[:].rearrange("p (h c) -> p h c", h=8))
                    # own tokens: LN + transpose for Q (overlaps K/V matmuls)
                    for so in (2 * n, 2 * n + 1):
                        ln_transpose(x_ownr_d[so * 128:(so + 1) * 128, :],
                                     xownT, so)
                aw_cm.__exit__(None, None, None)
                with tc.tile_pool(name="a_wq", bufs=2) as awq:
                    for qc in range(8):
                        wcol = awq.tile([128, HT, 128], BF16, tag="wcol")
                        nc.sync.dma_start(
                            wcol[:], wq_d[:, qc * 128:(qc + 1) * 128]
                            .rearrange("(kt p) c -> p kt c", p=128))
                        for half in range(2):
                            ps = apsk.tile([128, 512], F32, tag="kps")
                            for kt in range(HT):
                                nc.tensor.matmul(
                                    ps[:], wcol[:, kt, :],
                                    xownT[:, kt, half * 4:(half + 1) * 4, :],
                                    start=(kt == 0), stop=(kt == HT - 1))
                            nc.scalar.activation(
                                QTb[:, qc, half * 512:(half + 1) * 512], ps[:],
                                AF.Identity, bias=bqkv_sb[:, qc:qc + 1])

        # =========== Phase B: attention ===========
        attn_cm = tc.tile_pool(name="attn_p", bufs=1, side="right")
        attn_p = attn_cm.__enter__()
        attnT = attn_p.tile([128, HT, OWN], F32R)
        bmask_sb = attn_p.tile([128, 16 * 128], BF16)
        nc.sync.dma_start(bmask_sb[:], bmask_d[:])
        with (
            tc.tile_pool(name="b_p", bufs=5) as bp,
            tc.tile_pool(name="b_sc", bufs=3) as bsc,
            tc.tile_pool(name="b_ps", bufs=2, space="PSUM") as bps,
            tc.tile_pool(name="b_pv", bufs=2, space="PSUM") as bpv,
        ):
            with nc.named_scope("phaseB"):
                for h in range(NH):
                    hp, hb = h // 2, 64 * (h % 2)
                    pv0 = bpv.tile([65, 512], F32, tag="pv0")
                    pv1 = bpv.tile([65, 512], F32, tag="pv1")
                    for item in SCHED:
                        if len(item) == 1:
                            kt = item[0]
                            a = FS[kt] * 128
                            sc = bps.tile([128, 1024], F32, tag="sc")
                            stat = KTb[hb:hb + 64, hp, kt * 128:(kt + 1) * 128]
                            nc.tensor.matmul(sc[:, a:512], stat,
                                             QTb[hb:hb + 64, hp, a:512],
                                             start=True, stop=True)
                            nc.tensor.matmul(sc[:, 512:1024], stat,
                                             QTb[hb:hb + 64, hp, 512:1024],
                                             start=True, stop=True)
                            p = bp.tile([128, 1024], BF16, tag="p")
                            nc.scalar.activation(p[:, a:1024], sc[:, a:1024],
                                                 AF.Exp, scale=SCALE)
                            nc.vector.tensor_tensor(
                                p[:, a:a + 128], p[:, a:a + 128],
                                bmask_sb[:, kt * 128:(kt + 1) * 128],
                                AluOpType.mult)
                            vstat = Vb[:, kt, h, :]
                            nc.tensor.matmul(pv0[:, a:512], vstat, p[:, a:512],
                                             start=(kt == 0), stop=(kt == 7))
                            nc.tensor.matmul(pv1[:, 0:512], vstat, p[:, 512:1024],
                                             start=(kt == 0), stop=False)
                        else:
                            a = FS[item[0]] * 128
                            L = 1024 - a
                            sc = bps.tile([128, 2, 512], F32, tag="sc")
                            p = bp.tile([128, 2, 512], BF16, tag="p")
                            for jj, ktj in enumerate(item):
                                nc.tensor.matmul(
                                    sc[:, jj, 0:L],
                                    KTb[hb:hb + 64, hp, ktj * 128:(ktj + 1) * 128],
                                    QTb[hb:hb + 64, hp, a:1024],
                                    start=True, stop=True)
                            nc.scalar.activation(p[:, :, 0:L], sc[:, :, 0:L],
                                                 AF.Exp, scale=SCALE)
                            for jj, ktj in enumerate(item):
                                nc.vector.tensor_tensor(
                                    p[:, jj, 0:128], p[:, jj, 0:128],
                                    bmask_sb[:, ktj * 128:(ktj + 1) * 128],
                                    AluOpType.mult)
                            for jj, ktj in enumerate(item):
                                nc.tensor.matmul(
                                    pv1[:, a - 512:a - 512 + L],
                                    Vb[:, ktj, h, :], p[:, jj, 0:L],
                                    start=False, stop=(ktj == 15))
                    for qb, pv in ((0, pv0), (1, pv1)):
                        rd = bsc.tile([1, 512], F32, tag="rd")
                        nc.vector.reciprocal(rd[:], pv[64:65, :])
                        bc = bsc.tile([64, 512], F32, tag="bc")
                        nc.gpsimd.partition_broadcast(bc[:], rd[0:1, :])
                        nc.vector.tensor_tensor(
                            attnT[hb:hb + 64, hp, qb * 512:(qb + 1) * 512],
                            pv[0:64, :], bc[:], AluOpType.mult)

        # =========== Phase C: out-proj + residual + LN2 + y^T ===========
        kvq_cm.__exit__(None, None, None)
        hy_cm = tc.tile_pool(name="hy", bufs=1)
        hy = hy_cm.__enter__()
        h_sb = hy.tile([128, NSLOT, H], F32)
        yT_sb = hy.tile([128, HT, OWN], F32R)
        with (
            tc.tile_pool(name="c_w", bufs=2) as cw,
            tc.tile_pool(name="c_io", bufs=3) as cio,
            tc.tile_pool(name="c_ps", bufs=2, space="PSUM") as cps,
            tc.tile_pool(name="c_pst", bufs=4, space="PSUM") as cpst,
        ):
            with nc.named_scope("phaseC"):
                for n in range(2):
                    won = cw.tile([128, HT, 512], F32R, tag="won")
                    nc.sync.dma_start(
                        won[:], wout_d[:, n * 512:(n + 1) * 512]
                        .rearrange("(kt p) c -> p kt c", p=128))
                    for tt in range(NSLOT):
                        ps = cps.tile([128, 512], F32, tag="ops")
                        for kt in range(HT):
                            nc.tensor.matmul(ps[:], attnT[:, kt, tt * 128:(tt + 1) * 128],
                                             won[:, kt, :],
                                             start=(kt == 0), stop=(kt == HT - 1))
                        xo = cio.tile([128, 512], F32, tag="xo")
                        nc.sync.dma_start(
                            xo[:], x_own_d[tt * 128:(tt + 1) * 128, n * 512:(n + 1) * 512])
                        nc.vector.tensor_tensor(h_sb[:, tt, n * 512:(n + 1) * 512],
                                                ps[:], xo[:], AluOpType.add)
                for tt in range(NSLOT):
                    yt = cio.tile([128, H], F32R, tag="yt")
                    layer_norm_apply(cio, h_sb[:, tt, :], yt[:])
                    for kt in range(HT):
                        tp = cpst.tile([128, 128], F32R, tag="tp2")
                        nc.tensor.transpose(tp[:], yt[:, kt * 128:(kt + 1) * 128], id_r)
                        nc.vector.tensor_copy(yT_sb[:, kt, tt * 128:(tt + 1) * 128], tp[:])

        # =========== Phase D: router + experts ===========
        attn_cm.__exit__(None, None, None)
        h2p_cm = tc.tile_pool(name="h2p", bufs=1, side="right")
        h2p = h2p_cm.__enter__()
        h2_sb = h2p.tile([128, NSLOT, H], F32)
        rw_sb = h2p.tile([128, NSLOT, E], F32)
        rwT_cat = h2p.tile([1, E * OWN], F32)
        rwT_r = h2p.tile([8, OWN], F32R)
        with (
            tc.tile_pool(name="d_sc", bufs=2) as dsc,
            tc.tile_pool(name="d_ps", bufs=2, space="PSUM") as dps,
            tc.tile_pool(name="d_pst", bufs=2, space="PSUM") as dpst,
        ):
            with nc.named_scope("phaseD_router"):
                wr = dsc.tile([128, HT, E], F32R, tag="wr")
                nc.sync.dma_start(wr[:], wrout_d.rearrange("(kt p) e -> p kt e", p=128))
                br = dsc.tile([128, E], F32, tag="br")
                nc.sync.dma_start(br[:], brout_d[:])
                for tt in range(NSLOT):
                    ps = dps.tile([128, E], F32, tag="rps")
                    for kt in range(HT):
                        nc.tensor.matmul(ps[:], yT_sb[:, kt, tt * 128:(tt + 1) * 128],
                                         wr[:, kt, :],
                                         start=(kt == 0), stop=(kt == HT - 1))
                    lg = dsc.tile([128, E], F32, tag="lg")
                    nc.vector.tensor_tensor(lg[:], ps[:], br[:], AluOpType.add)
                    ex = dsc.tile([128, E], F32, tag="ex")
                    rsum = dsc.tile([128, 1], F32, tag="rsum")
                    nc.scalar.activation(ex[:], lg[:], AF.Exp, accum_out=rsum[:])
                    rrec = dsc.tile([128, 1], F32, tag="rrec")
                    nc.vector.reciprocal(rrec[:], rsum[:])
                    nc.vector.tensor_scalar(rw_sb[:, tt, :], ex[:], rrec[:], None,
                                            AluOpType.mult)
                    tp = dpst.tile([8, 128], F32, tag="rtp")
                    nc.tensor.transpose(tp[:], rw_sb[:, tt, :], id_f[:])
                    nc.vector.tensor_copy(rwT_r[:, tt * 128:(tt + 1) * 128], tp[:])
                for e in range(E):
                    # gather rw[:, :, e] into a partition-0 row (token order tt*128+p)
                    for tt in range(NSLOT):
                        nc.sync.dma_start(
                            rwT_cat[0:1, e * OWN + tt * 128:e * OWN + (tt + 1) * 128],
                            rw_sb[:, tt, e:e + 1])

        with (
            tc.tile_pool(name="d_y", bufs=2) as dy,
            tc.tile_pool(name="d_w", bufs=4) as dw,
            tc.tile_pool(name="d_b", bufs=1) as db,
            tc.tile_pool(name="d_moe", bufs=1, space="PSUM") as dmoe,
        ):
            with nc.named_scope("phaseD_experts"):
                bexp_sb = db.tile([8, H], F32R)
                nc.sync.dma_start(bexp_sb[:], bexp_d[:])
                for fh in range(2):
                    mps = [dmoe.tile([128, 512], F32, tag=f"mps{tt}",
                                     name=f"mps_{fh}_{tt}")
                           for tt in range(NSLOT)]
                    for e in range(E):
                        bcr = dy.tile([128, OWN], F32, tag="bcr")
                        nc.gpsimd.partition_broadcast(bcr[:], rwT_cat[0:1, e * OWN:(e + 1) * OWN])
                        for kt in range(HT):
                            ye = dy.tile([128, OWN], F32R, tag="ye", bufs=4)
                            nc.vector.tensor_tensor(ye[:], yT_sb[:, kt, :],
                                                    bcr[:], AluOpType.mult)
                            we = dw.tile([128, 512], F32R, tag="we")
                            nc.sync.dma_start(
                                we[:],
                                wexp_d[e, kt * 128:(kt + 1) * 128,
                                       fh * 512:(fh + 1) * 512])
                            for tt in range(NSLOT):
                                nc.tensor.matmul(
                                    mps[tt][:], ye[:, tt * 128:(tt + 1) * 128],
                                    we[:],
                                    start=(e == 0 and kt == 0), stop=False)
                    for tt in range(NSLOT):
                        nc.tensor.matmul(mps[tt][:],
                                         rwT_r[:, tt * 128:(tt + 1) * 128],
                                         bexp_sb[:, fh * 512:(fh + 1) * 512],
                                         start=False, stop=True)
                        nc.vector.tensor_tensor(
                            h2_sb[:, tt, fh * 512:(fh + 1) * 512], mps[tt][:],
                            h_sb[:, tt, fh * 512:(fh + 1) * 512], AluOpType.add)

        # =========== Phase E: gate + LNf + output ===========
        hy_cm.__exit__(None, None, None)
        with (
            tc.tile_pool(name="e_sc", bufs=1) as esc,
            tc.tile_pool(name="e_tmp", bufs=2) as etmp,
            tc.tile_pool(name="e_pst", bufs=3, space="PSUM") as epst,
            tc.tile_pool(name="e_psg", bufs=2, space="PSUM") as epsg,
            tc.tile_pool(name="e_psal", bufs=1, space="PSUM") as epsal,
            tc.tile_pool(name="e_out", bufs=3) as eout,
        ):
            with nc.named_scope("phaseE"):
                mask_pp = esc.tile([128, NSLOT], F32)
                maskrow = esc.tile([1, OWN], F32)
                h2T = esc.tile([128, HT, OWN], F32R)
                for tt in range(NSLOT):
                    for kt in range(HT):
                        tp = epst.tile([128, 128], F32, tag="tp3")
                        nc.tensor.transpose(
                            tp[:], h2_sb[:, tt, kt * 128:(kt + 1) * 128], id_f[:])
                        nc.vector.tensor_copy(h2T[:, kt, tt * 128:(tt + 1) * 128], tp[:])
                wal1 = esc.tile([128, HT, 256], F32R)
                nc.sync.dma_start(wal1[:], wal1_d.rearrange("(kt p) c -> p kt c", p=128))
                bal1 = esc.tile([128, 2], F32)
                nc.sync.dma_start(bal1[:], bal1_d[:])
                wal2 = esc.tile([128, 2, 1], F32R)
                nc.sync.dma_start(wal2[:], wal2_d.rearrange("(m p) c -> p m c", p=128))
                gT = esc.tile([128, 2, OWN], F32R)
                for m2 in range(2):
                    for n in range(2):
                        ps = epsg.tile([128, 512], F32, tag="gps")
                        for kt in range(HT):
                            nc.tensor.matmul(ps[:],
                                             wal1[:, kt, m2 * 128:(m2 + 1) * 128],
                                             h2T[:, kt, n * 512:(n + 1) * 512],
                                             start=(kt == 0), stop=(kt == HT - 1))
                        nc.scalar.activation(gT[:, m2, n * 512:(n + 1) * 512], ps[:],
                                             AF.Gelu, bias=bal1[:, m2:m2 + 1])
                for n in range(2):
                    ps = epsal.tile([1, 512], F32, tag="alps")
                    for m2 in range(2):
                        nc.tensor.matmul(ps[:], wal2[:, m2, :],
                                         gT[:, m2, n * 512:(n + 1) * 512],
                                         start=(m2 == 0), stop=(m2 == 1))
                    nc.vector.tensor_scalar(maskrow[:, n * 512:(n + 1) * 512], ps[:],
                                            thresh_sb[0:1, 0:1], None, AluOpType.is_gt)
                for tt in range(NSLOT):
                    nc.sync.dma_start(
                        mask_pp[:, tt:tt + 1],
                        maskrow[0:1, tt * 128:(tt + 1) * 128])
                for tt in range(NSLOT):
                    stats = eout.tile([128, 2, 6], F32, tag="st3")
                    nc.vector.bn_stats(stats[:, 0, :], h2_sb[:, tt, 0:512])
                    nc.vector.bn_stats(stats[:, 1, :], h2_sb[:, tt, 512:1024])
                    mv = eout.tile([128, 2], F32, tag="mv3")
                    nc.vector.bn_aggr(mv[:], stats[:])
                    sd = eout.tile([128, 1], F32, tag="sd3")
                    nc.scalar.activation(sd[:], mv[:, 1:2], AF.Sqrt, bias=eps_sb[:])
                    rstd = eout.tile([128, 1], F32, tag="rs3")
                    nc.vector.reciprocal(rstd[:], sd[:])
                    seff = eout.tile([128, 1], F32, tag="se3")
                    nc.vector.tensor_tensor(seff[:], rstd[:], mask_pp[:, tt:tt + 1],
                                            AluOpType.mult)
                    beff = eout.tile([128, 1], F32, tag="be3")
                    nc.vector.scalar_tensor_tensor(beff[:], mv[:, 0:1], -1.0, seff[:],
                                                   AluOpType.mult, AluOpType.mult)
                    ot = eout.tile([128, H], F32, tag="ot")
                    nc.scalar.activation(ot[:], h2_sb[:, tt, :], AF.Identity,
                                         bias=beff[:], scale=seff[:])
                    nc.sync.dma_start(out_d[tt * 128:(tt + 1) * 128, :], ot[:])

        h2p_cm.__exit__(None, None, None)
        small_cm.__exit__(None, None, None)

    nc.compile()
    return nc


def _prep_host(inputs):
    f32 = np.float32
    bf16 = ml_dtypes.bfloat16
    x = np.asarray(inputs["inputs"], f32)
    ln1_g = np.asarray(inputs["ln1_g"], f32); ln1_b = np.asarray(inputs["ln1_b"], f32)
    w_qkv = np.asarray(inputs["w_qkv"], f32); b_qkv = np.asarray(inputs["b_qkv"], f32)
    w_out = np.asarray(inputs["w_out"], f32); b_out = np.asarray(inputs["b_out"], f32)
    ln2_g = np.asarray(inputs["ln2_g"], f32); ln2_b = np.asarray(inputs["ln2_b"], f32)
    w_router = np.asarray(inputs["w_router"], f32)
    b_router = np.asarray(inputs["b_router"], f32)
    w_exp = np.asarray(inputs["w_exp"], f32); b_exp = np.asarray(inputs["b_exp"], f32)
    w_al1 = np.asarray(inputs["w_al1"], f32); b_al1 = np.asarray(inputs["b_al1"], f32)
    w_al2 = np.asarray(inputs["w_al2"], f32); b_al2 = np.asarray(inputs["b_al2"], f32)

    wq_f = (ln1_g[:, None] * w_qkv).astype(bf16)
    bq_f = b_qkv + ln1_b @ w_qkv
    assert np.all(bq_f[2 * H:] == 0.0), "nonzero V bias not supported"
    bqkv_t = np.zeros((128, 16), f32)
    for j in range(16):
        bqkv_t[:, j] = bq_f[j * 128:(j + 1) * 128]
    wr_f = ln2_g[:, None] * w_router
    br_f = b_router + ln2_b @ w_router
    we_f = ln2_g[None, :, None] * w_exp
    be_f = b_exp + np.einsum("h,ehf->ef", ln2_b, w_exp)
    x_pb = x + b_out[None, None, :]

    def mk_bmask(par):
        # first-suffix-block mask per key chunk kt: key j (row), query i (col)
        # within the block; slot s0 = kt//2 has global query chunk c = 2*s0+par.
        m = np.zeros((128, 16, 128), f32)
        for kt in range(16):
            c = 2 * (kt // 2) + par
            if c == kt:
                m[:, kt, :] = (np.arange(128)[:, None] <= np.arange(128)[None, :])
            elif c > kt:
                m[:, kt, :] = 1.0
            # c < kt: fully masked -> zeros
        return m.reshape(128, 16 * 128).astype(bf16)
    bmasks = [mk_bmask(0), mk_bmask(1)]

    thresh = np.full((128, 1), 0.8 - float(b_al2[0]), f32)
    bal1_t = np.zeros((128, 2), f32)
    bal1_t[:, 0] = b_al1[0:128]
    bal1_t[:, 1] = b_al1[128:256]

    shared = dict(
        wq=np.ascontiguousarray(wq_f),
        bqkv=bqkv_t,
        wout=np.ascontiguousarray(w_out),
        wrout=np.ascontiguousarray(wr_f.astype(f32)),
        brout=np.tile(br_f[None, :], (128, 1)).astype(f32),
        wexp=np.ascontiguousarray(we_f.astype(f32)),
        bexp=np.ascontiguousarray(be_f.astype(f32)),
        wal1=np.ascontiguousarray(w_al1),
        bal1=bal1_t,
        wal2=np.ascontiguousarray(w_al2),
        thresh=thresh,
    )
    per_core = []
    for c in range(N_CORES):
        b, par = c // 2, c % 2
        own_idx = np.concatenate(
            [np.arange(128) + (2 * s + par) * 128 for s in range(NSLOT)])
        m = dict(shared)
        m["x_kv"] = np.ascontiguousarray(x[b])
        m["x_ownr"] = np.ascontiguousarray(x[b][own_idx])
        m["x_own"] = np.ascontiguousarray(x_pb[b][own_idx])
        m["bmask"] = bmasks[par]
        per_core.append(m)
    return per_core


def kernel(**inputs):
    from concourse.bass_utils import run_bass_kernel_spmd

    if "prog" not in _prog_cache:
        _prog_cache["prog"] = _build_program()
    nc = _prog_cache["prog"]

    per_core = _prep_host(inputs)
    trace = bool(globals().get("TRACE", False))
    res = run_bass_kernel_spmd(nc, per_core, core_ids=list(range(N_CORES)),
                               trace=trace)
    _prog_cache["last_result"] = res

    lnf_g = np.asarray(inputs["lnf_g"], np.float32)
    lnf_b = np.asarray(inputs["lnf_b"], np.float32)
    out = np.zeros((B, S, H), np.float32)
    for c in range(N_CORES):
        b, par = c // 2, c % 2
        o = res.results[c]["out"]
        for s in range(NSLOT):
            g0 = (2 * s + par) * 128
            out[b, g0:g0 + 128, :] = o[s * 128:(s + 1) * 128, :]
    return out * lnf_g[None, None, :] + lnf_b[None, None, :]


# revision 15
# speedup vs baseline: 1.1956x; 1.0321x over previous
"""Trainium2 Bass kernel for nn_EnhancedTransformerBlock_80169859548047.

Sharding: 8 cores = (batch b, parity par). Core c handles batch b=c//2 and the
even (par=0) or odd (par=1) 128-token chunks of that batch's 2048-token
sequence. The attention schedule is parity-uniform: for key chunk kt the query
suffix starts at slot fs(kt)=kt//2; the first suffix block's causal mask
(triangular / all-ones / all-zeros, depending on parity) is host data, so the
instruction stream is identical on all cores.

Dtypes: attention path (LN1 out, w_qkv, Q/K/V, P) in bf16; out-proj, experts,
router and gate matmuls in float32r; residual stream and LN math in fp32.
Softmax denominators come from a ones column appended per head to V (exact
PSUM accumulation). LN1/LN2 affines are folded into the following weights on
the host; the final LN affine is applied on the host after gathering.
"""

import numpy as np
import ml_dtypes

B, S, H, E, NH, HD = 4, 2048, 1024, 8, 16, 64
N_CORES = 8
EPS = 1e-12
SCALE = HD ** -0.5
NSLOT = 8                # 128-token chunks per core
OWN = NSLOT * 128        # own tokens per core
HT = H // 128            # 8 H-tiles
FS = [kt // 2 for kt in range(16)]          # suffix first slot per key chunk
# attention schedule: key chunks 0..7 processed singly (suffix > 512 wide),
# 8..15 in pairs (same fs, suffix <= 512)
SCHED = [(0,), (1,), (2,), (3,), (4,), (5,), (6,), (7,),
         (8, 9), (10, 11), (12, 13), (14, 15)]

_prog_cache = {}


def _build_program():
    import concourse.bacc as bacc
    import concourse.tile as tile
    import concourse.mybir as mybir
    from concourse.masks import make_identity
    from concourse.alu_op_type import AluOpType
    from contextlib import ExitStack

    F32 = mybir.dt.float32
    F32R = mybir.dt.float32r
    BF16 = mybir.dt.bfloat16
    AF = mybir.ActivationFunctionType

    nc = bacc.Bacc("TRN2", target_bir_lowering=False, debug=False, num_devices=1)

    def din(name, shape, dt):
        return nc.dram_tensor(name, list(shape), dt, kind="ExternalInput").ap()

    x_kv_d = din("x_kv", (S, H), F32)
    x_ownr_d = din("x_ownr", (OWN, H), F32)   # raw inputs, own tokens, slot order
    x_own_d = din("x_own", (OWN, H), F32)     # inputs + b_out, own tokens
    wq_d = din("wq", (H, 3 * H), BF16)
    bqkv_d = din("bqkv", (128, 16), F32)
    wout_d = din("wout", (H, H), F32R)
    wrout_d = din("wrout", (H, E), F32R)
    brout_d = din("brout", (128, E), F32)
    wexp_d = din("wexp", (E, H, H), F32R)
    bexp_d = din("bexp", (E, H), F32R)
    wal1_d = din("wal1", (H, 256), F32R)
    bal1_d = din("bal1", (128, 2), F32)
    wal2_d = din("wal2", (256, 1), F32R)
    bmask_d = din("bmask", (128, 16 * 128), BF16)  # per-kt first-block masks
    thresh_d = din("thresh", (128, 1), F32)   # 0.8 - b_al2, replicated
    out_d = nc.dram_tensor("out", [OWN, H], F32, kind="ExternalOutput").ap()

    with tile.TileContext(nc) as tc, ExitStack() as st:
        # manually-managed pools (non-LIFO lifetimes)
        small_cm = tc.tile_pool(name="small", bufs=1)
        small = small_cm.__enter__()
        id_bf = small.tile([128, 128], BF16)
        id_f = small.tile([128, 128], F32)
        bqkv_sb = small.tile([128, 16], F32)
        thresh_sb = small.tile([128, 1], F32)
        eps_sb = small.tile([128, 1], F32)
        nc.gpsimd.memset(eps_sb[:], EPS)
        nc.sync.dma_start(bqkv_sb[:], bqkv_d[:])
        nc.sync.dma_start(thresh_sb[:], thresh_d[:])
        id_r_t = small.tile([128, 128], F32R)
        make_identity(nc, id_bf[:])
        make_identity(nc, id_f[:])
        nc.vector.tensor_copy(id_r_t[:], id_f[:])
        id_r = id_r_t[:]

        kvq_cm = tc.tile_pool(name="kvq", bufs=1)
        kvq = kvq_cm.__enter__()
        KTb = kvq.tile([128, HT, S], BF16)            # K^T [kcol, tok]
        Vb = kvq.tile([128, 16, NH, 65], BF16)        # V token-major + ones col
        QTb = kvq.tile([128, HT, OWN], BF16)          # Q^T [qcol, own tok]

        # =========== Phase A: LN1 + transpose + QKV ===========
        def layer_norm_apply(pool, src_ap, out_ap):
            stats = pool.tile([128, 2, 6], F32, tag="st")
            nc.vector.bn_stats(stats[:, 0, :], src_ap[:, 0:512])
            nc.vector.bn_stats(stats[:, 1, :], src_ap[:, 512:1024])
            mv = pool.tile([128, 2], F32, tag="mv")
            nc.vector.bn_aggr(mv[:], stats[:])
            sd = pool.tile([128, 1], F32, tag="sd")
            nc.scalar.activation(sd[:], mv[:, 1:2], AF.Sqrt, bias=eps_sb[:])
            rstd = pool.tile([128, 1], F32, tag="rs")
            nc.vector.reciprocal(rstd[:], sd[:])
            nbias = pool.tile([128, 1], F32, tag="nb")
            nc.vector.scalar_tensor_tensor(
                nbias[:], mv[:, 0:1], -1.0, rstd[:],
                AluOpType.mult, AluOpType.mult)
            nc.scalar.activation(out_ap, src_ap, AF.Identity,
                                 bias=nbias[:], scale=rstd[:])
            return mv, rstd

        with (
            tc.tile_pool(name="xln_pool", bufs=1) as xlnp,
            tc.tile_pool(name="a_io", bufs=3) as aio,
            tc.tile_pool(name="a_pst", bufs=3, space="PSUM") as apst,
            tc.tile_pool(name="a_psk", bufs=2, space="PSUM") as apsk,
            tc.tile_pool(name="a_psv", bufs=2, space="PSUM") as apsv,
        ):
            with nc.named_scope("phaseA"):
                # xlnT: [h-part, kt, global token chunk, col]
                xlnT = xlnp.tile([128, HT, 16, 128], BF16)
                xownT = xlnp.tile([128, HT, NSLOT, 128], BF16)
                aw_cm = tc.tile_pool(name="a_w", bufs=1)
                aw = aw_cm.__enter__()
                wk = aw.tile([128, HT, H], BF16)
                wv = aw.tile([128, HT, H], BF16)
                nc.sync.dma_start(
                    wk[:], wq_d[:, H:2 * H].rearrange("(kt p) c -> p kt c", p=128))
                nc.sync.dma_start(
                    wv[:], wq_d[:, 2 * H:3 * H].rearrange("(kt p) c -> p kt c", p=128))
                for tt in range(16):
                    nc.gpsimd.memset(Vb[:, tt, :, 64:65], 1.0)

                def ln_transpose(tt_src, dst_tile, dst_tt):
                    xt = aio.tile([128, H], F32, tag="xt")
                    nc.sync.dma_start(xt[:], tt_src)
                    xl = aio.tile([128, H], BF16, tag="xl")
                    layer_norm_apply(aio, xt[:], xl[:])
                    for kt2 in range(2):
                        tp = apst.tile([128, 4, 128], BF16, tag="tp")
                        for q in range(4):
                            kt = kt2 * 4 + q
                            nc.tensor.transpose(
                                tp[:, q, :], xl[:, kt * 128:(kt + 1) * 128], id_bf[:])
                        nc.vector.tensor_copy(
                            dst_tile[:, kt2 * 4:(kt2 + 1) * 4, dst_tt, :], tp[:])

                for n in range(4):
                    for j in range(4):
                        tt = 4 * n + j
                        ln_transpose(x_kv_d[tt * 128:(tt + 1) * 128, :],
                                     xlnT, tt)
                    # K matmuls for this 512-token block
                    for kc in range(8):
                        ps = apsk.tile([128, 512], F32, tag="kps")
                        for kt in range(HT):
                            nc.tensor.matmul(
                                ps[:], wk[:, kt, kc * 128:(kc + 1) * 128],
                                xlnT[:, kt, 4 * n:4 * n + 4, :],
                                start=(kt == 0), stop=(kt == HT - 1))
                        nc.scalar.activation(
                            KTb[:, kc, n * 512:(n + 1) * 512], ps[:],
                            AF.Identity, bias=bqkv_sb[:, 8 + kc:9 + kc])
                    # V matmuls for this block (token-major)
                    for j in range(4):
                        tt = 4 * n + j
                        for vh in range(2):
                            ps = apsv.tile([128, 512], F32, tag="vps")
                            for kt in range(HT):
                                nc.tensor.matmul(
                                    ps[:], xlnT[:, kt, tt, :],
                                    wv[:, kt, vh * 512:(vh + 1) * 512],
                                    start=(kt == 0), stop=(kt == HT - 1))
                            nc.vector.tensor_copy(
                                Vb[:, tt, vh * 8:(vh + 1) * 8, 0:64],
                                ps[:].rearrange("p (h c) -> p h c", h=8))
                    # own tokens: LN + transpose for Q (overlaps K/V matmuls)
                    for so in (2 * n, 2 * n + 1):
                        ln_transpose(x_ownr_d[so * 128:(so + 1) * 128, :],
                                     xownT, so)
                aw_cm.__exit__(None, None, None)
                with tc.tile_pool(name="a_wq", bufs=2) as awq:
                    for qc in range(8):
                        wcol = awq.tile([128, HT, 128], BF16, tag="wcol")
                        nc.sync.dma_start(
                            wcol[:], wq_d[:, qc * 128:(qc + 1) * 128]
                            .rearrange("(kt p) c -> p kt c", p=128))
                        for half in range(2):
                            ps = apsk.tile([128, 512], F32, tag="kps")
                            for kt in range(HT):
                                nc.tensor.matmul(
                                    ps[:], wcol[:, kt, :],
                                    xownT[:, kt, half * 4:(half + 1) * 4, :],
                                    start=(kt == 0), stop=(kt == HT - 1))
                            nc.scalar.activation(
                                QTb[:, qc, half * 512:(half + 1) * 512], ps[:],
                                AF.Identity, bias=bqkv_sb[:, qc:qc + 1])

        # =========== Phase B: attention ===========
        attn_cm = tc.tile_pool(name="attn_p", bufs=1, side="right")
        attn_p = attn_cm.__enter__()
        attnT = attn_p.tile([128, HT, OWN], F32R)
        bmask_sb = attn_p.tile([128, 16 * 128], BF16)
        nc.sync.dma_start(bmask_sb[:], bmask_d[:])
        with (
            tc.tile_pool(name="b_p", bufs=5) as bp,
            tc.tile_pool(name="b_sc", bufs=3) as bsc,
            tc.tile_pool(name="b_ps", bufs=2, space="PSUM") as bps,
            tc.tile_pool(name="b_pv", bufs=2, space="PSUM") as bpv,
        ):
            with nc.named_scope("phaseB"):
                # Software-pipelined: scores/exp/mask of unit j+1 are emitted
                # before the PV matmuls of unit j so the in-order PE never
                # waits on the Act-engine exp of the unit it is consuming.
                units = [(h, item) for h in range(NH) for item in SCHED]
                pvs = {}   # h -> (pv0, pv1)

                def emit_scores(h, item):
                    hp, hb = h // 2, 64 * (h % 2)
                    if len(item) == 1:
                        kt = item[0]
                        a = FS[kt] * 128
                        sc = bps.tile([128, 1024], F32, tag="sc")
                        stat = KTb[hb:hb + 64, hp, kt * 128:(kt + 1) * 128]
                        nc.tensor.matmul(sc[:, a:512], stat,
                                         QTb[hb:hb + 64, hp, a:512],
                                         start=True, stop=True)
                        nc.tensor.matmul(sc[:, 512:1024], stat,
                                         QTb[hb:hb + 64, hp, 512:1024],
                                         start=True, stop=True)
                        p = bp.tile([128, 1024], BF16, tag="p")
                        nc.scalar.activation(p[:, a:1024], sc[:, a:1024],
                                             AF.Exp, scale=SCALE)
                        nc.vector.tensor_tensor(
                            p[:, a:a + 128], p[:, a:a + 128],
                            bmask_sb[:, kt * 128:(kt + 1) * 128],
                            AluOpType.mult)
                    else:
                        a = FS[item[0]] * 128
                        L = 1024 - a
                        sc = bps.tile([128, 2, 512], F32, tag="sc")
                        p = bp.tile([128, 2, 512], BF16, tag="p")
                        for jj, ktj in enumerate(item):
                            nc.tensor.matmul(
                                sc[:, jj, 0:L],
                                KTb[hb:hb + 64, hp, ktj * 128:(ktj + 1) * 128],
                                QTb[hb:hb + 64, hp, a:1024],
                                start=True, stop=True)
                        nc.scalar.activation(p[:, :, 0:L], sc[:, :, 0:L],
                                             AF.Exp, scale=SCALE)
                        for jj, ktj in enumerate(item):
                            nc.vector.tensor_tensor(
                                p[:, jj, 0:128], p[:, jj, 0:128],
                                bmask_sb[:, ktj * 128:(ktj + 1) * 128],
                                AluOpType.mult)
                    return p

                def emit_pv(h, item, p):
                    if h not in pvs:
                        pvs[h] = (bpv.tile([65, 512], F32, tag="pv0",
                                           name=f"pv0_{h}"),
                                  bpv.tile([65, 512], F32, tag="pv1",
                                           name=f"pv1_{h}"))
                    pv0, pv1 = pvs[h]
                    if len(item) == 1:
                        kt = item[0]
                        a = FS[kt] * 128
                        vstat = Vb[:, kt, h, :]
                        nc.tensor.matmul(pv0[:, a:512], vstat, p[:, a:512],
                                         start=(kt == 0), stop=(kt == 7))
                        nc.tensor.matmul(pv1[:, 0:512], vstat, p[:, 512:1024],
                                         start=(kt == 0), stop=False)
                    else:
                        a = FS[item[0]] * 128
                        L = 1024 - a
                        for jj, ktj in enumerate(item):
                            nc.tensor.matmul(
                                pv1[:, a - 512:a - 512 + L],
                                Vb[:, ktj, h, :], p[:, jj, 0:L],
                                start=False, stop=(ktj == 15))

                def emit_norm(h):
                    hp, hb = h // 2, 64 * (h % 2)
                    for qb, pv in ((0, pvs[h][0]), (1, pvs[h][1])):
                        rd = bsc.tile([1, 512], F32, tag="rd")
                        nc.vector.reciprocal(rd[:], pv[64:65, :])
                        bc = bsc.tile([64, 512], F32, tag="bc")
                        nc.gpsimd.partition_broadcast(bc[:], rd[0:1, :])
                        nc.vector.tensor_tensor(
                            attnT[hb:hb + 64, hp, qb * 512:(qb + 1) * 512],
                            pv[0:64, :], bc[:], AluOpType.mult)

                prev = None
                for unit in units:
                    p = emit_scores(*unit)
                    if prev is not None:
                        emit_pv(*prev)
                        if prev[1] is SCHED[-1]:
                            emit_norm(prev[0])
                    prev = (unit[0], unit[1], p)
                emit_pv(*prev)
                emit_norm(prev[0])

        # =========== Phase C: out-proj + residual + LN2 + y^T ===========
        kvq_cm.__exit__(None, None, None)
        hy_cm = tc.tile_pool(name="hy", bufs=1)
        hy = hy_cm.__enter__()
        h_sb = hy.tile([128, NSLOT, H], F32)
        yT_sb = hy.tile([128, HT, OWN], F32R)
        with (
            tc.tile_pool(name="c_w", bufs=2) as cw,
            tc.tile_pool(name="c_io", bufs=3) as cio,
            tc.tile_pool(name="c_ps", bufs=2, space="PSUM") as cps,
            tc.tile_pool(name="c_pst", bufs=4, space="PSUM") as cpst,
        ):
            with nc.named_scope("phaseC"):
                def ln2_transpose(tt):
                    yt = cio.tile([128, H], F32R, tag="yt")
                    layer_norm_apply(cio, h_sb[:, tt, :], yt[:])
                    for kt2 in range(2):
                        tp = cpst.tile([128, 4, 128], F32R, tag="tp2")
                        for q in range(4):
                            kt = kt2 * 4 + q
                            nc.tensor.transpose(
                                tp[:, q, :], yt[:, kt * 128:(kt + 1) * 128], id_r)
                        nc.vector.tensor_copy(
                            yT_sb[:, kt2 * 4:(kt2 + 1) * 4, tt * 128:(tt + 1) * 128],
                            tp[:])

                for n in range(2):
                    won = cw.tile([128, HT, 512], F32R, tag="won")
                    nc.sync.dma_start(
                        won[:], wout_d[:, n * 512:(n + 1) * 512]
                        .rearrange("(kt p) c -> p kt c", p=128))
                    for tt in range(NSLOT):
                        ps = cps.tile([128, 512], F32, tag="ops")
                        for kt in range(HT):
                            nc.tensor.matmul(ps[:], attnT[:, kt, tt * 128:(tt + 1) * 128],
                                             won[:, kt, :],
                                             start=(kt == 0), stop=(kt == HT - 1))
                        xo = cio.tile([128, 512], F32, tag="xo")
                        nc.sync.dma_start(
                            xo[:], x_own_d[tt * 128:(tt + 1) * 128, n * 512:(n + 1) * 512])
                        nc.vector.tensor_tensor(h_sb[:, tt, n * 512:(n + 1) * 512],
                                                ps[:], xo[:], AluOpType.add)
                        if n == 1 and tt >= 1:
                            ln2_transpose(tt - 1)
                ln2_transpose(NSLOT - 1)

        # =========== Phase D: router + experts ===========
        attn_cm.__exit__(None, None, None)
        h2p_cm = tc.tile_pool(name="h2p", bufs=1, side="right")
        h2p = h2p_cm.__enter__()
        h2_sb = h2p.tile([128, NSLOT, H], F32)
        rw_sb = h2p.tile([128, NSLOT, E], F32)
        rwT_cat = h2p.tile([1, E * OWN], F32)
        rwT_r = h2p.tile([8, OWN], F32R)
        with (
            tc.tile_pool(name="d_sc", bufs=2) as dsc,
            tc.tile_pool(name="d_ps", bufs=2, space="PSUM") as dps,
            tc.tile_pool(name="d_pst", bufs=2, space="PSUM") as dpst,
        ):
            with nc.named_scope("phaseD_router"):
                wr = dsc.tile([128, HT, E], F32R, tag="wr")
                nc.sync.dma_start(wr[:], wrout_d.rearrange("(kt p) e -> p kt e", p=128))
                br = dsc.tile([128, E], F32, tag="br")
                nc.sync.dma_start(br[:], brout_d[:])
                for tt in range(NSLOT):
                    ps = dps.tile([128, E], F32, tag="rps")
                    for kt in range(HT):
                        nc.tensor.matmul(ps[:], yT_sb[:, kt, tt * 128:(tt + 1) * 128],
                                         wr[:, kt, :],
                                         start=(kt == 0), stop=(kt == HT - 1))
                    lg = dsc.tile([128, E], F32, tag="lg")
                    nc.vector.tensor_tensor(lg[:], ps[:], br[:], AluOpType.add)
                    ex = dsc.tile([128, E], F32, tag="ex")
                    rsum = dsc.tile([128, 1], F32, tag="rsum")
                    nc.scalar.activation(ex[:], lg[:], AF.Exp, accum_out=rsum[:])
                    rrec = dsc.tile([128, 1], F32, tag="rrec")
                    nc.vector.reciprocal(rrec[:], rsum[:])
                    nc.vector.tensor_scalar(rw_sb[:, tt, :], ex[:], rrec[:], None,
                                            AluOpType.mult)
                    tp = dpst.tile([8, 128], F32, tag="rtp")
                    nc.tensor.transpose(tp[:], rw_sb[:, tt, :], id_f[:])
                    nc.vector.tensor_copy(rwT_r[:, tt * 128:(tt + 1) * 128], tp[:])
                for e in range(E):
                    # gather rw[:, :, e] into a partition-0 row (token order tt*128+p)
                    for tt in range(NSLOT):
                        nc.sync.dma_start(
                            rwT_cat[0:1, e * OWN + tt * 128:e * OWN + (tt + 1) * 128],
                            rw_sb[:, tt, e:e + 1])

        with (
            tc.tile_pool(name="d_y", bufs=2) as dy,
            tc.tile_pool(name="d_w", bufs=4) as dw,
            tc.tile_pool(name="d_b", bufs=1) as db,
            tc.tile_pool(name="d_moe", bufs=1, space="PSUM") as dmoe,
        ):
            with nc.named_scope("phaseD_experts"):
                bexp_sb = db.tile([8, H], F32R)
                nc.sync.dma_start(bexp_sb[:], bexp_d[:])
                for fh in range(2):
                    mps = [dmoe.tile([128, 512], F32, tag=f"mps{tt}",
                                     name=f"mps_{fh}_{tt}")
                           for tt in range(NSLOT)]
                    for e in range(E):
                        bcr = dy.tile([128, OWN], F32, tag="bcr")
                        nc.gpsimd.partition_broadcast(bcr[:], rwT_cat[0:1, e * OWN:(e + 1) * OWN])
                        for kt in range(HT):
                            ye = dy.tile([128, OWN], F32R, tag="ye", bufs=4)
                            nc.vector.tensor_tensor(ye[:], yT_sb[:, kt, :],
                                                    bcr[:], AluOpType.mult)
                            we = dw.tile([128, 512], F32R, tag="we")
                            nc.sync.dma_start(
                                we[:],
                                wexp_d[e, kt * 128:(kt + 1) * 128,
                                       fh * 512:(fh + 1) * 512])
                            for tt in range(NSLOT):
                                nc.tensor.matmul(
                                    mps[tt][:], ye[:, tt * 128:(tt + 1) * 128],
                                    we[:],
                                    start=(e == 0 and kt == 0), stop=False)
                    for tt in range(NSLOT):
                        nc.tensor.matmul(mps[tt][:],
                                         rwT_r[:, tt * 128:(tt + 1) * 128],
                                         bexp_sb[:, fh * 512:(fh + 1) * 512],
                                         start=False, stop=True)
                        nc.vector.tensor_tensor(
                            h2_sb[:, tt, fh * 512:(fh + 1) * 512], mps[tt][:],
                            h_sb[:, tt, fh * 512:(fh + 1) * 512], AluOpType.add)

        # =========== Phase E: gate + LNf + output ===========
        hy_cm.__exit__(None, None, None)
        with (
            tc.tile_pool(name="e_sc", bufs=1) as esc,
            tc.tile_pool(name="e_tmp", bufs=2) as etmp,
            tc.tile_pool(name="e_pst", bufs=3, space="PSUM") as epst,
            tc.tile_pool(name="e_psg", bufs=2, space="PSUM") as epsg,
            tc.tile_pool(name="e_psal", bufs=1, space="PSUM") as epsal,
            tc.tile_pool(name="e_out", bufs=3) as eout,
        ):
            with nc.named_scope("phaseE"):
                mask_pp = esc.tile([128, NSLOT], F32)
                maskrow = esc.tile([1, OWN], F32)
                h2T = esc.tile([128, HT, OWN], F32R)
                for tt in range(NSLOT):
                    for kt2 in range(2):
                        tp = epst.tile([128, 4, 128], F32, tag="tp3")
                        for q in range(4):
                            kt = kt2 * 4 + q
                            nc.tensor.transpose(
                                tp[:, q, :], h2_sb[:, tt, kt * 128:(kt + 1) * 128],
                                id_f[:])
                        nc.vector.tensor_copy(
                            h2T[:, kt2 * 4:(kt2 + 1) * 4, tt * 128:(tt + 1) * 128],
                            tp[:])
                wal1 = esc.tile([128, HT, 256], F32R)
                nc.sync.dma_start(wal1[:], wal1_d.rearrange("(kt p) c -> p kt c", p=128))
                bal1 = esc.tile([128, 2], F32)
                nc.sync.dma_start(bal1[:], bal1_d[:])
                wal2 = esc.tile([128, 2, 1], F32R)
                nc.sync.dma_start(wal2[:], wal2_d.rearrange("(m p) c -> p m c", p=128))
                gT = esc.tile([128, 2, OWN], F32R)
                for m2 in range(2):
                    for n in range(2):
                        ps = epsg.tile([128, 512], F32, tag="gps")
                        for kt in range(HT):
                            nc.tensor.matmul(ps[:],
                                             wal1[:, kt, m2 * 128:(m2 + 1) * 128],
                                             h2T[:, kt, n * 512:(n + 1) * 512],
                                             start=(kt == 0), stop=(kt == HT - 1))
                        nc.scalar.activation(gT[:, m2, n * 512:(n + 1) * 512], ps[:],
                                             AF.Gelu, bias=bal1[:, m2:m2 + 1])
                for n in range(2):
                    ps = epsal.tile([1, 512], F32, tag="alps")
                    for m2 in range(2):
                        nc.tensor.matmul(ps[:], wal2[:, m2, :],
                                         gT[:, m2, n * 512:(n + 1) * 512],
                                         start=(m2 == 0), stop=(m2 == 1))
                    nc.vector.tensor_scalar(maskrow[:, n * 512:(n + 1) * 512], ps[:],
                                            thresh_sb[0:1, 0:1], None, AluOpType.is_gt)
                for tt in range(NSLOT):
                    nc.sync.dma_start(
                        mask_pp[:, tt:tt + 1],
                        maskrow[0:1, tt * 128:(tt + 1) * 128])
                for tt in range(NSLOT):
                    stats = eout.tile([128, 2, 6], F32, tag="st3")
                    nc.vector.bn_stats(stats[:, 0, :], h2_sb[:, tt, 0:512])
                    nc.vector.bn_stats(stats[:, 1, :], h2_sb[:, tt, 512:1024])
                    mv = eout.tile([128, 2], F32, tag="mv3")
                    nc.vector.bn_aggr(mv[:], stats[:])
                    sd = eout.tile([128, 1], F32, tag="sd3")
                    nc.scalar.activation(sd[:], mv[:, 1:2], AF.Sqrt, bias=eps_sb[:])
                    rstd = eout.tile([128, 1], F32, tag="rs3")
                    nc.vector.reciprocal(rstd[:], sd[:])
                    seff = eout.tile([128, 1], F32, tag="se3")
                    nc.vector.tensor_tensor(seff[:], rstd[:], mask_pp[:, tt:tt + 1],
                                            AluOpType.mult)
                    beff = eout.tile([128, 1], F32, tag="be3")
                    nc.vector.scalar_tensor_tensor(beff[:], mv[:, 0:1], -1.0, seff[:],
                                                   AluOpType.mult, AluOpType.mult)
                    ot = eout.tile([128, H], F32, tag="ot")
                    nc.scalar.activation(ot[:], h2_sb[:, tt, :], AF.Identity,
                                         bias=beff[:], scale=seff[:])
                    nc.sync.dma_start(out_d[tt * 128:(tt + 1) * 128, :], ot[:])

        h2p_cm.__exit__(None, None, None)
        small_cm.__exit__(None, None, None)

    nc.compile()
    return nc


def _prep_host(inputs):
    f32 = np.float32
    bf16 = ml_dtypes.bfloat16
    x = np.asarray(inputs["inputs"], f32)
    ln1_g = np.asarray(inputs["ln1_g"], f32); ln1_b = np.asarray(inputs["ln1_b"], f32)
    w_qkv = np.asarray(inputs["w_qkv"], f32); b_qkv = np.asarray(inputs["b_qkv"], f32)
    w_out = np.asarray(inputs["w_out"], f32); b_out = np.asarray(inputs["b_out"], f32)
    ln2_g = np.asarray(inputs["ln2_g"], f32); ln2_b = np.asarray(inputs["ln2_b"], f32)
    w_router = np.asarray(inputs["w_router"], f32)
    b_router = np.asarray(inputs["b_router"], f32)
    w_exp = np.asarray(inputs["w_exp"], f32); b_exp = np.asarray(inputs["b_exp"], f32)
    w_al1 = np.asarray(inputs["w_al1"], f32); b_al1 = np.asarray(inputs["b_al1"], f32)
    w_al2 = np.asarray(inputs["w_al2"], f32); b_al2 = np.asarray(inputs["b_al2"], f32)

    wq_f = (ln1_g[:, None] * w_qkv).astype(bf16)
    bq_f = b_qkv + ln1_b @ w_qkv
    assert np.all(bq_f[2 * H:] == 0.0), "nonzero V bias not supported"
    bqkv_t = np.zeros((128, 16), f32)
    for j in range(16):
        bqkv_t[:, j] = bq_f[j * 128:(j + 1) * 128]
    wr_f = ln2_g[:, None] * w_router
    br_f = b_router + ln2_b @ w_router
    we_f = ln2_g[None, :, None] * w_exp
    be_f = b_exp + np.einsum("h,ehf->ef", ln2_b, w_exp)
    x_pb = x + b_out[None, None, :]

    def mk_bmask(par):
        # first-suffix-block mask per key chunk kt: key j (row), query i (col)
        # within the block; slot s0 = kt//2 has global query chunk c = 2*s0+par.
        m = np.zeros((128, 16, 128), f32)
        for kt in range(16):
            c = 2 * (kt // 2) + par
            if c == kt:
                m[:, kt, :] = (np.arange(128)[:, None] <= np.arange(128)[None, :])
            elif c > kt:
                m[:, kt, :] = 1.0
            # c < kt: fully masked -> zeros
        return m.reshape(128, 16 * 128).astype(bf16)
    bmasks = [mk_bmask(0), mk_bmask(1)]

    thresh = np.full((128, 1), 0.8 - float(b_al2[0]), f32)
    bal1_t = np.zeros((128, 2), f32)
    bal1_t[:, 0] = b_al1[0:128]
    bal1_t[:, 1] = b_al1[128:256]

    shared = dict(
        wq=np.ascontiguousarray(wq_f),
        bqkv=bqkv_t,
        wout=np.ascontiguousarray(w_out),
        wrout=np.ascontiguousarray(wr_f.astype(f32)),
        brout=np.tile(br_f[None, :], (128, 1)).astype(f32),
        wexp=np.ascontiguousarray(we_f.astype(f32)),
        bexp=np.ascontiguousarray(be_f.astype(f32)),
        wal1=np.ascontiguousarray(w_al1),
        bal1=bal1_t,
        wal2=np.ascontiguousarray(w_al2),
        thresh=thresh,
    )
    per_core = []
    for c in range(N_CORES):
        b, par = c // 2, c % 2
        own_idx = np.concatenate(
            [np.arange(128) + (2 * s + par) * 128 for s in range(NSLOT)])
        m = dict(shared)
        m["x_kv"] = np.ascontiguousarray(x[b])
        m["x_ownr"] = np.ascontiguousarray(x[b][own_idx])
        m["x_own"] = np.ascontiguousarray(x_pb[b][own_idx])
        m["bmask"] = bmasks[par]
        per_core.append(m)
    return per_core


def kernel(**inputs):
    from concourse.bass_utils import run_bass_kernel_spmd

    if "prog" not in _prog_cache:
        _prog_cache["prog"] = _build_program()
    nc = _prog_cache["prog"]

    per_core = _prep_host(inputs)
    trace = bool(globals().get("TRACE", False))
    res = run_bass_kernel_spmd(nc, per_core, core_ids=list(range(N_CORES)),
                               trace=trace)
    _prog_cache["last_result"] = res

    lnf_g = np.asarray(inputs["lnf_g"], np.float32)
    lnf_b = np.asarray(inputs["lnf_b"], np.float32)
    out = np.zeros((B, S, H), np.float32)
    for c in range(N_CORES):
        b, par = c // 2, c % 2
        o = res.results[c]["out"]
        for s in range(NSLOT):
            g0 = (2 * s + par) * 128
            out[b, g0:g0 + 128, :] = o[s * 128:(s + 1) * 128, :]
    return out * lnf_g[None, None, :] + lnf_b[None, None, :]


# revision 23
# speedup vs baseline: 1.2348x; 1.0328x over previous
"""Trainium2 Bass kernel for nn_EnhancedTransformerBlock_80169859548047.

Sharding: 8 cores = (batch b, parity par). Core c handles batch b=c//2 and the
even (par=0) or odd (par=1) 128-token chunks of that batch's 2048-token
sequence. The attention schedule is parity-uniform: for key chunk kt the query
suffix starts at slot fs(kt)=kt//2; the first suffix block's causal mask
(triangular / all-ones / all-zeros, depending on parity) is host data, so the
instruction stream is identical on all cores.

Dtypes: attention path (LN1 out, w_qkv, Q/K/V, P) in bf16; out-proj, experts,
router and gate matmuls in float32r; residual stream and LN math in fp32.
Softmax denominators come from a ones column appended per head to V (exact
PSUM accumulation). LN1/LN2 affines are folded into the following weights on
the host; the final LN affine is applied on the host after gathering.
"""

import numpy as np
import ml_dtypes

B, S, H, E, NH, HD = 4, 2048, 1024, 8, 16, 64
N_CORES = 8
EPS = 1e-12
SCALE = HD ** -0.5
NSLOT = 8                # 128-token chunks per core
OWN = NSLOT * 128        # own tokens per core
HT = H // 128            # 8 H-tiles
FS = [kt // 2 for kt in range(16)]          # suffix first slot per key chunk
# attention schedule: key chunks 0..7 processed singly (suffix > 512 wide),
# 8..15 in pairs (same fs, suffix <= 512)
SCHED = [(0,), (1,), (2,), (3,), (4,), (5,), (6,), (7,),
         (8, 9), (10, 11), (12, 13), (14, 15)]

_prog_cache = {}


def _build_program():
    import concourse.bacc as bacc
    import concourse.tile as tile
    import concourse.mybir as mybir
    from concourse.masks import make_identity
    from concourse.alu_op_type import AluOpType
    from contextlib import ExitStack

    F32 = mybir.dt.float32
    F32R = mybir.dt.float32r
    BF16 = mybir.dt.bfloat16
    AF = mybir.ActivationFunctionType

    nc = bacc.Bacc("TRN2", target_bir_lowering=False, debug=False, num_devices=1)

    def din(name, shape, dt):
        return nc.dram_tensor(name, list(shape), dt, kind="ExternalInput").ap()

    x_kv_d = din("x_kv", (S, H), F32)
    x_ownr_d = din("x_ownr", (OWN, H), F32)   # raw inputs, own tokens, slot order
    x_own_d = din("x_own", (OWN, H), F32)     # inputs + b_out, own tokens
    wq_d = din("wq", (H, 3 * H), BF16)
    bqkv_d = din("bqkv", (128, 16), F32)
    wout_d = din("wout", (H, H), F32R)
    wrout_d = din("wrout", (H, E), F32R)
    brout_d = din("brout", (128, E), F32)
    wexp_d = din("wexp", (E, H, H), F32R)
    bexp_d = din("bexp", (E, H), F32R)
    wal1_d = din("wal1", (H, 256), F32R)
    bal1_d = din("bal1", (128, 2), F32)
    wal2_d = din("wal2", (256, 1), F32R)
    bmask_d = din("bmask", (128, 16 * 128), BF16)  # per-kt first-block masks
    thresh_d = din("thresh", (128, 1), F32)   # 0.8 - b_al2, replicated
    out_d = nc.dram_tensor("out", [OWN, H], F32, kind="ExternalOutput").ap()

    with tile.TileContext(nc) as tc, ExitStack() as st:
        # manually-managed pools (non-LIFO lifetimes)
        small_cm = tc.tile_pool(name="small", bufs=1)
        small = small_cm.__enter__()
        id_bf = small.tile([128, 128], BF16)
        id_f = small.tile([128, 128], F32)
        bqkv_sb = small.tile([128, 16], F32)
        thresh_sb = small.tile([128, 1], F32)
        eps_sb = small.tile([128, 1], F32)
        nc.gpsimd.memset(eps_sb[:], EPS)
        nc.sync.dma_start(bqkv_sb[:], bqkv_d[:])
        nc.sync.dma_start(thresh_sb[:], thresh_d[:])
        id_r_t = small.tile([128, 128], F32R)
        make_identity(nc, id_bf[:])
        make_identity(nc, id_f[:])
        nc.vector.tensor_copy(id_r_t[:], id_f[:])
        id_r = id_r_t[:]

        kvq_cm = tc.tile_pool(name="kvq", bufs=1)
        kvq = kvq_cm.__enter__()
        KTb = kvq.tile([128, HT, S], BF16)            # K^T [kcol, tok]
        Vb = kvq.tile([128, 16, NH, 65], BF16)        # V token-major + ones col
        QTb = kvq.tile([128, HT, OWN], BF16)          # Q^T [qcol, own tok]
        xownT = kvq.tile([128, HT, NSLOT, 128], BF16)  # own tokens, LN1'd, ^T

        # =========== Phase A: LN1 + transpose + QKV ===========
        def layer_norm_apply(pool, src_ap, out_ap):
            stats = pool.tile([128, 2, 6], F32, tag="st")
            nc.vector.bn_stats(stats[:, 0, :], src_ap[:, 0:512])
            nc.vector.bn_stats(stats[:, 1, :], src_ap[:, 512:1024])
            mv = pool.tile([128, 2], F32, tag="mv")
            nc.vector.bn_aggr(mv[:], stats[:])
            sd = pool.tile([128, 1], F32, tag="sd")
            nc.scalar.activation(sd[:], mv[:, 1:2], AF.Sqrt, bias=eps_sb[:])
            rstd = pool.tile([128, 1], F32, tag="rs")
            nc.vector.reciprocal(rstd[:], sd[:])
            nbias = pool.tile([128, 1], F32, tag="nb")
            nc.vector.scalar_tensor_tensor(
                nbias[:], mv[:, 0:1], -1.0, rstd[:],
                AluOpType.mult, AluOpType.mult)
            nc.scalar.activation(out_ap, src_ap, AF.Identity,
                                 bias=nbias[:], scale=rstd[:])
            return mv, rstd

        with (
            tc.tile_pool(name="xln_pool", bufs=1) as xlnp,
            tc.tile_pool(name="a_io", bufs=3) as aio,
            tc.tile_pool(name="a_pst", bufs=3, space="PSUM") as apst,
            tc.tile_pool(name="a_psk", bufs=2, space="PSUM") as apsk,
            tc.tile_pool(name="a_psv", bufs=2, space="PSUM") as apsv,
        ):
            with nc.named_scope("phaseA"):
                # xlnT: [h-part, kt, global token chunk, col]
                xlnT = xlnp.tile([128, HT, 16, 128], BF16)
                aw_cm = tc.tile_pool(name="a_w", bufs=1)
                aw = aw_cm.__enter__()
                wk = aw.tile([128, HT, H], BF16)
                wv = aw.tile([128, HT, H], BF16)
                nc.sync.dma_start(
                    wk[:], wq_d[:, H:2 * H].rearrange("(kt p) c -> p kt c", p=128))
                nc.sync.dma_start(
                    wv[:], wq_d[:, 2 * H:3 * H].rearrange("(kt p) c -> p kt c", p=128))
                for tt in range(16):
                    nc.gpsimd.memset(Vb[:, tt, :, 64:65], 1.0)

                def ln_transpose(tt_src, dst_tile, dst_tt):
                    xt = aio.tile([128, H], F32, tag="xt")
                    nc.sync.dma_start(xt[:], tt_src)
                    xl = aio.tile([128, H], BF16, tag="xl")
                    layer_norm_apply(aio, xt[:], xl[:])
                    for kt2 in range(2):
                        tp = apst.tile([128, 4, 128], BF16, tag="tp")
                        for q in range(4):
                            kt = kt2 * 4 + q
                            nc.tensor.transpose(
                                tp[:, q, :], xl[:, kt * 128:(kt + 1) * 128], id_bf[:])
                        nc.vector.tensor_copy(
                            dst_tile[:, kt2 * 4:(kt2 + 1) * 4, dst_tt, :], tp[:])

                for n in range(4):
                    for j in range(4):
                        tt = 4 * n + j
                        ln_transpose(x_kv_d[tt * 128:(tt + 1) * 128, :],
                                     xlnT, tt)
                    # K matmuls for this 512-token block
                    for kc in range(8):
                        ps = apsk.tile([128, 512], F32, tag="kps")
                        for kt in range(HT):
                            nc.tensor.matmul(
                                ps[:], wk[:, kt, kc * 128:(kc + 1) * 128],
                                xlnT[:, kt, 4 * n:4 * n + 4, :],
                                start=(kt == 0), stop=(kt == HT - 1))
                        nc.scalar.activation(
                            KTb[:, kc, n * 512:(n + 1) * 512], ps[:],
                            AF.Identity, bias=bqkv_sb[:, 8 + kc:9 + kc])
                    # V matmuls for this block (token-major)
                    for j in range(4):
                        tt = 4 * n + j
                        for vh in range(2):
                            ps = apsv.tile([128, 512], F32, tag="vps")
                            for kt in range(HT):
                                nc.tensor.matmul(
                                    ps[:], xlnT[:, kt, tt, :],
                                    wv[:, kt, vh * 512:(vh + 1) * 512],
                                    start=(kt == 0), stop=(kt == HT - 1))
                            nc.vector.tensor_copy(
                                Vb[:, tt, vh * 8:(vh + 1) * 8, 0:64],
                                ps[:].rearrange("p (h c) -> p h c", h=8))
                    # own tokens: LN + transpose for Q (overlaps K/V matmuls)
                    for so in (2 * n, 2 * n + 1):
                        ln_transpose(x_ownr_d[so * 128:(so + 1) * 128, :],
                                     xownT, so)
                aw_cm.__exit__(None, None, None)

        # =========== Phase B: attention ===========
        attn_cm = tc.tile_pool(name="attn_p", bufs=1, side="right")
        attn_p = attn_cm.__enter__()
        attnT = attn_p.tile([128, HT, OWN], F32R)
        bmask_sb = attn_p.tile([128, 16 * 128], BF16)
        nc.sync.dma_start(bmask_sb[:], bmask_d[:])
        with (
            tc.tile_pool(name="b_p", bufs=5) as bp,
            tc.tile_pool(name="b_sc", bufs=3) as bsc,
            tc.tile_pool(name="b_wq", bufs=2) as bwq,
            tc.tile_pool(name="b_ps", bufs=2, space="PSUM") as bps,
            tc.tile_pool(name="b_pv", bufs=1, space="PSUM") as bpv,
        ):
            with nc.named_scope("phaseB"):
                # Software-pipelined: scores/exp/mask of unit j+1 are emitted
                # before the PV matmuls of unit j so the in-order PE never
                # waits on the Act-engine exp of the unit it is consuming.
                # Q-projection column blocks are interleaved as PE-only
                # pseudo-units before each head pair: they give the tensor
                # engine a dependency-free stretch that keeps its clock
                # ramped across head boundaries.
                pvs = {}   # h -> (pv0, pv1)

                def emit_q(qc):
                    wcol = bwq.tile([128, HT, 128], BF16, tag="wcol")
                    nc.sync.dma_start(
                        wcol[:], wq_d[:, qc * 128:(qc + 1) * 128]
                        .rearrange("(kt p) c -> p kt c", p=128))
                    for half in range(2):
                        ps = bps.tile([128, 512], F32, tag="qps")
                        for kt in range(HT):
                            nc.tensor.matmul(
                                ps[:], wcol[:, kt, :],
                                xownT[:, kt, half * 4:(half + 1) * 4, :],
                                start=(kt == 0), stop=(kt == HT - 1))
                        nc.scalar.activation(
                            QTb[:, qc, half * 512:(half + 1) * 512], ps[:],
                            AF.Identity, bias=bqkv_sb[:, qc:qc + 1])

                def emit_scores(h, item):
                    hp, hb = h // 2, 64 * (h % 2)
                    if len(item) == 1:
                        kt = item[0]
                        a = FS[kt] * 128
                        sc = bps.tile([128, 1024], F32, tag="sc")
                        stat = KTb[hb:hb + 64, hp, kt * 128:(kt + 1) * 128]
                        nc.tensor.matmul(sc[:, a:512], stat,
                                         QTb[hb:hb + 64, hp, a:512],
                                         start=True, stop=True)
                        nc.tensor.matmul(sc[:, 512:1024], stat,
                                         QTb[hb:hb + 64, hp, 512:1024],
                                         start=True, stop=True)
                        p = bp.tile([128, 1024], BF16, tag="p")
                        nc.scalar.activation(p[:, a:1024], sc[:, a:1024],
                                             AF.Exp, scale=SCALE)
                        nc.vector.tensor_tensor(
                            p[:, a:a + 128], p[:, a:a + 128],
                            bmask_sb[:, kt * 128:(kt + 1) * 128],
                            AluOpType.mult)
                    else:
                        a = FS[item[0]] * 128
                        L = 1024 - a
                        sc = bps.tile([128, 2, 512], F32, tag="sc")
                        p = bp.tile([128, 2, 512], BF16, tag="p")
                        for jj, ktj in enumerate(item):
                            nc.tensor.matmul(
                                sc[:, jj, 0:L],
                                KTb[hb:hb + 64, hp, ktj * 128:(ktj + 1) * 128],
                                QTb[hb:hb + 64, hp, a:1024],
                                start=True, stop=True)
                        nc.scalar.activation(p[:, :, 0:L], sc[:, :, 0:L],
                                             AF.Exp, scale=SCALE)
                        for jj, ktj in enumerate(item):
                            nc.vector.tensor_tensor(
                                p[:, jj, 0:128], p[:, jj, 0:128],
                                bmask_sb[:, ktj * 128:(ktj + 1) * 128],
                                AluOpType.mult)
                    return p

                def emit_pv(h, item, p):
                    if h not in pvs:
                        pvs[h] = (bpv.tile([65, 512], F32, tag="pv0",
                                           name=f"pv0_{h}"),
                                  bpv.tile([65, 512], F32, tag="pv1",
                                           name=f"pv1_{h}"))
                    pv0, pv1 = pvs[h]
                    if len(item) == 1:
                        kt = item[0]
                        a = FS[kt] * 128
                        vstat = Vb[:, kt, h, :]
                        nc.tensor.matmul(pv0[:, a:512], vstat, p[:, a:512],
                                         start=(kt == 0), stop=(kt == 7))
                        nc.tensor.matmul(pv1[:, 0:512], vstat, p[:, 512:1024],
                                         start=(kt == 0), stop=False)
                    else:
                        a = FS[item[0]] * 128
                        L = 1024 - a
                        for jj, ktj in enumerate(item):
                            nc.tensor.matmul(
                                pv1[:, a - 512:a - 512 + L],
                                Vb[:, ktj, h, :], p[:, jj, 0:L],
                                start=False, stop=(ktj == 15))

                def emit_norm(h):
                    hp, hb = h // 2, 64 * (h % 2)
                    for qb, pv in ((0, pvs[h][0]), (1, pvs[h][1])):
                        rd = bsc.tile([1, 512], F32, tag="rd")
                        nc.vector.reciprocal(rd[:], pv[64:65, :])
                        bc = bsc.tile([64, 512], F32, tag="bc")
                        nc.gpsimd.partition_broadcast(bc[:], rd[0:1, :])
                        nc.vector.tensor_tensor(
                            attnT[hb:hb + 64, hp, qb * 512:(qb + 1) * 512],
                            pv[0:64, :], bc[:], AluOpType.mult)

                prev = None
                for qc in range(8):
                    emit_q(qc)
                    for h in (2 * qc, 2 * qc + 1):
                        for item in SCHED:
                            p = emit_scores(h, item)
                            if prev is not None:
                                emit_pv(*prev)
                                if prev[1] is SCHED[-1]:
                                    emit_norm(prev[0])
                            prev = (h, item, p)
                emit_pv(*prev)
                emit_norm(prev[0])

        # =========== Phase C: out-proj + residual + LN2 + y^T ===========
        kvq_cm.__exit__(None, None, None)
        hy_cm = tc.tile_pool(name="hy", bufs=1)
        hy = hy_cm.__enter__()
        h_sb = hy.tile([128, NSLOT, H], F32)
        yT_sb = hy.tile([128, HT, OWN], F32R)
        with (
            tc.tile_pool(name="c_w", bufs=2) as cw,
            tc.tile_pool(name="c_io", bufs=3) as cio,
            tc.tile_pool(name="c_ps", bufs=2, space="PSUM") as cps,
            tc.tile_pool(name="c_pst", bufs=4, space="PSUM") as cpst,
        ):
            with nc.named_scope("phaseC"):
                def ln2_transpose(tt):
                    yt = cio.tile([128, H], F32R, tag="yt")
                    layer_norm_apply(cio, h_sb[:, tt, :], yt[:])
                    for kt2 in range(2):
                        tp = cpst.tile([128, 4, 128], F32R, tag="tp2")
                        for q in range(4):
                            kt = kt2 * 4 + q
                            nc.tensor.transpose(
                                tp[:, q, :], yt[:, kt * 128:(kt + 1) * 128], id_r)
                        nc.vector.tensor_copy(
                            yT_sb[:, kt2 * 4:(kt2 + 1) * 4, tt * 128:(tt + 1) * 128],
                            tp[:])

                for n in range(2):
                    won = cw.tile([128, HT, 512], F32R, tag="won")
                    nc.sync.dma_start(
                        won[:], wout_d[:, n * 512:(n + 1) * 512]
                        .rearrange("(kt p) c -> p kt c", p=128))
                    for tt in range(NSLOT):
                        ps = cps.tile([128, 512], F32, tag="ops")
                        for kt in range(HT):
                            nc.tensor.matmul(ps[:], attnT[:, kt, tt * 128:(tt + 1) * 128],
                                             won[:, kt, :],
                                             start=(kt == 0), stop=(kt == HT - 1))
                        xo = cio.tile([128, 512], F32, tag="xo")
                        nc.sync.dma_start(
                            xo[:], x_own_d[tt * 128:(tt + 1) * 128, n * 512:(n + 1) * 512])
                        nc.vector.tensor_tensor(h_sb[:, tt, n * 512:(n + 1) * 512],
                                                ps[:], xo[:], AluOpType.add)
                        if n == 1 and tt >= 1:
                            ln2_transpose(tt - 1)
                ln2_transpose(NSLOT - 1)

        # =========== Phase D: router + experts ===========
        attn_cm.__exit__(None, None, None)
        h2p_cm = tc.tile_pool(name="h2p", bufs=1, side="right")
        h2p = h2p_cm.__enter__()
        h2_sb = h2p.tile([128, NSLOT, H], F32)
        rw_sb = h2p.tile([128, NSLOT, E], F32)
        rwT_f = h2p.tile([8, OWN], F32)
        rwT_r = h2p.tile([8, OWN], F32R)
        with (
            tc.tile_pool(name="d_sc", bufs=2) as dsc,
            tc.tile_pool(name="d_ps", bufs=2, space="PSUM") as dps,
            tc.tile_pool(name="d_pst", bufs=2, space="PSUM") as dpst,
        ):
            with nc.named_scope("phaseD_router"):
                wr = dsc.tile([128, HT, E], F32R, tag="wr")
                nc.sync.dma_start(wr[:], wrout_d.rearrange("(kt p) e -> p kt e", p=128))
                br = dsc.tile([128, E], F32, tag="br")
                nc.sync.dma_start(br[:], brout_d[:])
                for tt in range(NSLOT):
                    ps = dps.tile([128, E], F32, tag="rps")
                    for kt in range(HT):
                        nc.tensor.matmul(ps[:], yT_sb[:, kt, tt * 128:(tt + 1) * 128],
                                         wr[:, kt, :],
                                         start=(kt == 0), stop=(kt == HT - 1))
                    lg = dsc.tile([128, E], F32, tag="lg")
                    nc.vector.tensor_tensor(lg[:], ps[:], br[:], AluOpType.add)
                    ex = dsc.tile([128, E], F32, tag="ex")
                    rsum = dsc.tile([128, 1], F32, tag="rsum")
                    nc.scalar.activation(ex[:], lg[:], AF.Exp, accum_out=rsum[:])
                    rrec = dsc.tile([128, 1], F32, tag="rrec")
                    nc.vector.reciprocal(rrec[:], rsum[:])
                    nc.vector.tensor_scalar(rw_sb[:, tt, :], ex[:], rrec[:], None,
                                            AluOpType.mult)
                    tp = dpst.tile([8, 128], F32, tag="rtp")
                    nc.tensor.transpose(tp[:], rw_sb[:, tt, :], id_f[:])
                    nc.vector.tensor_copy(rwT_r[:, tt * 128:(tt + 1) * 128], tp[:])
                    nc.vector.tensor_copy(rwT_f[:, tt * 128:(tt + 1) * 128], tp[:])

        with (
            tc.tile_pool(name="d_y", bufs=2) as dy,
            tc.tile_pool(name="d_w", bufs=4) as dw,
            tc.tile_pool(name="d_b", bufs=1) as db,
            tc.tile_pool(name="d_moe", bufs=1, space="PSUM") as dmoe,
        ):
            with nc.named_scope("phaseD_experts"):
                bexp_sb = db.tile([8, H], F32R)
                nc.sync.dma_start(bexp_sb[:], bexp_d[:])
                for fh in range(2):
                    mps = [dmoe.tile([128, 512], F32, tag=f"mps{tt}",
                                     name=f"mps_{fh}_{tt}")
                           for tt in range(NSLOT)]
                    for e in range(E):
                        bcr = dy.tile([128, OWN], F32, tag="bcr")
                        nc.gpsimd.partition_broadcast(bcr[:], rwT_f[e:e + 1, :])
                        for kt in range(HT):
                            ye = dy.tile([128, OWN], F32R, tag="ye", bufs=4)
                            nc.vector.tensor_tensor(ye[:], yT_sb[:, kt, :],
                                                    bcr[:], AluOpType.mult)
                            we = dw.tile([128, 512], F32R, tag="we")
                            nc.sync.dma_start(
                                we[:],
                                wexp_d[e, kt * 128:(kt + 1) * 128,
                                       fh * 512:(fh + 1) * 512])
                            for tt in range(NSLOT):
                                nc.tensor.matmul(
                                    mps[tt][:], ye[:, tt * 128:(tt + 1) * 128],
                                    we[:],
                                    start=(e == 0 and kt == 0), stop=False)
                    for tt in range(NSLOT):
                        nc.tensor.matmul(mps[tt][:],
                                         rwT_r[:, tt * 128:(tt + 1) * 128],
                                         bexp_sb[:, fh * 512:(fh + 1) * 512],
                                         start=False, stop=True)
                        nc.vector.tensor_tensor(
                            h2_sb[:, tt, fh * 512:(fh + 1) * 512], mps[tt][:],
                            h_sb[:, tt, fh * 512:(fh + 1) * 512], AluOpType.add)

        # =========== Phase E: gate + LNf + output ===========
        hy_cm.__exit__(None, None, None)
        with (
            tc.tile_pool(name="e_sc", bufs=1) as esc,
            tc.tile_pool(name="e_tmp", bufs=2) as etmp,
            tc.tile_pool(name="e_pst", bufs=3, space="PSUM") as epst,
            tc.tile_pool(name="e_psg", bufs=2, space="PSUM") as epsg,
            tc.tile_pool(name="e_psal", bufs=1, space="PSUM") as epsal,
            tc.tile_pool(name="e_out", bufs=3) as eout,
        ):
            with nc.named_scope("phaseE"):
                mask_pp = esc.tile([128, NSLOT], F32)
                maskrow = esc.tile([1, OWN], F32)
                h2T = esc.tile([128, HT, OWN], F32R)
                for tt in range(NSLOT):
                    for kt2 in range(2):
                        tp = epst.tile([128, 4, 128], F32, tag="tp3")
                        for q in range(4):
                            kt = kt2 * 4 + q
                            nc.tensor.transpose(
                                tp[:, q, :], h2_sb[:, tt, kt * 128:(kt + 1) * 128],
                                id_f[:])
                        nc.vector.tensor_copy(
                            h2T[:, kt2 * 4:(kt2 + 1) * 4, tt * 128:(tt + 1) * 128],
                            tp[:])
                wal1 = esc.tile([128, HT, 256], F32R)
                nc.sync.dma_start(wal1[:], wal1_d.rearrange("(kt p) c -> p kt c", p=128))
                bal1 = esc.tile([128, 2], F32)
                nc.sync.dma_start(bal1[:], bal1_d[:])
                wal2 = esc.tile([128, 2, 1], F32R)
                nc.sync.dma_start(wal2[:], wal2_d.rearrange("(m p) c -> p m c", p=128))
                gT = esc.tile([128, 2, OWN], F32R)
                for m2 in range(2):
                    for n in range(2):
                        ps = epsg.tile([128, 512], F32, tag="gps")
                        for kt in range(HT):
                            nc.tensor.matmul(ps[:],
                                             wal1[:, kt, m2 * 128:(m2 + 1) * 128],
                                             h2T[:, kt, n * 512:(n + 1) * 512],
                                             start=(kt == 0), stop=(kt == HT - 1))
                        nc.scalar.activation(gT[:, m2, n * 512:(n + 1) * 512], ps[:],
                                             AF.Gelu, bias=bal1[:, m2:m2 + 1])
                for n in range(2):
                    ps = epsal.tile([1, 512], F32, tag="alps")
                    for m2 in range(2):
                        nc.tensor.matmul(ps[:], wal2[:, m2, :],
                                         gT[:, m2, n * 512:(n + 1) * 512],
                                         start=(m2 == 0), stop=(m2 == 1))
                    nc.vector.tensor_scalar(maskrow[:, n * 512:(n + 1) * 512], ps[:],
                                            thresh_sb[0:1, 0:1], None, AluOpType.is_gt)
                for tt in range(NSLOT):
                    nc.sync.dma_start(
                        mask_pp[:, tt:tt + 1],
                        maskrow[0:1, tt * 128:(tt + 1) * 128])
                for tt in range(NSLOT):
                    stats = eout.tile([128, 2, 6], F32, tag="st3")
                    nc.vector.bn_stats(stats[:, 0, :], h2_sb[:, tt, 0:512])
                    nc.vector.bn_stats(stats[:, 1, :], h2_sb[:, tt, 512:1024])
                    mv = eout.tile([128, 2], F32, tag="mv3")
                    nc.vector.bn_aggr(mv[:], stats[:])
                    sd = eout.tile([128, 1], F32, tag="sd3")
                    nc.scalar.activation(sd[:], mv[:, 1:2], AF.Sqrt, bias=eps_sb[:])
                    rstd = eout.tile([128, 1], F32, tag="rs3")
                    nc.vector.reciprocal(rstd[:], sd[:])
                    seff = eout.tile([128, 1], F32, tag="se3")
                    nc.vector.tensor_tensor(seff[:], rstd[:], mask_pp[:, tt:tt + 1],
                                            AluOpType.mult)
                    beff = eout.tile([128, 1], F32, tag="be3")
                    nc.vector.scalar_tensor_tensor(beff[:], mv[:, 0:1], -1.0, seff[:],
                                                   AluOpType.mult, AluOpType.mult)
                    ot = eout.tile([128, H], F32, tag="ot")
                    nc.scalar.activation(ot[:], h2_sb[:, tt, :], AF.Identity,
                                         bias=beff[:], scale=seff[:])
                    nc.sync.dma_start(out_d[tt * 128:(tt + 1) * 128, :], ot[:])

        h2p_cm.__exit__(None, None, None)
        small_cm.__exit__(None, None, None)

    nc.compile()
    return nc


def _prep_host(inputs):
    f32 = np.float32
    bf16 = ml_dtypes.bfloat16
    x = np.asarray(inputs["inputs"], f32)
    ln1_g = np.asarray(inputs["ln1_g"], f32); ln1_b = np.asarray(inputs["ln1_b"], f32)
    w_qkv = np.asarray(inputs["w_qkv"], f32); b_qkv = np.asarray(inputs["b_qkv"], f32)
    w_out = np.asarray(inputs["w_out"], f32); b_out = np.asarray(inputs["b_out"], f32)
    ln2_g = np.asarray(inputs["ln2_g"], f32); ln2_b = np.asarray(inputs["ln2_b"], f32)
    w_router = np.asarray(inputs["w_router"], f32)
    b_router = np.asarray(inputs["b_router"], f32)
    w_exp = np.asarray(inputs["w_exp"], f32); b_exp = np.asarray(inputs["b_exp"], f32)
    w_al1 = np.asarray(inputs["w_al1"], f32); b_al1 = np.asarray(inputs["b_al1"], f32)
    w_al2 = np.asarray(inputs["w_al2"], f32); b_al2 = np.asarray(inputs["b_al2"], f32)

    wq_f = (ln1_g[:, None] * w_qkv).astype(bf16)
    bq_f = b_qkv + ln1_b @ w_qkv
    assert np.all(bq_f[2 * H:] == 0.0), "nonzero V bias not supported"
    bqkv_t = np.zeros((128, 16), f32)
    for j in range(16):
        bqkv_t[:, j] = bq_f[j * 128:(j + 1) * 128]
    wr_f = ln2_g[:, None] * w_router
    br_f = b_router + ln2_b @ w_router
    we_f = ln2_g[None, :, None] * w_exp
    be_f = b_exp + np.einsum("h,ehf->ef", ln2_b, w_exp)
    x_pb = x + b_out[None, None, :]

    def mk_bmask(par):
        # first-suffix-block mask per key chunk kt: key j (row), query i (col)
        # within the block; slot s0 = kt//2 has global query chunk c = 2*s0+par.
        m = np.zeros((128, 16, 128), f32)
        for kt in range(16):
            c = 2 * (kt // 2) + par
            if c == kt:
                m[:, kt, :] = (np.arange(128)[:, None] <= np.arange(128)[None, :])
            elif c > kt:
                m[:, kt, :] = 1.0
            # c < kt: fully masked -> zeros
        return m.reshape(128, 16 * 128).astype(bf16)
    bmasks = [mk_bmask(0), mk_bmask(1)]

    thresh = np.full((128, 1), 0.8 - float(b_al2[0]), f32)
    bal1_t = np.zeros((128, 2), f32)
    bal1_t[:, 0] = b_al1[0:128]
    bal1_t[:, 1] = b_al1[128:256]

    shared = dict(
        wq=np.ascontiguousarray(wq_f),
        bqkv=bqkv_t,
        wout=np.ascontiguousarray(w_out),
        wrout=np.ascontiguousarray(wr_f.astype(f32)),
        brout=np.tile(br_f[None, :], (128, 1)).astype(f32),
        wexp=np.ascontiguousarray(we_f.astype(f32)),
        bexp=np.ascontiguousarray(be_f.astype(f32)),
        wal1=np.ascontiguousarray(w_al1),
        bal1=bal1_t,
        wal2=np.ascontiguousarray(w_al2),
        thresh=thresh,
    )
    per_core = []
    for c in range(N_CORES):
        b, par = c // 2, c % 2
        own_idx = np.concatenate(
            [np.arange(128) + (2 * s + par) * 128 for s in range(NSLOT)])
        m = dict(shared)
        m["x_kv"] = np.ascontiguousarray(x[b])
        m["x_ownr"] = np.ascontiguousarray(x[b][own_idx])
        m["x_own"] = np.ascontiguousarray(x_pb[b][own_idx])
        m["bmask"] = bmasks[par]
        per_core.append(m)
    return per_core


def kernel(**inputs):
    from concourse.bass_utils import run_bass_kernel_spmd

    if "prog" not in _prog_cache:
        _prog_cache["prog"] = _build_program()
    nc = _prog_cache["prog"]

    per_core = _prep_host(inputs)
    trace = bool(globals().get("TRACE", False))
    res = run_bass_kernel_spmd(nc, per_core, core_ids=list(range(N_CORES)),
                               trace=trace)
    _prog_cache["last_result"] = res

    lnf_g = np.asarray(inputs["lnf_g"], np.float32)
    lnf_b = np.asarray(inputs["lnf_b"], np.float32)
    out = np.zeros((B, S, H), np.float32)
    for c in range(N_CORES):
        b, par = c // 2, c % 2
        o = res.results[c]["out"]
        for s in range(NSLOT):
            g0 = (2 * s + par) * 128
            out[b, g0:g0 + 128, :] = o[s * 128:(s + 1) * 128, :]
    return out * lnf_g[None, None, :] + lnf_b[None, None, :]


# revision 32
# speedup vs baseline: 1.2547x; 1.0161x over previous
"""Trainium2 Bass kernel for nn_EnhancedTransformerBlock_80169859548047.

Sharding: 8 cores = (batch b, parity par). Core c handles batch b=c//2 and the
even (par=0) or odd (par=1) 128-token chunks of that batch's 2048-token
sequence. The attention schedule is parity-uniform: for key chunk kt the query
suffix starts at slot fs(kt)=kt//2; the first suffix block's causal mask
(triangular / all-ones / all-zeros, depending on parity) is host data, so the
instruction stream is identical on all cores.

Dtypes: attention path (LN1 out, w_qkv, Q/K/V, P) in bf16; out-proj, experts,
router and gate matmuls in float32r; residual stream and LN math in fp32.
Softmax denominators come from a ones column appended per head to V (exact
PSUM accumulation). LN1/LN2 affines are folded into the following weights on
the host; the final LN affine is applied on the host after gathering.
"""

import numpy as np
import ml_dtypes

B, S, H, E, NH, HD = 4, 2048, 1024, 8, 16, 64
N_CORES = 8
EPS = 1e-12
SCALE = HD ** -0.5
NSLOT = 8                # 128-token chunks per core
OWN = NSLOT * 128        # own tokens per core
HT = H // 128            # 8 H-tiles
FS = [kt // 2 for kt in range(16)]          # suffix first slot per key chunk
# attention schedule: key chunks 0..7 processed singly (suffix > 512 wide),
# 8..15 in pairs (same fs, suffix <= 512)
SCHED = [(0,), (1,), (2,), (3,), (4,), (5,), (6,), (7,),
         (8, 9), (10, 11), (12, 13), (14, 15)]

_prog_cache = {}


def _build_program():
    import concourse.bacc as bacc
    import concourse.tile as tile
    import concourse.mybir as mybir
    from concourse.masks import make_identity
    from concourse.alu_op_type import AluOpType
    from contextlib import ExitStack

    F32 = mybir.dt.float32
    F32R = mybir.dt.float32r
    BF16 = mybir.dt.bfloat16
    AF = mybir.ActivationFunctionType

    nc = bacc.Bacc("TRN2", target_bir_lowering=False, debug=False, num_devices=1)

    def din(name, shape, dt):
        return nc.dram_tensor(name, list(shape), dt, kind="ExternalInput").ap()

    x_kv_d = din("x_kv", (S, H), F32)
    x_ownr_d = din("x_ownr", (OWN, H), F32)   # raw inputs, own tokens, slot order
    x_own_d = din("x_own", (OWN, H), F32)     # inputs + b_out, own tokens
    wq_d = din("wq", (H, 3 * H), BF16)
    bqkv_d = din("bqkv", (128, 16), F32)
    wout_d = din("wout", (H, H), F32R)
    wrout_d = din("wrout", (H, E), F32R)
    brout_d = din("brout", (128, E), F32)
    wexp_d = din("wexp", (E, H, H), F32R)
    bexp_d = din("bexp", (E, H), F32R)
    wal1_d = din("wal1", (H, 256), F32R)
    bal1_d = din("bal1", (128, 2), F32)
    wal2_d = din("wal2", (256, 1), F32R)
    bmask_d = din("bmask", (128, 16 * 128), BF16)  # per-kt first-block masks
    thresh_d = din("thresh", (128, 1), F32)   # 0.8 - b_al2, replicated
    out_d = nc.dram_tensor("out", [OWN, H], F32, kind="ExternalOutput").ap()

    with tile.TileContext(nc) as tc, ExitStack() as st:
        # manually-managed pools (non-LIFO lifetimes)
        small_cm = tc.tile_pool(name="small", bufs=1)
        small = small_cm.__enter__()
        id_bf = small.tile([128, 128], BF16)
        id_f = small.tile([128, 128], F32)
        bqkv_sb = small.tile([128, 16], F32)
        thresh_sb = small.tile([128, 1], F32)
        eps_sb = small.tile([128, 1], F32)
        nc.gpsimd.memset(eps_sb[:], EPS)
        nc.sync.dma_start(bqkv_sb[:], bqkv_d[:])
        nc.sync.dma_start(thresh_sb[:], thresh_d[:])
        id_r_t = small.tile([128, 128], F32R)
        make_identity(nc, id_bf[:])
        make_identity(nc, id_f[:])
        nc.vector.tensor_copy(id_r_t[:], id_f[:])
        id_r = id_r_t[:]

        kvq_cm = tc.tile_pool(name="kvq", bufs=1)
        kvq = kvq_cm.__enter__()
        KTb = kvq.tile([128, HT, S], BF16)            # K^T [kcol, tok]
        Vb = kvq.tile([128, 16, NH, 65], BF16)        # V token-major + ones col
        QTb = kvq.tile([128, HT, OWN], BF16)          # Q^T [qcol, own tok]
        xownT = kvq.tile([128, HT, NSLOT, 128], BF16)  # own tokens, LN1'd, ^T

        # =========== Phase A: LN1 + transpose + QKV ===========
        def layer_norm_apply(pool, src_ap, out_ap):
            stats = pool.tile([128, 2, 6], F32, tag="st")
            nc.vector.bn_stats(stats[:, 0, :], src_ap[:, 0:512])
            nc.vector.bn_stats(stats[:, 1, :], src_ap[:, 512:1024])
            mv = pool.tile([128, 2], F32, tag="mv")
            nc.vector.bn_aggr(mv[:], stats[:])
            sd = pool.tile([128, 1], F32, tag="sd")
            nc.scalar.activation(sd[:], mv[:, 1:2], AF.Sqrt, bias=eps_sb[:])
            rstd = pool.tile([128, 1], F32, tag="rs")
            nc.vector.reciprocal(rstd[:], sd[:])
            nbias = pool.tile([128, 1], F32, tag="nb")
            nc.vector.scalar_tensor_tensor(
                nbias[:], mv[:, 0:1], -1.0, rstd[:],
                AluOpType.mult, AluOpType.mult)
            nc.scalar.activation(out_ap, src_ap, AF.Identity,
                                 bias=nbias[:], scale=rstd[:])
            return mv, rstd

        with (
            tc.tile_pool(name="xln_pool", bufs=1) as xlnp,
            tc.tile_pool(name="a_io", bufs=3) as aio,
            tc.tile_pool(name="a_pst", bufs=3, space="PSUM") as apst,
            tc.tile_pool(name="a_psk", bufs=2, space="PSUM") as apsk,
            tc.tile_pool(name="a_psv", bufs=2, space="PSUM") as apsv,
        ):
            with nc.named_scope("phaseA"):
                # xlnT: [h-part, kt, global token chunk, col]
                xlnT = xlnp.tile([128, HT, 16, 128], BF16)
                aw_cm = tc.tile_pool(name="a_w", bufs=1)
                aw = aw_cm.__enter__()
                wk = aw.tile([128, HT, H], BF16)
                wv = aw.tile([128, HT, H], BF16)
                nc.sync.dma_start(
                    wk[:], wq_d[:, H:2 * H].rearrange("(kt p) c -> p kt c", p=128))
                nc.sync.dma_start(
                    wv[:], wq_d[:, 2 * H:3 * H].rearrange("(kt p) c -> p kt c", p=128))
                for tt in range(16):
                    nc.gpsimd.memset(Vb[:, tt, :, 64:65], 1.0)

                def ln_transpose(tt_src, dst_tile, dst_tt):
                    xt = aio.tile([128, H], F32, tag="xt")
                    nc.sync.dma_start(xt[:], tt_src)
                    xl = aio.tile([128, H], BF16, tag="xl")
                    layer_norm_apply(aio, xt[:], xl[:])
                    for kt2 in range(2):
                        tp = apst.tile([128, 4, 128], BF16, tag="tp")
                        for q in range(4):
                            kt = kt2 * 4 + q
                            nc.tensor.transpose(
                                tp[:, q, :], xl[:, kt * 128:(kt + 1) * 128], id_bf[:])
                        nc.vector.tensor_copy(
                            dst_tile[:, kt2 * 4:(kt2 + 1) * 4, dst_tt, :], tp[:])

                for n in range(4):
                    for j in range(4):
                        tt = 4 * n + j
                        ln_transpose(x_kv_d[tt * 128:(tt + 1) * 128, :],
                                     xlnT, tt)
                    # K matmuls for this 512-token block
                    for kc in range(8):
                        ps = apsk.tile([128, 512], F32, tag="kps")
                        for kt in range(HT):
                            nc.tensor.matmul(
                                ps[:], wk[:, kt, kc * 128:(kc + 1) * 128],
                                xlnT[:, kt, 4 * n:4 * n + 4, :],
                                start=(kt == 0), stop=(kt == HT - 1))
                        nc.scalar.activation(
                            KTb[:, kc, n * 512:(n + 1) * 512], ps[:],
                            AF.Identity, bias=bqkv_sb[:, 8 + kc:9 + kc])
                    # V matmuls for this block (token-major)
                    for j in range(4):
                        tt = 4 * n + j
                        for vh in range(2):
                            ps = apsv.tile([128, 512], F32, tag="vps")
                            for kt in range(HT):
                                nc.tensor.matmul(
                                    ps[:], xlnT[:, kt, tt, :],
                                    wv[:, kt, vh * 512:(vh + 1) * 512],
                                    start=(kt == 0), stop=(kt == HT - 1))
                            nc.vector.tensor_copy(
                                Vb[:, tt, vh * 8:(vh + 1) * 8, 0:64],
                                ps[:].rearrange("p (h c) -> p h c", h=8))
                    # own tokens: LN + transpose for Q (overlaps K/V matmuls)
                    for so in (2 * n, 2 * n + 1):
                        ln_transpose(x_ownr_d[so * 128:(so + 1) * 128, :],
                                     xownT, so)
                aw_cm.__exit__(None, None, None)

        # =========== Phase B: attention ===========
        attn_cm = tc.tile_pool(name="attn_p", bufs=1, side="right")
        attn_p = attn_cm.__enter__()
        attnT = attn_p.tile([128, HT, OWN], F32R)
        bmask_sb = attn_p.tile([128, 16 * 128], BF16)
        nc.sync.dma_start(bmask_sb[:], bmask_d[:])
        with (
            tc.tile_pool(name="b_p", bufs=7) as bp,
            tc.tile_pool(name="b_sc", bufs=3) as bsc,
            tc.tile_pool(name="b_wq", bufs=2) as bwq,
            tc.tile_pool(name="b_ps", bufs=2, space="PSUM") as bps,
            tc.tile_pool(name="b_pv", bufs=2, space="PSUM") as bpv,
        ):
            with nc.named_scope("phaseB"):
                # Software-pipelined: scores/exp/mask of unit j+1 are emitted
                # before the PV matmuls of unit j so the in-order PE never
                # waits on the Act-engine exp of the unit it is consuming.
                # Q-projection column blocks are interleaved as PE-only
                # pseudo-units before each head pair: they give the tensor
                # engine a dependency-free stretch that keeps its clock
                # ramped across head boundaries.
                pvs = {}   # h -> (pv0, pv1)

                def emit_q(qc):
                    wcol = bwq.tile([128, HT, 128], BF16, tag="wcol")
                    nc.sync.dma_start(
                        wcol[:], wq_d[:, qc * 128:(qc + 1) * 128]
                        .rearrange("(kt p) c -> p kt c", p=128))
                    for half in range(2):
                        ps = bps.tile([128, 1024], F32, tag="sc")
                        for kt in range(HT):
                            nc.tensor.matmul(
                                ps[:, 0:512], wcol[:, kt, :],
                                xownT[:, kt, half * 4:(half + 1) * 4, :],
                                start=(kt == 0), stop=(kt == HT - 1))
                        nc.vector.tensor_scalar(
                            QTb[:, qc, half * 512:(half + 1) * 512], ps[:, 0:512],
                            bqkv_sb[:, qc:qc + 1], None, AluOpType.add)

                def emit_scores(h, item):
                    hp, hb = h // 2, 64 * (h % 2)
                    if len(item) == 1:
                        kt = item[0]
                        a = FS[kt] * 128
                        sc = bps.tile([128, 1024], F32, tag="sc")
                        stat = KTb[hb:hb + 64, hp, kt * 128:(kt + 1) * 128]
                        nc.tensor.matmul(sc[:, a:512], stat,
                                         QTb[hb:hb + 64, hp, a:512],
                                         start=True, stop=True)
                        nc.tensor.matmul(sc[:, 512:1024], stat,
                                         QTb[hb:hb + 64, hp, 512:1024],
                                         start=True, stop=True)
                        p = bp.tile([128, 1024], BF16, tag="p")
                        nc.scalar.activation(p[:, a:1024], sc[:, a:1024],
                                             AF.Exp, scale=SCALE)
                        nc.vector.tensor_tensor(
                            p[:, a:a + 128], p[:, a:a + 128],
                            bmask_sb[:, kt * 128:(kt + 1) * 128],
                            AluOpType.mult)
                    else:
                        a = FS[item[0]] * 128
                        L = 1024 - a
                        sc = bps.tile([128, 2, 512], F32, tag="sc")
                        p = bp.tile([128, 2, 512], BF16, tag="p")
                        for jj, ktj in enumerate(item):
                            nc.tensor.matmul(
                                sc[:, jj, 0:L],
                                KTb[hb:hb + 64, hp, ktj * 128:(ktj + 1) * 128],
                                QTb[hb:hb + 64, hp, a:1024],
                                start=True, stop=True)
                        nc.scalar.activation(p[:, :, 0:L], sc[:, :, 0:L],
                                             AF.Exp, scale=SCALE)
                        for jj, ktj in enumerate(item):
                            nc.vector.tensor_tensor(
                                p[:, jj, 0:128], p[:, jj, 0:128],
                                bmask_sb[:, ktj * 128:(ktj + 1) * 128],
                                AluOpType.mult)
                    return p

                def emit_pv(h, item, p):
                    if h not in pvs:
                        pvs[h] = (bpv.tile([65, 512], F32, tag="pv0",
                                           name=f"pv0_{h}"),
                                  bpv.tile([65, 512], F32, tag="pv1",
                                           name=f"pv1_{h}"))
                    pv0, pv1 = pvs[h]
                    if len(item) == 1:
                        kt = item[0]
                        a = FS[kt] * 128
                        vstat = Vb[:, kt, h, :]
                        nc.tensor.matmul(pv0[:, a:512], vstat, p[:, a:512],
                                         start=(kt == 0), stop=(kt == 7))
                        nc.tensor.matmul(pv1[:, 0:512], vstat, p[:, 512:1024],
                                         start=(kt == 0), stop=False)
                    else:
                        a = FS[item[0]] * 128
                        L = 1024 - a
                        for jj, ktj in enumerate(item):
                            nc.tensor.matmul(
                                pv1[:, a - 512:a - 512 + L],
                                Vb[:, ktj, h, :], p[:, jj, 0:L],
                                start=False, stop=(ktj == 15))

                # Deferred, split normalization: recip / broadcast / multiply
                # are emitted one-per-unit a few units after the head's last
                # PV, so the in-order DVE queue never waits on the gpsimd
                # broadcast and the PE never waits on the DVE masks.
                def norm_tasks(h):
                    hp, hb = h // 2, 64 * (h % 2)
                    rds = [bsc.tile([1, 512], F32, tag="rd", name=f"rd{h}_{qb}")
                           for qb in range(2)]
                    bcs = [bsc.tile([64, 512], F32, tag="bc", name=f"bc{h}_{qb}")
                           for qb in range(2)]

                    def t_recip():
                        for qb in range(2):
                            nc.vector.reciprocal(rds[qb][:], pvs[h][qb][64:65, :])

                    def t_bcast():
                        for qb in range(2):
                            nc.gpsimd.partition_broadcast(bcs[qb][:], rds[qb][0:1, :])

                    def t_mult():
                        for qb in range(2):
                            nc.vector.tensor_tensor(
                                attnT[hb:hb + 64, hp, qb * 512:(qb + 1) * 512],
                                pvs[h][qb][0:64, :], bcs[qb][:], AluOpType.mult)
                    return [t_recip, t_bcast, t_mult]

                lagq = []      # pending (h, item, p) awaiting PV emission
                normq = []     # pending norm tasks, one popped per unit

                def advance(new=None):
                    if len(lagq) > 2 or (new is None and lagq):
                        h, item, p = lagq.pop(0)
                        emit_pv(h, item, p)
                        if item is SCHED[-1]:
                            normq.extend(norm_tasks(h))
                    if normq:
                        normq.pop(0)()
                    if new is not None:
                        lagq.append(new)

                for qc in range(8):
                    emit_q(qc)
                    for h in (2 * qc, 2 * qc + 1):
                        for item in SCHED:
                            p = emit_scores(h, item)
                            advance((h, item, p))
                while lagq or normq:
                    advance()

        # =========== Phase C: out-proj + residual + LN2 + y^T ===========
        kvq_cm.__exit__(None, None, None)
        hy_cm = tc.tile_pool(name="hy", bufs=1)
        hy = hy_cm.__enter__()
        h_sb = hy.tile([128, NSLOT, H], F32)
        yT_sb = hy.tile([128, HT, OWN], F32R)
        with (
            tc.tile_pool(name="c_w", bufs=2) as cw,
            tc.tile_pool(name="c_io", bufs=3) as cio,
            tc.tile_pool(name="c_ps", bufs=2, space="PSUM") as cps,
            tc.tile_pool(name="c_pst", bufs=4, space="PSUM") as cpst,
        ):
            with nc.named_scope("phaseC"):
                def ln2_transpose(tt):
                    yt = cio.tile([128, H], F32R, tag="yt")
                    layer_norm_apply(cio, h_sb[:, tt, :], yt[:])
                    for kt2 in range(2):
                        tp = cpst.tile([128, 4, 128], F32R, tag="tp2")
                        for q in range(4):
                            kt = kt2 * 4 + q
                            nc.tensor.transpose(
                                tp[:, q, :], yt[:, kt * 128:(kt + 1) * 128], id_r)
                        nc.vector.tensor_copy(
                            yT_sb[:, kt2 * 4:(kt2 + 1) * 4, tt * 128:(tt + 1) * 128],
                            tp[:])

                for n in range(2):
                    won = cw.tile([128, HT, 512], F32R, tag="won")
                    nc.sync.dma_start(
                        won[:], wout_d[:, n * 512:(n + 1) * 512]
                        .rearrange("(kt p) c -> p kt c", p=128))
                    for tt in range(NSLOT):
                        ps = cps.tile([128, 512], F32, tag="ops")
                        for kt in range(HT):
                            nc.tensor.matmul(ps[:], attnT[:, kt, tt * 128:(tt + 1) * 128],
                                             won[:, kt, :],
                                             start=(kt == 0), stop=(kt == HT - 1))
                        xo = cio.tile([128, 512], F32, tag="xo")
                        nc.sync.dma_start(
                            xo[:], x_own_d[tt * 128:(tt + 1) * 128, n * 512:(n + 1) * 512])
                        nc.vector.tensor_tensor(h_sb[:, tt, n * 512:(n + 1) * 512],
                                                ps[:], xo[:], AluOpType.add)
                        if n == 1 and tt >= 1:
                            ln2_transpose(tt - 1)
                ln2_transpose(NSLOT - 1)

        # =========== Phase D: router + experts ===========
        attn_cm.__exit__(None, None, None)
        h2p_cm = tc.tile_pool(name="h2p", bufs=1, side="right")
        h2p = h2p_cm.__enter__()
        h2_sb = h2p.tile([128, NSLOT, H], F32)
        rw_sb = h2p.tile([128, NSLOT, E], F32)
        rwT_f0 = h2p.tile([1, E, OWN], F32R)   # expert rows on partition 0
        rwT_r = h2p.tile([8, OWN], F32R)
        with (
            tc.tile_pool(name="d_sc", bufs=2) as dsc,
            tc.tile_pool(name="d_ps", bufs=2, space="PSUM") as dps,
            tc.tile_pool(name="d_pst", bufs=2, space="PSUM") as dpst,
        ):
            with nc.named_scope("phaseD_router"):
                wr = dsc.tile([128, HT, E], F32R, tag="wr")
                nc.sync.dma_start(wr[:], wrout_d.rearrange("(kt p) e -> p kt e", p=128))
                br = dsc.tile([128, E], F32, tag="br")
                nc.sync.dma_start(br[:], brout_d[:])
                for tt in range(NSLOT):
                    ps = dps.tile([128, E], F32, tag="rps")
                    for kt in range(HT):
                        nc.tensor.matmul(ps[:], yT_sb[:, kt, tt * 128:(tt + 1) * 128],
                                         wr[:, kt, :],
                                         start=(kt == 0), stop=(kt == HT - 1))
                    lg = dsc.tile([128, E], F32, tag="lg")
                    nc.vector.tensor_tensor(lg[:], ps[:], br[:], AluOpType.add)
                    ex = dsc.tile([128, E], F32, tag="ex")
                    rsum = dsc.tile([128, 1], F32, tag="rsum")
                    nc.scalar.activation(ex[:], lg[:], AF.Exp, accum_out=rsum[:])
                    rrec = dsc.tile([128, 1], F32, tag="rrec")
                    nc.vector.reciprocal(rrec[:], rsum[:])
                    nc.vector.tensor_scalar(rw_sb[:, tt, :], ex[:], rrec[:], None,
                                            AluOpType.mult)
                    tp = dpst.tile([8, 128], F32, tag="rtp")
                    nc.tensor.transpose(tp[:], rw_sb[:, tt, :], id_f[:])
                    nc.vector.tensor_copy(rwT_r[:, tt * 128:(tt + 1) * 128], tp[:])
                    nc.sync.dma_start(rwT_f0[0:1, :, tt * 128:(tt + 1) * 128],
                                      rwT_r[:, tt * 128:(tt + 1) * 128])

        with (
            tc.tile_pool(name="d_y", bufs=2) as dy,
            tc.tile_pool(name="d_w", bufs=4) as dw,
            tc.tile_pool(name="d_b", bufs=1) as db,
            tc.tile_pool(name="d_moe", bufs=1, space="PSUM") as dmoe,
        ):
            with nc.named_scope("phaseD_experts"):
                bexp_sb = db.tile([8, H], F32R)
                nc.sync.dma_start(bexp_sb[:], bexp_d[:])
                for fh in range(2):
                    mps = [dmoe.tile([128, 512], F32, tag=f"mps{tt}",
                                     name=f"mps_{fh}_{tt}")
                           for tt in range(NSLOT)]
                    for e in range(E):
                        bcr = dy.tile([128, OWN], F32R, tag="bcr")
                        nc.gpsimd.partition_broadcast(bcr[:], rwT_f0[0:1, e, :])
                        for kt in range(HT):
                            ye = dy.tile([128, OWN], F32R, tag="ye", bufs=4)
                            nc.vector.tensor_tensor(ye[:], yT_sb[:, kt, :],
                                                    bcr[:], AluOpType.mult)
                            we = dw.tile([128, 512], F32R, tag="we")
                            nc.sync.dma_start(
                                we[:],
                                wexp_d[e, kt * 128:(kt + 1) * 128,
                                       fh * 512:(fh + 1) * 512])
                            for tt in range(NSLOT):
                                nc.tensor.matmul(
                                    mps[tt][:], ye[:, tt * 128:(tt + 1) * 128],
                                    we[:],
                                    start=(e == 0 and kt == 0), stop=False)
                    for tt in range(NSLOT):
                        nc.tensor.matmul(mps[tt][:],
                                         rwT_r[:, tt * 128:(tt + 1) * 128],
                                         bexp_sb[:, fh * 512:(fh + 1) * 512],
                                         start=False, stop=True)
                        nc.vector.tensor_tensor(
                            h2_sb[:, tt, fh * 512:(fh + 1) * 512], mps[tt][:],
                            h_sb[:, tt, fh * 512:(fh + 1) * 512], AluOpType.add)

        # =========== Phase E: gate + LNf + output ===========
        hy_cm.__exit__(None, None, None)
        with (
            tc.tile_pool(name="e_sc", bufs=1) as esc,
            tc.tile_pool(name="e_tmp", bufs=2) as etmp,
            tc.tile_pool(name="e_pst", bufs=3, space="PSUM") as epst,
            tc.tile_pool(name="e_psg", bufs=2, space="PSUM") as epsg,
            tc.tile_pool(name="e_psal", bufs=1, space="PSUM") as epsal,
            tc.tile_pool(name="e_out", bufs=3) as eout,
        ):
            with nc.named_scope("phaseE"):
                mask_pp = esc.tile([128, NSLOT], F32)
                maskrow = esc.tile([1, OWN], F32)
                h2T = esc.tile([128, HT, OWN], F32R)
                for tt in range(NSLOT):
                    for kt2 in range(2):
                        tp = epst.tile([128, 4, 128], F32, tag="tp3")
                        for q in range(4):
                            kt = kt2 * 4 + q
                            nc.tensor.transpose(
                                tp[:, q, :], h2_sb[:, tt, kt * 128:(kt + 1) * 128],
                                id_f[:])
                        nc.vector.tensor_copy(
                            h2T[:, kt2 * 4:(kt2 + 1) * 4, tt * 128:(tt + 1) * 128],
                            tp[:])
                wal1 = esc.tile([128, HT, 256], F32R)
                nc.sync.dma_start(wal1[:], wal1_d.rearrange("(kt p) c -> p kt c", p=128))
                bal1 = esc.tile([128, 2], F32)
                nc.sync.dma_start(bal1[:], bal1_d[:])
                wal2 = esc.tile([128, 2, 1], F32R)
                nc.sync.dma_start(wal2[:], wal2_d.rearrange("(m p) c -> p m c", p=128))
                gT = esc.tile([128, 2, OWN], F32R)
                for m2 in range(2):
                    for n in range(2):
                        ps = epsg.tile([128, 512], F32, tag="gps")
                        for kt in range(HT):
                            nc.tensor.matmul(ps[:],
                                             wal1[:, kt, m2 * 128:(m2 + 1) * 128],
                                             h2T[:, kt, n * 512:(n + 1) * 512],
                                             start=(kt == 0), stop=(kt == HT - 1))
                        nc.scalar.activation(gT[:, m2, n * 512:(n + 1) * 512], ps[:],
                                             AF.Gelu, bias=bal1[:, m2:m2 + 1])
                for n in range(2):
                    ps = epsal.tile([1, 512], F32, tag="alps")
                    for m2 in range(2):
                        nc.tensor.matmul(ps[:], wal2[:, m2, :],
                                         gT[:, m2, n * 512:(n + 1) * 512],
                                         start=(m2 == 0), stop=(m2 == 1))
                    nc.vector.tensor_scalar(maskrow[:, n * 512:(n + 1) * 512], ps[:],
                                            thresh_sb[0:1, 0:1], None, AluOpType.is_gt)
                for tt in range(NSLOT):
                    nc.sync.dma_start(
                        mask_pp[:, tt:tt + 1],
                        maskrow[0:1, tt * 128:(tt + 1) * 128])
                for tt in range(NSLOT):
                    stats = eout.tile([128, 2, 6], F32, tag="st3")
                    nc.vector.bn_stats(stats[:, 0, :], h2_sb[:, tt, 0:512])
                    nc.vector.bn_stats(stats[:, 1, :], h2_sb[:, tt, 512:1024])
                    mv = eout.tile([128, 2], F32, tag="mv3")
                    nc.vector.bn_aggr(mv[:], stats[:])
                    sd = eout.tile([128, 1], F32, tag="sd3")
                    nc.scalar.activation(sd[:], mv[:, 1:2], AF.Sqrt, bias=eps_sb[:])
                    rstd = eout.tile([128, 1], F32, tag="rs3")
                    nc.vector.reciprocal(rstd[:], sd[:])
                    seff = eout.tile([128, 1], F32, tag="se3")
                    nc.vector.tensor_tensor(seff[:], rstd[:], mask_pp[:, tt:tt + 1],
                                            AluOpType.mult)
                    beff = eout.tile([128, 1], F32, tag="be3")
                    nc.vector.scalar_tensor_tensor(beff[:], mv[:, 0:1], -1.0, seff[:],
                                                   AluOpType.mult, AluOpType.mult)
                    ot = eout.tile([128, H], F32, tag="ot")
                    nc.scalar.activation(ot[:], h2_sb[:, tt, :], AF.Identity,
                                         bias=beff[:], scale=seff[:])
                    nc.sync.dma_start(out_d[tt * 128:(tt + 1) * 128, :], ot[:])

        h2p_cm.__exit__(None, None, None)
        small_cm.__exit__(None, None, None)

    nc.compile()
    return nc


def _prep_host(inputs):
    f32 = np.float32
    bf16 = ml_dtypes.bfloat16
    x = np.asarray(inputs["inputs"], f32)
    ln1_g = np.asarray(inputs["ln1_g"], f32); ln1_b = np.asarray(inputs["ln1_b"], f32)
    w_qkv = np.asarray(inputs["w_qkv"], f32); b_qkv = np.asarray(inputs["b_qkv"], f32)
    w_out = np.asarray(inputs["w_out"], f32); b_out = np.asarray(inputs["b_out"], f32)
    ln2_g = np.asarray(inputs["ln2_g"], f32); ln2_b = np.asarray(inputs["ln2_b"], f32)
    w_router = np.asarray(inputs["w_router"], f32)
    b_router = np.asarray(inputs["b_router"], f32)
    w_exp = np.asarray(inputs["w_exp"], f32); b_exp = np.asarray(inputs["b_exp"], f32)
    w_al1 = np.asarray(inputs["w_al1"], f32); b_al1 = np.asarray(inputs["b_al1"], f32)
    w_al2 = np.asarray(inputs["w_al2"], f32); b_al2 = np.asarray(inputs["b_al2"], f32)

    wq_f = (ln1_g[:, None] * w_qkv).astype(bf16)
    bq_f = b_qkv + ln1_b @ w_qkv
    assert np.all(bq_f[2 * H:] == 0.0), "nonzero V bias not supported"
    bqkv_t = np.zeros((128, 16), f32)
    for j in range(16):
        bqkv_t[:, j] = bq_f[j * 128:(j + 1) * 128]
    wr_f = ln2_g[:, None] * w_router
    br_f = b_router + ln2_b @ w_router
    we_f = ln2_g[None, :, None] * w_exp
    be_f = b_exp + np.einsum("h,ehf->ef", ln2_b, w_exp)
    x_pb = x + b_out[None, None, :]

    def mk_bmask(par):
        # first-suffix-block mask per key chunk kt: key j (row), query i (col)
        # within the block; slot s0 = kt//2 has global query chunk c = 2*s0+par.
        m = np.zeros((128, 16, 128), f32)
        for kt in range(16):
            c = 2 * (kt // 2) + par
            if c == kt:
                m[:, kt, :] = (np.arange(128)[:, None] <= np.arange(128)[None, :])
            elif c > kt:
                m[:, kt, :] = 1.0
            # c < kt: fully masked -> zeros
        return m.reshape(128, 16 * 128).astype(bf16)
    bmasks = [mk_bmask(0), mk_bmask(1)]

    thresh = np.full((128, 1), 0.8 - float(b_al2[0]), f32)
    bal1_t = np.zeros((128, 2), f32)
    bal1_t[:, 0] = b_al1[0:128]
    bal1_t[:, 1] = b_al1[128:256]

    shared = dict(
        wq=np.ascontiguousarray(wq_f),
        bqkv=bqkv_t,
        wout=np.ascontiguousarray(w_out),
        wrout=np.ascontiguousarray(wr_f.astype(f32)),
        brout=np.tile(br_f[None, :], (128, 1)).astype(f32),
        wexp=np.ascontiguousarray(we_f.astype(f32)),
        bexp=np.ascontiguousarray(be_f.astype(f32)),
        wal1=np.ascontiguousarray(w_al1),
        bal1=bal1_t,
        wal2=np.ascontiguousarray(w_al2),
        thresh=thresh,
    )
    per_core = []
    for c in range(N_CORES):
        b, par = c // 2, c % 2
        own_idx = np.concatenate(
            [np.arange(128) + (2 * s + par) * 128 for s in range(NSLOT)])
        m = dict(shared)
        m["x_kv"] = np.ascontiguousarray(x[b])
        m["x_ownr"] = np.ascontiguousarray(x[b][own_idx])
        m["x_own"] = np.ascontiguousarray(x_pb[b][own_idx])
        m["bmask"] = bmasks[par]
        per_core.append(m)
    return per_core


def kernel(**inputs):
    from concourse.bass_utils import run_bass_kernel_spmd

    if "prog" not in _prog_cache:
        _prog_cache["prog"] = _build_program()
    nc = _prog_cache["prog"]

    per_core = _prep_host(inputs)
    trace = bool(globals().get("TRACE", False))
    res = run_bass_kernel_spmd(nc, per_core, core_ids=list(range(N_CORES)),
                               trace=trace)
    _prog_cache["last_result"] = res

    lnf_g = np.asarray(inputs["lnf_g"], np.float32)
    lnf_b = np.asarray(inputs["lnf_b"], np.float32)
    out = np.zeros((B, S, H), np.float32)
    for c in range(N_CORES):
        b, par = c // 2, c % 2
        o = res.results[c]["out"]
        for s in range(NSLOT):
            g0 = (2 * s + par) * 128
            out[b, g0:g0 + 128, :] = o[s * 128:(s + 1) * 128, :]
    return out * lnf_g[None, None, :] + lnf_b[None, None, :]


# revision 46
# speedup vs baseline: 1.4377x; 1.1458x over previous
"""Trainium2 Bass kernel for nn_EnhancedTransformerBlock_80169859548047.

Sharding: 8 cores = (batch b, parity par). Core c handles batch b=c//2 and the
even (par=0) or odd (par=1) 128-token chunks of that batch's 2048-token
sequence. The attention schedule is parity-uniform: for key chunk kt the query
suffix starts at slot fs(kt)=kt//2; the first suffix block's causal mask
(triangular / all-ones / all-zeros, depending on parity) is host data, so the
instruction stream is identical on all cores.

Dtypes: attention path (LN1 out, w_qkv, Q/K/V, P) in bf16; out-proj, experts,
router and gate matmuls in float32r; residual stream and LN math in fp32.
Softmax denominators come from a ones column appended per head to V (exact
PSUM accumulation). LN1/LN2 affines are folded into the following weights on
the host; the final LN affine is applied on the host after gathering.
"""

import numpy as np
import ml_dtypes

B, S, H, E, NH, HD = 4, 2048, 1024, 8, 16, 64
N_CORES = 8
EPS = 1e-12
SCALE = HD ** -0.5
NSLOT = 8                # 128-token chunks per core
OWN = NSLOT * 128        # own tokens per core
HT = H // 128            # 8 H-tiles
FS = [kt // 2 for kt in range(16)]          # suffix first slot per key chunk
# attention schedule: key chunks 0..7 processed singly (suffix > 512 wide),
# 8..15 in pairs (same fs, suffix <= 512)
SCHED = [(0,), (1,), (2,), (3,), (4,), (5,), (6,), (7,),
         (8, 9), (10, 11), (12, 13), (14, 15)]

_prog_cache = {}


def _build_program():
    import concourse.bacc as bacc
    import concourse.tile as tile
    import concourse.mybir as mybir
    from concourse.masks import make_identity
    from concourse.alu_op_type import AluOpType
    from contextlib import ExitStack

    F32 = mybir.dt.float32
    F32R = mybir.dt.float32r
    BF16 = mybir.dt.bfloat16
    AF = mybir.ActivationFunctionType

    nc = bacc.Bacc("TRN2", target_bir_lowering=False, debug=False, num_devices=1)

    def din(name, shape, dt):
        return nc.dram_tensor(name, list(shape), dt, kind="ExternalInput").ap()

    x_kv_d = din("x_kv", (S, H), F32)
    x_ownr_d = din("x_ownr", (OWN, H), F32)   # raw inputs, own tokens, slot order
    x_own_d = din("x_own", (OWN, H), F32)     # inputs + b_out, own tokens
    wq_d = din("wq", (H, 3 * H), BF16)
    bqkv_d = din("bqkv", (128, 16), F32)
    wout_d = din("wout", (H, H), F32R)
    wrout_d = din("wrout", (H, E), F32R)
    brout_d = din("brout", (128, E), F32)
    wexp_d = din("wexp", (E, H, H), F32R)
    bexp_d = din("bexp", (E, H), F32R)
    wal1_d = din("wal1", (H, 256), F32R)
    bal1_d = din("bal1", (128, 2), F32)
    wal2_d = din("wal2", (256, 2), F32R)   # col 1 zero-padded
    bmask_d = din("bmask", (128, 16 * 128), BF16)  # per-kt first-block masks
    thresh_d = din("thresh", (128, 1), F32)   # 0.8 - b_al2, replicated
    out_d = nc.dram_tensor("out", [OWN, H], F32, kind="ExternalOutput").ap()

    with tile.TileContext(nc) as tc, ExitStack() as st:
        # manually-managed pools (non-LIFO lifetimes)
        small_cm = tc.tile_pool(name="small", bufs=1)
        small = small_cm.__enter__()
        id_bf = small.tile([128, 128], BF16)
        id_f = small.tile([128, 128], F32)
        bqkv_sb = small.tile([128, 16], F32)
        thresh_sb = small.tile([128, 1], F32)
        eps_sb = small.tile([128, 1], F32)
        nc.gpsimd.memset(eps_sb[:], EPS)
        nc.sync.dma_start(bqkv_sb[:], bqkv_d[:])
        nc.sync.dma_start(thresh_sb[:], thresh_d[:])
        id_r_t = small.tile([128, 128], F32R)
        make_identity(nc, id_bf[:])
        make_identity(nc, id_f[:])
        nc.vector.tensor_copy(id_r_t[:], id_f[:])
        id_r = id_r_t[:]

        kvq_cm = tc.tile_pool(name="kvq", bufs=1)
        kvq = kvq_cm.__enter__()
        KTb = kvq.tile([128, HT, S], BF16)            # K^T [kcol, tok]
        Vb = kvq.tile([128, 16, NH, 65], BF16)        # V token-major + ones col
        QTb = kvq.tile([128, HT, OWN], BF16)          # Q^T [qcol, own tok]
        xownT = kvq.tile([128, HT, NSLOT, 128], BF16)  # own tokens, LN1'd, ^T

        # =========== Phase A: LN1 + transpose + QKV ===========
        def layer_norm_apply(pool, src_ap, out_ap):
            stats = pool.tile([128, 2, 6], F32, tag="st")
            nc.vector.bn_stats(stats[:, 0, :], src_ap[:, 0:512])
            nc.vector.bn_stats(stats[:, 1, :], src_ap[:, 512:1024])
            mv = pool.tile([128, 2], F32, tag="mv")
            nc.vector.bn_aggr(mv[:], stats[:])
            sd = pool.tile([128, 1], F32, tag="sd")
            nc.scalar.activation(sd[:], mv[:, 1:2], AF.Sqrt, bias=eps_sb[:])
            rstd = pool.tile([128, 1], F32, tag="rs")
            nc.vector.reciprocal(rstd[:], sd[:])
            nbias = pool.tile([128, 1], F32, tag="nb")
            nc.vector.scalar_tensor_tensor(
                nbias[:], mv[:, 0:1], -1.0, rstd[:],
                AluOpType.mult, AluOpType.mult)
            nc.scalar.activation(out_ap, src_ap, AF.Identity,
                                 bias=nbias[:], scale=rstd[:])
            return mv, rstd

        with (
            tc.tile_pool(name="xln_pool", bufs=1) as xlnp,
            tc.tile_pool(name="a_io", bufs=3) as aio,
            tc.tile_pool(name="a_pst", bufs=3, space="PSUM") as apst,
            tc.tile_pool(name="a_psk", bufs=2, space="PSUM") as apsk,
            tc.tile_pool(name="a_psv", bufs=2, space="PSUM") as apsv,
        ):
            with nc.named_scope("phaseA"):
                # xlnT: [h-part, kt, global token chunk, col]
                xlnT = xlnp.tile([128, HT, 16, 128], BF16)
                aw_cm = tc.tile_pool(name="a_w", bufs=1)
                aw = aw_cm.__enter__()
                wk = aw.tile([128, HT, H], BF16)
                wv = aw.tile([128, HT, H], BF16)
                nc.sync.dma_start(
                    wk[:], wq_d[:, H:2 * H].rearrange("(kt p) c -> p kt c", p=128))
                nc.sync.dma_start(
                    wv[:], wq_d[:, 2 * H:3 * H].rearrange("(kt p) c -> p kt c", p=128))
                for tt in range(16):
                    nc.gpsimd.memset(Vb[:, tt, :, 64:65], 1.0)

                def ln_transpose(tt_src, dst_tile, dst_tt):
                    xt = aio.tile([128, H], F32, tag="xt")
                    nc.sync.dma_start(xt[:], tt_src)
                    xl = aio.tile([128, H], BF16, tag="xl")
                    layer_norm_apply(aio, xt[:], xl[:])
                    for kt2 in range(2):
                        tp = apst.tile([128, 4, 128], BF16, tag="tp")
                        for q in range(4):
                            kt = kt2 * 4 + q
                            nc.tensor.transpose(
                                tp[:, q, :], xl[:, kt * 128:(kt + 1) * 128], id_bf[:])
                        nc.vector.tensor_copy(
                            dst_tile[:, kt2 * 4:(kt2 + 1) * 4, dst_tt, :], tp[:])

                for n in range(4):
                    for j in range(4):
                        tt = 4 * n + j
                        ln_transpose(x_kv_d[tt * 128:(tt + 1) * 128, :],
                                     xlnT, tt)
                    # K matmuls for this 512-token block
                    for kc in range(8):
                        ps = apsk.tile([128, 512], F32, tag="kps")
                        for kt in range(HT):
                            nc.tensor.matmul(
                                ps[:], wk[:, kt, kc * 128:(kc + 1) * 128],
                                xlnT[:, kt, 4 * n:4 * n + 4, :],
                                start=(kt == 0), stop=(kt == HT - 1))
                        nc.scalar.activation(
                            KTb[:, kc, n * 512:(n + 1) * 512], ps[:],
                            AF.Identity, bias=bqkv_sb[:, 8 + kc:9 + kc])
                    # V matmuls for this block (token-major)
                    for j in range(4):
                        tt = 4 * n + j
                        for vh in range(2):
                            ps = apsv.tile([128, 512], F32, tag="vps")
                            for kt in range(HT):
                                nc.tensor.matmul(
                                    ps[:], xlnT[:, kt, tt, :],
                                    wv[:, kt, vh * 512:(vh + 1) * 512],
                                    start=(kt == 0), stop=(kt == HT - 1))
                            nc.vector.tensor_copy(
                                Vb[:, tt, vh * 8:(vh + 1) * 8, 0:64],
                                ps[:].rearrange("p (h c) -> p h c", h=8))
                    # own tokens: LN + transpose for Q (overlaps K/V matmuls)
                    for so in (2 * n, 2 * n + 1):
                        ln_transpose(x_ownr_d[so * 128:(so + 1) * 128, :],
                                     xownT, so)
                aw_cm.__exit__(None, None, None)

        # =========== Phase B: attention ===========
        attn_cm = tc.tile_pool(name="attn_p", bufs=1, side="right")
        attn_p = attn_cm.__enter__()
        attnT = attn_p.tile([128, HT, OWN], F32R)
        bmask_sb = attn_p.tile([128, 16 * 128], BF16)
        nc.sync.dma_start(bmask_sb[:], bmask_d[:])
        # out-proj weights, prefetched during attention
        wons = [attn_p.tile([128, HT, 512], F32R, name=f"won{n}") for n in range(2)]
        for n in range(2):
            nc.sync.dma_start(
                wons[n][:], wout_d[:, n * 512:(n + 1) * 512]
                .rearrange("(kt p) c -> p kt c", p=128))
        with (
            tc.tile_pool(name="b_p", bufs=7) as bp,
            tc.tile_pool(name="b_sc", bufs=3) as bsc,
            tc.tile_pool(name="b_wq", bufs=2) as bwq,
            tc.tile_pool(name="b_ps", bufs=2, space="PSUM") as bps,
            tc.tile_pool(name="b_pv", bufs=2, space="PSUM") as bpv,
        ):
            with nc.named_scope("phaseB"):
                # Software-pipelined: scores/exp/mask of unit j+1 are emitted
                # before the PV matmuls of unit j so the in-order PE never
                # waits on the Act-engine exp of the unit it is consuming.
                # Q-projection column blocks are interleaved as PE-only
                # pseudo-units before each head pair: they give the tensor
                # engine a dependency-free stretch that keeps its clock
                # ramped across head boundaries.
                pvs = {}   # h -> (pv0, pv1)

                def emit_q(qc):
                    wcol = bwq.tile([128, HT, 128], BF16, tag="wcol")
                    nc.sync.dma_start(
                        wcol[:], wq_d[:, qc * 128:(qc + 1) * 128]
                        .rearrange("(kt p) c -> p kt c", p=128))
                    for half in range(2):
                        ps = bps.tile([128, 1024], F32, tag="sc")
                        for kt in range(HT):
                            nc.tensor.matmul(
                                ps[:, 0:512], wcol[:, kt, :],
                                xownT[:, kt, half * 4:(half + 1) * 4, :],
                                start=(kt == 0), stop=(kt == HT - 1))
                        nc.vector.tensor_scalar(
                            QTb[:, qc, half * 512:(half + 1) * 512], ps[:, 0:512],
                            bqkv_sb[:, qc:qc + 1], None, AluOpType.add)

                def emit_scores(h, item):
                    hp, hb = h // 2, 64 * (h % 2)
                    if len(item) == 1:
                        kt = item[0]
                        a = FS[kt] * 128
                        sc = bps.tile([128, 1024], F32, tag="sc")
                        stat = KTb[hb:hb + 64, hp, kt * 128:(kt + 1) * 128]
                        nc.tensor.matmul(sc[:, a:512], stat,
                                         QTb[hb:hb + 64, hp, a:512],
                                         start=True, stop=True)
                        nc.tensor.matmul(sc[:, 512:1024], stat,
                                         QTb[hb:hb + 64, hp, 512:1024],
                                         start=True, stop=True)
                        p = bp.tile([128, 1024], BF16, tag="p")
                        nc.scalar.activation(p[:, a:1024], sc[:, a:1024],
                                             AF.Exp, scale=SCALE)
                        nc.vector.tensor_tensor(
                            p[:, a:a + 128], p[:, a:a + 128],
                            bmask_sb[:, kt * 128:(kt + 1) * 128],
                            AluOpType.mult)
                    else:
                        a = FS[item[0]] * 128
                        L = 1024 - a
                        sc = bps.tile([128, 2, 512], F32, tag="sc")
                        p = bp.tile([128, 2, 512], BF16, tag="p")
                        for jj, ktj in enumerate(item):
                            nc.tensor.matmul(
                                sc[:, jj, 0:L],
                                KTb[hb:hb + 64, hp, ktj * 128:(ktj + 1) * 128],
                                QTb[hb:hb + 64, hp, a:1024],
                                start=True, stop=True)
                        nc.scalar.activation(p[:, :, 0:L], sc[:, :, 0:L],
                                             AF.Exp, scale=SCALE)
                        for jj, ktj in enumerate(item):
                            nc.vector.tensor_tensor(
                                p[:, jj, 0:128], p[:, jj, 0:128],
                                bmask_sb[:, ktj * 128:(ktj + 1) * 128],
                                AluOpType.mult)
                    return p

                def emit_pv(h, item, p):
                    if h not in pvs:
                        pvs[h] = (bpv.tile([65, 512], F32, tag="pv0",
                                           name=f"pv0_{h}"),
                                  bpv.tile([65, 512], F32, tag="pv1",
                                           name=f"pv1_{h}"))
                    pv0, pv1 = pvs[h]
                    if len(item) == 1:
                        kt = item[0]
                        a = FS[kt] * 128
                        vstat = Vb[:, kt, h, :]
                        nc.tensor.matmul(pv0[:, a:512], vstat, p[:, a:512],
                                         start=(kt == 0), stop=(kt == 7))
                        nc.tensor.matmul(pv1[:, 0:512], vstat, p[:, 512:1024],
                                         start=(kt == 0), stop=False)
                    else:
                        a = FS[item[0]] * 128
                        L = 1024 - a
                        for jj, ktj in enumerate(item):
                            nc.tensor.matmul(
                                pv1[:, a - 512:a - 512 + L],
                                Vb[:, ktj, h, :], p[:, jj, 0:L],
                                start=False, stop=(ktj == 15))

                # Deferred, split normalization: recip / broadcast / multiply
                # are emitted one-per-unit a few units after the head's last
                # PV, so the in-order DVE queue never waits on the gpsimd
                # broadcast and the PE never waits on the DVE masks.
                def norm_tasks(h):
                    hp, hb = h // 2, 64 * (h % 2)
                    rdt = [bsc.tile([1, 512], F32, tag="rdt", name=f"rdt{h}_{qb}")
                           for qb in range(2)]
                    rds = [bsc.tile([1, 512], F32, tag="rd", name=f"rd{h}_{qb}")
                           for qb in range(2)]
                    bcs = [bsc.tile([64, 512], F32, tag="bc", name=f"bc{h}_{qb}")
                           for qb in range(2)]

                    def t_recip(qb):
                        def run():
                            nc.vector.tensor_copy(rdt[qb][:], pvs[h][qb][64:65, :])
                            nc.vector.reciprocal_approx_fast(rds[qb][:], rdt[qb][:])
                        return run

                    def t_bcast(qb):
                        def run():
                            nc.gpsimd.partition_broadcast(bcs[qb][:], rds[qb][0:1, :])
                        return run

                    def t_mult(qb):
                        def run():
                            nc.vector.tensor_tensor(
                                attnT[hb:hb + 64, hp, qb * 512:(qb + 1) * 512],
                                pvs[h][qb][0:64, :], bcs[qb][:], AluOpType.mult)
                        return run
                    return [t_recip(0), t_recip(1), t_bcast(0), t_bcast(1),
                            t_mult(0), t_mult(1)]

                lagq = []      # pending (h, item, p) awaiting PV emission
                normq = []     # pending norm tasks, one popped per unit

                def advance(new=None):
                    if len(lagq) > 2 or (new is None and lagq):
                        h, item, p = lagq.pop(0)
                        emit_pv(h, item, p)
                        if item is SCHED[-1]:
                            normq.extend(norm_tasks(h))
                    if normq:
                        normq.pop(0)()
                    if new is not None:
                        lagq.append(new)

                for qc in range(8):
                    emit_q(qc)
                    for h in (2 * qc, 2 * qc + 1):
                        for item in SCHED:
                            p = emit_scores(h, item)
                            advance((h, item, p))
                while lagq or normq:
                    advance()

        # =========== Phase C: out-proj + residual + LN2 + y^T ===========
        kvq_cm.__exit__(None, None, None)
        hy_cm = tc.tile_pool(name="hy", bufs=1)
        hy = hy_cm.__enter__()
        h_sb = hy.tile([128, NSLOT, H], F32)
        yT_sb = hy.tile([128, HT, OWN], F32R)
        rwp_cm = tc.tile_pool(name="rwp", bufs=1)
        rwp = rwp_cm.__enter__()
        rw_sb = rwp.tile([128, NSLOT, E], F32)
        rlg_sb = rwp.tile([128, NSLOT, E], F32)
        rwT_f0 = rwp.tile([1, E, OWN], F32R)   # expert rows on partition 0
        rwT_r = rwp.tile([8, OWN], F32R)
        wr = rwp.tile([128, HT, E], F32R)
        nc.sync.dma_start(wr[:], wrout_d.rearrange("(kt p) e -> p kt e", p=128))
        br = rwp.tile([128, E], F32)
        nc.sync.dma_start(br[:], brout_d[:])
        with (
            tc.tile_pool(name="c_io", bufs=3) as cio,
            tc.tile_pool(name="c_ps", bufs=2, space="PSUM") as cps,
            tc.tile_pool(name="c_pst", bufs=3, space="PSUM") as cpst,
            tc.tile_pool(name="c_pr", bufs=2, space="PSUM") as cpr,
        ):
            with nc.named_scope("phaseC"):
                def ln2_transpose(tt):
                    yt = cio.tile([128, H], F32R, tag="yt")
                    layer_norm_apply(cio, h_sb[:, tt, :], yt[:])
                    for kt2 in range(2):
                        tp = cpst.tile([128, 4, 128], F32R, tag="tp2")
                        for q in range(4):
                            kt = kt2 * 4 + q
                            nc.tensor.transpose(
                                tp[:, q, :], yt[:, kt * 128:(kt + 1) * 128], id_r)
                        nc.vector.tensor_copy(
                            yT_sb[:, kt2 * 4:(kt2 + 1) * 4, tt * 128:(tt + 1) * 128],
                            tp[:])

                def router_logits(tt):
                    ps = cpr.tile([128, E], F32, tag="rps")
                    for kt in range(HT):
                        nc.tensor.matmul(ps[:], yT_sb[:, kt, tt * 128:(tt + 1) * 128],
                                         wr[:, kt, :],
                                         start=(kt == 0), stop=(kt == HT - 1))
                    nc.vector.tensor_tensor(rlg_sb[:, tt, :], ps[:], br[:],
                                            AluOpType.add)

                for n in range(2):
                    for tt in range(NSLOT):
                        ps = cps.tile([128, 512], F32, tag="ops")
                        for kt in range(HT):
                            nc.tensor.matmul(ps[:], attnT[:, kt, tt * 128:(tt + 1) * 128],
                                             wons[n][:, kt, :],
                                             start=(kt == 0), stop=(kt == HT - 1))
                        xo = cio.tile([128, 512], F32, tag="xo")
                        nc.sync.dma_start(
                            xo[:], x_own_d[tt * 128:(tt + 1) * 128, n * 512:(n + 1) * 512])
                        nc.vector.tensor_tensor(h_sb[:, tt, n * 512:(n + 1) * 512],
                                                ps[:], xo[:], AluOpType.add)
                        if n == 1:
                            if tt >= 1:
                                ln2_transpose(tt - 1)
                            if tt >= 2:
                                router_logits(tt - 2)
                ln2_transpose(NSLOT - 1)
                router_logits(NSLOT - 2)
                router_logits(NSLOT - 1)

        # =========== Phase D: router softmax + experts ===========
        attn_cm.__exit__(None, None, None)
        h2p_cm = tc.tile_pool(name="h2p", bufs=1, side="right")
        h2p = h2p_cm.__enter__()
        h2_sb = h2p.tile([128, NSLOT, H], F32)
        with (
            tc.tile_pool(name="d_sc", bufs=2) as dsc,
            tc.tile_pool(name="d_pst", bufs=2, space="PSUM") as dpst,
        ):
            with nc.named_scope("phaseD_router"):
                for tt in range(NSLOT):
                    ex = dsc.tile([128, E], F32, tag="ex")
                    rsum = dsc.tile([128, 1], F32, tag="rsum")
                    nc.scalar.activation(ex[:], rlg_sb[:, tt, :], AF.Exp,
                                         accum_out=rsum[:])
                    rrec = dsc.tile([128, 1], F32, tag="rrec")
                    nc.vector.reciprocal(rrec[:], rsum[:])
                    nc.vector.tensor_scalar(rw_sb[:, tt, :], ex[:], rrec[:], None,
                                            AluOpType.mult)
                    tp = dpst.tile([8, 128], F32, tag="rtp")
                    nc.tensor.transpose(tp[:], rw_sb[:, tt, :], id_f[:])
                    nc.vector.tensor_copy(rwT_r[:, tt * 128:(tt + 1) * 128], tp[:])
                    nc.sync.dma_start(rwT_f0[0:1, :, tt * 128:(tt + 1) * 128],
                                      rwT_r[:, tt * 128:(tt + 1) * 128])

        with (
            tc.tile_pool(name="d_y", bufs=2) as dy,
            tc.tile_pool(name="d_w", bufs=4) as dw,
            tc.tile_pool(name="d_b", bufs=1) as db,
            tc.tile_pool(name="d_moe", bufs=1, space="PSUM") as dmoe,
        ):
            with nc.named_scope("phaseD_experts"):
                bexp_sb = db.tile([8, H], F32R)
                nc.sync.dma_start(bexp_sb[:], bexp_d[:])
                for fh in range(2):
                    mps = [dmoe.tile([128, 512], F32, tag=f"mps{tt}",
                                     name=f"mps_{fh}_{tt}")
                           for tt in range(NSLOT)]
                    for e in range(E):
                        bcr = dy.tile([128, OWN], F32R, tag="bcr")
                        nc.gpsimd.partition_broadcast(bcr[:], rwT_f0[0:1, e, :])
                        for kt in range(HT):
                            ye = dy.tile([128, OWN], F32R, tag="ye", bufs=4)
                            nc.vector.tensor_tensor(ye[:], yT_sb[:, kt, :],
                                                    bcr[:], AluOpType.mult)
                            we = dw.tile([128, 512], F32R, tag="we")
                            nc.sync.dma_start(
                                we[:],
                                wexp_d[e, kt * 128:(kt + 1) * 128,
                                       fh * 512:(fh + 1) * 512])
                            for tt in range(NSLOT):
                                nc.tensor.matmul(
                                    mps[tt][:], ye[:, tt * 128:(tt + 1) * 128],
                                    we[:],
                                    start=(e == 0 and kt == 0), stop=False)
                    for tt in range(NSLOT):
                        nc.tensor.matmul(mps[tt][:],
                                         rwT_r[:, tt * 128:(tt + 1) * 128],
                                         bexp_sb[:, fh * 512:(fh + 1) * 512],
                                         start=False, stop=True)
                        nc.vector.tensor_tensor(
                            h2_sb[:, tt, fh * 512:(fh + 1) * 512], mps[tt][:],
                            h_sb[:, tt, fh * 512:(fh + 1) * 512], AluOpType.add)

        # =========== Phase E: gate + LNf + output ===========
        rwp_cm.__exit__(None, None, None)
        hy_cm.__exit__(None, None, None)
        with (
            tc.tile_pool(name="e_sc", bufs=1) as esc,
            tc.tile_pool(name="e_tmp", bufs=2) as etmp,
            tc.tile_pool(name="e_pst", bufs=3, space="PSUM") as epst,
            tc.tile_pool(name="e_psg", bufs=2, space="PSUM") as epsg,
            tc.tile_pool(name="e_psal", bufs=1, space="PSUM") as epsal,
            tc.tile_pool(name="e_out", bufs=3) as eout,
        ):
            with nc.named_scope("phaseE"):
                mask_pp = esc.tile([128, NSLOT], F32)
                h2T = esc.tile([128, HT, OWN], F32R)
                for tt in range(NSLOT):
                    for kt2 in range(2):
                        tp = epst.tile([128, 4, 128], F32, tag="tp3")
                        for q in range(4):
                            kt = kt2 * 4 + q
                            nc.tensor.transpose(
                                tp[:, q, :], h2_sb[:, tt, kt * 128:(kt + 1) * 128],
                                id_f[:])
                        nc.vector.tensor_copy(
                            h2T[:, kt2 * 4:(kt2 + 1) * 4, tt * 128:(tt + 1) * 128],
                            tp[:])
                wal1 = esc.tile([128, HT, 256], F32R)
                nc.sync.dma_start(wal1[:], wal1_d.rearrange("(kt p) c -> p kt c", p=128))
                bal1 = esc.tile([128, 2], F32)
                nc.sync.dma_start(bal1[:], bal1_d[:])
                wal2 = esc.tile([128, 2, 2], F32R)
                nc.sync.dma_start(wal2[:], wal2_d.rearrange("(m p) c -> p m c", p=128))
                gT = esc.tile([128, 2, OWN], F32R)
                for m2 in range(2):
                    for n in range(2):
                        ps = epsg.tile([128, 512], F32, tag="gps")
                        for kt in range(HT):
                            nc.tensor.matmul(ps[:],
                                             wal1[:, kt, m2 * 128:(m2 + 1) * 128],
                                             h2T[:, kt, n * 512:(n + 1) * 512],
                                             start=(kt == 0), stop=(kt == HT - 1))
                        nc.scalar.activation(gT[:, m2, n * 512:(n + 1) * 512], ps[:],
                                             AF.Gelu, bias=bal1[:, m2:m2 + 1])
                for tt in range(NSLOT):
                    ps = epsal.tile([128, 2], F32, tag="alps")
                    for m2 in range(2):
                        nc.tensor.matmul(ps[:], gT[:, m2, tt * 128:(tt + 1) * 128],
                                         wal2[:, m2, :],
                                         start=(m2 == 0), stop=(m2 == 1))
                    nc.vector.tensor_scalar(mask_pp[:, tt:tt + 1], ps[:, 0:1],
                                            thresh_sb[:, 0:1], None, AluOpType.is_gt)
                for tt in range(NSLOT):
                    stats = eout.tile([128, 2, 6], F32, tag="st3")
                    nc.vector.bn_stats(stats[:, 0, :], h2_sb[:, tt, 0:512])
                    nc.vector.bn_stats(stats[:, 1, :], h2_sb[:, tt, 512:1024])
                    mv = eout.tile([128, 2], F32, tag="mv3")
                    nc.vector.bn_aggr(mv[:], stats[:])
                    sd = eout.tile([128, 1], F32, tag="sd3")
                    nc.scalar.activation(sd[:], mv[:, 1:2], AF.Sqrt, bias=eps_sb[:])
                    rstd = eout.tile([128, 1], F32, tag="rs3")
                    nc.vector.reciprocal(rstd[:], sd[:])
                    seff = eout.tile([128, 1], F32, tag="se3")
                    nc.vector.tensor_tensor(seff[:], rstd[:], mask_pp[:, tt:tt + 1],
                                            AluOpType.mult)
                    beff = eout.tile([128, 1], F32, tag="be3")
                    nc.vector.scalar_tensor_tensor(beff[:], mv[:, 0:1], -1.0, seff[:],
                                                   AluOpType.mult, AluOpType.mult)
                    ot = eout.tile([128, H], F32, tag="ot")
                    nc.scalar.activation(ot[:], h2_sb[:, tt, :], AF.Identity,
                                         bias=beff[:], scale=seff[:])
                    nc.sync.dma_start(out_d[tt * 128:(tt + 1) * 128, :], ot[:])

        h2p_cm.__exit__(None, None, None)
        small_cm.__exit__(None, None, None)

    nc.compile()
    return nc


def _prep_host(inputs):
    f32 = np.float32
    bf16 = ml_dtypes.bfloat16
    x = np.asarray(inputs["inputs"], f32)
    ln1_g = np.asarray(inputs["ln1_g"], f32); ln1_b = np.asarray(inputs["ln1_b"], f32)
    w_qkv = np.asarray(inputs["w_qkv"], f32); b_qkv = np.asarray(inputs["b_qkv"], f32)
    w_out = np.asarray(inputs["w_out"], f32); b_out = np.asarray(inputs["b_out"], f32)
    ln2_g = np.asarray(inputs["ln2_g"], f32); ln2_b = np.asarray(inputs["ln2_b"], f32)
    w_router = np.asarray(inputs["w_router"], f32)
    b_router = np.asarray(inputs["b_router"], f32)
    w_exp = np.asarray(inputs["w_exp"], f32); b_exp = np.asarray(inputs["b_exp"], f32)
    w_al1 = np.asarray(inputs["w_al1"], f32); b_al1 = np.asarray(inputs["b_al1"], f32)
    w_al2 = np.asarray(inputs["w_al2"], f32); b_al2 = np.asarray(inputs["b_al2"], f32)

    wq_f = (ln1_g[:, None] * w_qkv).astype(bf16)
    bq_f = b_qkv + ln1_b @ w_qkv
    assert np.all(bq_f[2 * H:] == 0.0), "nonzero V bias not supported"
    bqkv_t = np.zeros((128, 16), f32)
    for j in range(16):
        bqkv_t[:, j] = bq_f[j * 128:(j + 1) * 128]
    wr_f = ln2_g[:, None] * w_router
    br_f = b_router + ln2_b @ w_router
    we_f = ln2_g[None, :, None] * w_exp
    be_f = b_exp + np.einsum("h,ehf->ef", ln2_b, w_exp)
    x_pb = x + b_out[None, None, :]

    def mk_bmask(par):
        # first-suffix-block mask per key chunk kt: key j (row), query i (col)
        # within the block; slot s0 = kt//2 has global query chunk c = 2*s0+par.
        m = np.zeros((128, 16, 128), f32)
        for kt in range(16):
            c = 2 * (kt // 2) + par
            if c == kt:
                m[:, kt, :] = (np.arange(128)[:, None] <= np.arange(128)[None, :])
            elif c > kt:
                m[:, kt, :] = 1.0
            # c < kt: fully masked -> zeros
        return m.reshape(128, 16 * 128).astype(bf16)
    bmasks = [mk_bmask(0), mk_bmask(1)]

    thresh = np.full((128, 1), 0.8 - float(b_al2[0]), f32)
    bal1_t = np.zeros((128, 2), f32)
    bal1_t[:, 0] = b_al1[0:128]
    bal1_t[:, 1] = b_al1[128:256]

    shared = dict(
        wq=np.ascontiguousarray(wq_f),
        bqkv=bqkv_t,
        wout=np.ascontiguousarray(w_out),
        wrout=np.ascontiguousarray(wr_f.astype(f32)),
        brout=np.tile(br_f[None, :], (128, 1)).astype(f32),
        wexp=np.ascontiguousarray(we_f.astype(f32)),
        bexp=np.ascontiguousarray(be_f.astype(f32)),
        wal1=np.ascontiguousarray(w_al1),
        bal1=bal1_t,
        wal2=np.ascontiguousarray(
            np.concatenate([w_al2, np.zeros_like(w_al2)], axis=1)),
        thresh=thresh,
    )
    per_core = []
    for c in range(N_CORES):
        b, par = c // 2, c % 2
        own_idx = np.concatenate(
            [np.arange(128) + (2 * s + par) * 128 for s in range(NSLOT)])
        m = dict(shared)
        m["x_kv"] = np.ascontiguousarray(x[b])
        m["x_ownr"] = np.ascontiguousarray(x[b][own_idx])
        m["x_own"] = np.ascontiguousarray(x_pb[b][own_idx])
        m["bmask"] = bmasks[par]
        per_core.append(m)
    return per_core


def kernel(**inputs):
    from concourse.bass_utils import run_bass_kernel_spmd

    if "prog" not in _prog_cache:
        _prog_cache["prog"] = _build_program()
    nc = _prog_cache["prog"]

    per_core = _prep_host(inputs)
    trace = bool(globals().get("TRACE", False))
    res = run_bass_kernel_spmd(nc, per_core, core_ids=list(range(N_CORES)),
                               trace=trace)
    _prog_cache["last_result"] = res

    lnf_g = np.asarray(inputs["lnf_g"], np.float32)
    lnf_b = np.asarray(inputs["lnf_b"], np.float32)
    out = np.zeros((B, S, H), np.float32)
    for c in range(N_CORES):
        b, par = c // 2, c % 2
        o = res.results[c]["out"]
        for s in range(NSLOT):
            g0 = (2 * s + par) * 128
            out[b, g0:g0 + 128, :] = o[s * 128:(s + 1) * 128, :]
    return out * lnf_g[None, None, :] + lnf_b[None, None, :]


# revision 52
# speedup vs baseline: 1.5881x; 1.1046x over previous
"""Trainium2 Bass kernel for nn_EnhancedTransformerBlock_80169859548047.

Sharding: 8 cores = (batch b, parity par). Core c handles batch b=c//2 and the
even (par=0) or odd (par=1) 128-token chunks of that batch's 2048-token
sequence. The attention schedule is parity-uniform: for key chunk kt the query
suffix starts at slot fs(kt)=kt//2; the first suffix block's causal mask
(triangular / all-ones / all-zeros, depending on parity) is host data, so the
instruction stream is identical on all cores.

Dtypes: attention path (LN1 out, w_qkv, Q/K/V, P) in bf16; out-proj, experts,
router and gate matmuls in float32r; residual stream and LN math in fp32.
Softmax denominators come from a ones column appended per head to V (exact
PSUM accumulation). LN1/LN2 affines are folded into the following weights on
the host; the final LN affine is applied on the host after gathering.
"""

import numpy as np
import ml_dtypes

B, S, H, E, NH, HD = 4, 2048, 1024, 8, 16, 64
N_CORES = 8
EPS = 1e-12
SCALE = HD ** -0.5
NSLOT = 8                # 128-token chunks per core
OWN = NSLOT * 128        # own tokens per core
HT = H // 128            # 8 H-tiles
FS = [kt // 2 for kt in range(16)]          # suffix first slot per key chunk
# attention schedule: key chunks 0..7 processed singly (suffix > 512 wide),
# 8..15 in pairs (same fs, suffix <= 512)
SCHED = [(0,), (1,), (2,), (3,), (4,), (5,), (6,), (7,),
         (8, 9), (10, 11), (12, 13), (14, 15)]

_prog_cache = {}


def _build_program():
    import concourse.bacc as bacc
    import concourse.tile as tile
    import concourse.mybir as mybir
    from concourse.masks import make_identity
    from concourse.alu_op_type import AluOpType
    from contextlib import ExitStack

    F32 = mybir.dt.float32
    F32R = mybir.dt.float32r
    BF16 = mybir.dt.bfloat16
    AF = mybir.ActivationFunctionType

    nc = bacc.Bacc("TRN2", target_bir_lowering=False, debug=False, num_devices=1)

    def din(name, shape, dt):
        return nc.dram_tensor(name, list(shape), dt, kind="ExternalInput").ap()

    x_kv_d = din("x_kv", (S, H), F32)
    x_ownr_d = din("x_ownr", (OWN, H), F32)   # raw inputs, own tokens, slot order
    x_own_d = din("x_own", (OWN, H), F32)     # inputs + b_out, own tokens
    wq_d = din("wq", (H, 3 * H), BF16)
    bqkv_d = din("bqkv", (128, 16), F32)
    wout_d = din("wout", (H, H), F32R)
    wrout_d = din("wrout", (H, E), F32R)
    brout_d = din("brout", (128, E), F32)
    wexp_d = din("wexp", (E, H, H), F32R)
    bexp_d = din("bexp", (E, H), F32R)
    wal1_d = din("wal1", (H, 256), F32R)
    bal1_d = din("bal1", (128, 2), F32)
    wal2_d = din("wal2", (256, 2), F32R)   # col 1 zero-padded
    bmask_d = din("bmask", (128, 16 * 2 * 128), BF16)  # per-kt masks, x2 heads
    thresh_d = din("thresh", (128, 1), F32)   # 0.8 - b_al2, replicated
    out_d = nc.dram_tensor("out", [OWN, H], F32, kind="ExternalOutput").ap()

    with tile.TileContext(nc) as tc, ExitStack() as st:
        # manually-managed pools (non-LIFO lifetimes)
        small_cm = tc.tile_pool(name="small", bufs=1)
        small = small_cm.__enter__()
        id_bf = small.tile([128, 128], BF16)
        id_f = small.tile([128, 128], F32)
        bqkv_sb = small.tile([128, 16], F32)
        thresh_sb = small.tile([128, 1], F32)
        eps_sb = small.tile([128, 1], F32)
        nc.gpsimd.memset(eps_sb[:], EPS)
        nc.sync.dma_start(bqkv_sb[:], bqkv_d[:])
        nc.sync.dma_start(thresh_sb[:], thresh_d[:])
        id_r_t = small.tile([128, 128], F32R)
        make_identity(nc, id_bf[:])
        make_identity(nc, id_f[:])
        nc.vector.tensor_copy(id_r_t[:], id_f[:])
        id_r = id_r_t[:]

        kvq_cm = tc.tile_pool(name="kvq", bufs=1)
        kvq = kvq_cm.__enter__()
        KTb = kvq.tile([128, HT, S], BF16)            # K^T [kcol, tok]
        Vb = kvq.tile([128, 16, NH, 65], BF16)        # V token-major + ones col
        QTb = kvq.tile([128, HT, OWN], BF16)          # Q^T [qcol, own tok]
        xownT = kvq.tile([128, HT, NSLOT, 128], BF16)  # own tokens, LN1'd, ^T

        # =========== Phase A: LN1 + transpose + QKV ===========
        def layer_norm_apply(pool, src_ap, out_ap):
            stats = pool.tile([128, 2, 6], F32, tag="st")
            nc.vector.bn_stats(stats[:, 0, :], src_ap[:, 0:512])
            nc.vector.bn_stats(stats[:, 1, :], src_ap[:, 512:1024])
            mv = pool.tile([128, 2], F32, tag="mv")
            nc.vector.bn_aggr(mv[:], stats[:])
            sd = pool.tile([128, 1], F32, tag="sd")
            nc.scalar.activation(sd[:], mv[:, 1:2], AF.Sqrt, bias=eps_sb[:])
            rstd = pool.tile([128, 1], F32, tag="rs")
            nc.vector.reciprocal(rstd[:], sd[:])
            nbias = pool.tile([128, 1], F32, tag="nb")
            nc.vector.scalar_tensor_tensor(
                nbias[:], mv[:, 0:1], -1.0, rstd[:],
                AluOpType.mult, AluOpType.mult)
            nc.scalar.activation(out_ap, src_ap, AF.Identity,
                                 bias=nbias[:], scale=rstd[:])
            return mv, rstd

        with (
            tc.tile_pool(name="xln_pool", bufs=1) as xlnp,
            tc.tile_pool(name="a_io", bufs=3) as aio,
            tc.tile_pool(name="a_pst", bufs=3, space="PSUM") as apst,
            tc.tile_pool(name="a_psk", bufs=2, space="PSUM") as apsk,
            tc.tile_pool(name="a_psv", bufs=2, space="PSUM") as apsv,
        ):
            with nc.named_scope("phaseA"):
                # xlnT: [h-part, kt, global token chunk, col]
                xlnT = xlnp.tile([128, HT, 16, 128], BF16)
                aw_cm = tc.tile_pool(name="a_w", bufs=1)
                aw = aw_cm.__enter__()
                wk = aw.tile([128, HT, H], BF16)
                wv = aw.tile([128, HT, H], BF16)
                nc.sync.dma_start(
                    wk[:], wq_d[:, H:2 * H].rearrange("(kt p) c -> p kt c", p=128))
                nc.sync.dma_start(
                    wv[:], wq_d[:, 2 * H:3 * H].rearrange("(kt p) c -> p kt c", p=128))
                for tt in range(16):
                    nc.gpsimd.memset(Vb[:, tt, :, 64:65], 1.0)

                def ln_transpose(tt_src, dst_tile, dst_tt):
                    xt = aio.tile([128, H], F32, tag="xt")
                    nc.sync.dma_start(xt[:], tt_src)
                    xl = aio.tile([128, H], BF16, tag="xl")
                    layer_norm_apply(aio, xt[:], xl[:])
                    for kt2 in range(2):
                        tp = apst.tile([128, 4, 128], BF16, tag="tp")
                        for q in range(4):
                            kt = kt2 * 4 + q
                            nc.tensor.transpose(
                                tp[:, q, :], xl[:, kt * 128:(kt + 1) * 128], id_bf[:])
                        nc.vector.tensor_copy(
                            dst_tile[:, kt2 * 4:(kt2 + 1) * 4, dst_tt, :], tp[:])

                for n in range(4):
                    for j in range(4):
                        tt = 4 * n + j
                        ln_transpose(x_kv_d[tt * 128:(tt + 1) * 128, :],
                                     xlnT, tt)
                    # K matmuls for this 512-token block
                    for kc in range(8):
                        ps = apsk.tile([128, 512], F32, tag="kps")
                        for kt in range(HT):
                            nc.tensor.matmul(
                                ps[:], wk[:, kt, kc * 128:(kc + 1) * 128],
                                xlnT[:, kt, 4 * n:4 * n + 4, :],
                                start=(kt == 0), stop=(kt == HT - 1))
                        nc.scalar.activation(
                            KTb[:, kc, n * 512:(n + 1) * 512], ps[:],
                            AF.Identity, bias=bqkv_sb[:, 8 + kc:9 + kc])
                    # V matmuls for this block (token-major)
                    for j in range(4):
                        tt = 4 * n + j
                        for vh in range(2):
                            ps = apsv.tile([128, 512], F32, tag="vps")
                            for kt in range(HT):
                                nc.tensor.matmul(
                                    ps[:], xlnT[:, kt, tt, :],
                                    wv[:, kt, vh * 512:(vh + 1) * 512],
                                    start=(kt == 0), stop=(kt == HT - 1))
                            nc.vector.tensor_copy(
                                Vb[:, tt, vh * 8:(vh + 1) * 8, 0:64],
                                ps[:].rearrange("p (h c) -> p h c", h=8))
                    # own tokens: LN + transpose for Q (overlaps K/V matmuls)
                    for so in (2 * n, 2 * n + 1):
                        ln_transpose(x_ownr_d[so * 128:(so + 1) * 128, :],
                                     xownT, so)
                aw_cm.__exit__(None, None, None)

        # =========== Phase B: attention ===========
        attn_cm = tc.tile_pool(name="attn_p", bufs=1, side="right")
        attn_p = attn_cm.__enter__()
        attnT = attn_p.tile([128, HT, OWN], F32R)
        bmask_sb = attn_p.tile([128, 16, 2, 128], BF16)
        nc.sync.dma_start(bmask_sb[:], bmask_d[:])
        # out-proj weights, prefetched during attention
        wons = [attn_p.tile([128, HT, 512], F32R, name=f"won{n}") for n in range(2)]
        for n in range(2):
            nc.sync.dma_start(
                wons[n][:], wout_d[:, n * 512:(n + 1) * 512]
                .rearrange("(kt p) c -> p kt c", p=128))
        with (
            tc.tile_pool(name="b_p", bufs=7) as bp,
            tc.tile_pool(name="b_sc", bufs=3) as bsc,
            tc.tile_pool(name="b_wq", bufs=2) as bwq,
            tc.tile_pool(name="b_ps", bufs=2, space="PSUM") as bps,
            tc.tile_pool(name="b_pv", bufs=1, space="PSUM") as bpv,
        ):
            with nc.named_scope("phaseB"):
                # Head-PAIR units: the two heads of a 128-column chunk have
                # their K dims on partitions 0:64 / 64:128, so their score
                # matmuls land on disjoint PE-array quadrants (0,0)/(64,0)
                # and execute concurrently (measured 2x). Units are
                # (head-pair, key-tile, query-half); PV emission lags 3 units
                # behind scores/exp/mask; softmax normalization is deferred
                # and split so the in-order DVE queue never blocks.
                pvs = {}   # hp -> [[pv_qb0, pv_qb1] for head i in 0,1]

                def emit_q(qc):
                    wcol = bwq.tile([128, HT, 128], BF16, tag="wcol")
                    nc.sync.dma_start(
                        wcol[:], wq_d[:, qc * 128:(qc + 1) * 128]
                        .rearrange("(kt p) c -> p kt c", p=128))
                    for half in range(2):
                        ps = bps.tile([128, 1024], F32, tag="sc")
                        for kt in range(HT):
                            nc.tensor.matmul(
                                ps[:, 0:512], wcol[:, kt, :],
                                xownT[:, kt, half * 4:(half + 1) * 4, :],
                                start=(kt == 0), stop=(kt == HT - 1))
                        nc.vector.tensor_scalar(
                            QTb[:, qc, half * 512:(half + 1) * 512], ps[:, 0:512],
                            bqkv_sb[:, qc:qc + 1], None, AluOpType.add)

                # units per head-pair: query-half 0 for key tiles 0..7, then
                # query-half 1 for all 16 key tiles. Both heads of the pair
                # are processed in each unit: their score matmuls use
                # disjoint PE-array quadrants (rows 0:64 / 64:128) and run
                # concurrently.
                UNITS = [(kt, 0) for kt in range(8)] + [(kt, 1) for kt in range(16)]

                def emit_scores(hp, kt, qh):
                    a = FS[kt] * 128
                    q0 = a if qh == 0 else max(a, 512)
                    q1 = 512 if qh == 0 else 1024
                    w = q1 - q0
                    sc = bps.tile([128, 2, 512], F32, tag="sc")
                    p = bp.tile([128, 2, 512], BF16, tag="p")
                    for i in range(2):
                        hb = 64 * i
                        nc.tensor.matmul(
                            sc[:, i, 0:w],
                            KTb[hb:hb + 64, hp, kt * 128:(kt + 1) * 128],
                            QTb[hb:hb + 64, hp, q0:q1],
                            start=True, stop=True)
                    nc.scalar.activation(p[:, :, 0:w], sc[:, :, 0:w],
                                         AF.Exp, scale=SCALE)
                    if (qh == 0) == (kt <= 7):
                        # this query-half contains the first (masked) block
                        nc.vector.tensor_tensor(
                            p[:, :, 0:128], p[:, :, 0:128],
                            bmask_sb[:, kt, :, :], AluOpType.mult)
                    return p

                def get_pv(hp, i, qb):
                    key = (hp, i, qb)
                    if key not in pvs:
                        pvs[key] = bpv.tile([65, 512], F32, tag=f"pv{i}{qb}",
                                            name=f"pv_{hp}_{i}_{qb}")
                    return pvs[key]

                def emit_pv(hp, kt, qh, p):
                    a = FS[kt] * 128
                    for i in range(2):
                        h = 2 * hp + i
                        vstat = Vb[:, kt, h, :]
                        if qh == 0:
                            nc.tensor.matmul(get_pv(hp, i, 0)[:, a:512], vstat,
                                             p[:, i, 0:512 - a],
                                             start=(kt == 0), stop=(kt == 7))
                        else:
                            off = max(a, 512) - 512
                            nc.tensor.matmul(get_pv(hp, i, 1)[:, off:512], vstat,
                                             p[:, i, 0:512 - off],
                                             start=(kt == 0), stop=(kt == 15))

                # Deferred, split normalization for one query block; two tasks
                # popped per unit so each pv psum slot frees well before the
                # next head pair needs it.
                def norm_tasks(hp, qb):
                    tasks = []
                    for i in range(2):
                        rdt = bsc.tile([1, 512], F32, tag="rdt",
                                       name=f"rdt{hp}_{i}_{qb}")
                        rds = bsc.tile([1, 512], F32, tag="rd",
                                       name=f"rd{hp}_{i}_{qb}")
                        bcs = bsc.tile([64, 512], F32, tag="bc",
                                       name=f"bc{hp}_{i}_{qb}")

                        def t_recip(i=i, rdt=rdt, rds=rds):
                            nc.vector.tensor_copy(
                                rdt[:], pvs[(hp, i, qb)][64:65, :])
                            nc.vector.reciprocal_approx_fast(rds[:], rdt[:])

                        def t_bcast(rds=rds, bcs=bcs):
                            nc.gpsimd.partition_broadcast(bcs[:], rds[0:1, :])

                        def t_mult(i=i, bcs=bcs):
                            hb = 64 * i
                            nc.vector.tensor_tensor(
                                attnT[hb:hb + 64, hp, qb * 512:(qb + 1) * 512],
                                pvs[(hp, i, qb)][0:64, :], bcs[:],
                                AluOpType.mult)
                        tasks += [t_recip, t_bcast, t_mult]
                    return tasks

                lagq = []      # pending (hp, kt, qh, p) awaiting PV emission
                normq = []     # pending norm tasks, two popped per unit

                def advance(new=None):
                    if len(lagq) > 2 or (new is None and lagq):
                        hp, kt, qh, p = lagq.pop(0)
                        emit_pv(hp, kt, qh, p)
                        if (kt, qh) == (7, 0):
                            normq.extend(norm_tasks(hp, 0))
                        elif (kt, qh) == (15, 1):
                            normq.extend(norm_tasks(hp, 1))
                    for _ in range(2):
                        if normq:
                            normq.pop(0)()
                    if new is not None:
                        lagq.append(new)

                for hp in range(8):
                    emit_q(hp)
                    for kt, qh in UNITS:
                        p = emit_scores(hp, kt, qh)
                        advance((hp, kt, qh, p))
                while lagq or normq:
                    advance()

        # =========== Phase C: out-proj + residual + LN2 + y^T ===========
        kvq_cm.__exit__(None, None, None)
        hy_cm = tc.tile_pool(name="hy", bufs=1)
        hy = hy_cm.__enter__()
        h_sb = hy.tile([128, NSLOT, H], F32)
        yT_sb = hy.tile([128, HT, OWN], F32R)
        rwp_cm = tc.tile_pool(name="rwp", bufs=1)
        rwp = rwp_cm.__enter__()
        rw_sb = rwp.tile([128, NSLOT, E], F32)
        rlg_sb = rwp.tile([128, NSLOT, E], F32)
        rwT_f0 = rwp.tile([1, E, OWN], F32R)   # expert rows on partition 0
        rwT_r = rwp.tile([8, OWN], F32R)
        wr = rwp.tile([128, HT, E], F32R)
        nc.sync.dma_start(wr[:], wrout_d.rearrange("(kt p) e -> p kt e", p=128))
        br = rwp.tile([128, E], F32)
        nc.sync.dma_start(br[:], brout_d[:])
        with (
            tc.tile_pool(name="c_io", bufs=3) as cio,
            tc.tile_pool(name="c_ps", bufs=2, space="PSUM") as cps,
            tc.tile_pool(name="c_pst", bufs=3, space="PSUM") as cpst,
            tc.tile_pool(name="c_pr", bufs=2, space="PSUM") as cpr,
        ):
            with nc.named_scope("phaseC"):
                def ln2_transpose(tt):
                    yt = cio.tile([128, H], F32R, tag="yt")
                    layer_norm_apply(cio, h_sb[:, tt, :], yt[:])
                    for kt2 in range(2):
                        tp = cpst.tile([128, 4, 128], F32R, tag="tp2")
                        for q in range(4):
                            kt = kt2 * 4 + q
                            nc.tensor.transpose(
                                tp[:, q, :], yt[:, kt * 128:(kt + 1) * 128], id_r)
                        nc.vector.tensor_copy(
                            yT_sb[:, kt2 * 4:(kt2 + 1) * 4, tt * 128:(tt + 1) * 128],
                            tp[:])

                def router_logits(tt):
                    ps = cpr.tile([128, E], F32, tag="rps")
                    for kt in range(HT):
                        nc.tensor.matmul(ps[:], yT_sb[:, kt, tt * 128:(tt + 1) * 128],
                                         wr[:, kt, :],
                                         start=(kt == 0), stop=(kt == HT - 1))
                    nc.vector.tensor_tensor(rlg_sb[:, tt, :], ps[:], br[:],
                                            AluOpType.add)

                for n in range(2):
                    for tt in range(NSLOT):
                        ps = cps.tile([128, 512], F32, tag="ops")
                        for kt in range(HT):
                            nc.tensor.matmul(ps[:], attnT[:, kt, tt * 128:(tt + 1) * 128],
                                             wons[n][:, kt, :],
                                             start=(kt == 0), stop=(kt == HT - 1))
                        xo = cio.tile([128, 512], F32, tag="xo")
                        nc.sync.dma_start(
                            xo[:], x_own_d[tt * 128:(tt + 1) * 128, n * 512:(n + 1) * 512])
                        nc.vector.tensor_tensor(h_sb[:, tt, n * 512:(n + 1) * 512],
                                                ps[:], xo[:], AluOpType.add)
                        if n == 1:
                            if tt >= 1:
                                ln2_transpose(tt - 1)
                            if tt >= 2:
                                router_logits(tt - 2)
                ln2_transpose(NSLOT - 1)
                router_logits(NSLOT - 2)
                router_logits(NSLOT - 1)

        # =========== Phase D: router softmax + experts ===========
        attn_cm.__exit__(None, None, None)
        h2p_cm = tc.tile_pool(name="h2p", bufs=1, side="right")
        h2p = h2p_cm.__enter__()
        h2_sb = h2p.tile([128, NSLOT, H], F32)
        with (
            tc.tile_pool(name="d_sc", bufs=2) as dsc,
            tc.tile_pool(name="d_pst", bufs=2, space="PSUM") as dpst,
        ):
            with nc.named_scope("phaseD_router"):
                for tt in range(NSLOT):
                    ex = dsc.tile([128, E], F32, tag="ex")
                    rsum = dsc.tile([128, 1], F32, tag="rsum")
                    nc.scalar.activation(ex[:], rlg_sb[:, tt, :], AF.Exp,
                                         accum_out=rsum[:])
                    rrec = dsc.tile([128, 1], F32, tag="rrec")
                    nc.vector.reciprocal(rrec[:], rsum[:])
                    nc.vector.tensor_scalar(rw_sb[:, tt, :], ex[:], rrec[:], None,
                                            AluOpType.mult)
                    tp = dpst.tile([8, 128], F32, tag="rtp")
                    nc.tensor.transpose(tp[:], rw_sb[:, tt, :], id_f[:])
                    nc.vector.tensor_copy(rwT_r[:, tt * 128:(tt + 1) * 128], tp[:])
                    nc.sync.dma_start(rwT_f0[0:1, :, tt * 128:(tt + 1) * 128],
                                      rwT_r[:, tt * 128:(tt + 1) * 128])

        with (
            tc.tile_pool(name="d_y", bufs=2) as dy,
            tc.tile_pool(name="d_w", bufs=4) as dw,
            tc.tile_pool(name="d_b", bufs=1) as db,
            tc.tile_pool(name="d_moe", bufs=1, space="PSUM") as dmoe,
        ):
            with nc.named_scope("phaseD_experts"):
                bexp_sb = db.tile([8, H], F32R)
                nc.sync.dma_start(bexp_sb[:], bexp_d[:])
                for fh in range(2):
                    mps = [dmoe.tile([128, 512], F32, tag=f"mps{tt}",
                                     name=f"mps_{fh}_{tt}")
                           for tt in range(NSLOT)]
                    for e in range(E):
                        bcr = dy.tile([128, OWN], F32R, tag="bcr")
                        nc.gpsimd.partition_broadcast(bcr[:], rwT_f0[0:1, e, :])
                        for kt in range(HT):
                            ye = dy.tile([128, OWN], F32R, tag="ye", bufs=4)
                            nc.vector.tensor_tensor(ye[:], yT_sb[:, kt, :],
                                                    bcr[:], AluOpType.mult)
                            we = dw.tile([128, 512], F32R, tag="we")
                            nc.sync.dma_start(
                                we[:],
                                wexp_d[e, kt * 128:(kt + 1) * 128,
                                       fh * 512:(fh + 1) * 512])
                            for tt in range(NSLOT):
                                nc.tensor.matmul(
                                    mps[tt][:], ye[:, tt * 128:(tt + 1) * 128],
                                    we[:],
                                    start=(e == 0 and kt == 0), stop=False)
                    for tt in range(NSLOT):
                        nc.tensor.matmul(mps[tt][:],
                                         rwT_r[:, tt * 128:(tt + 1) * 128],
                                         bexp_sb[:, fh * 512:(fh + 1) * 512],
                                         start=False, stop=True)
                        nc.vector.tensor_tensor(
                            h2_sb[:, tt, fh * 512:(fh + 1) * 512], mps[tt][:],
                            h_sb[:, tt, fh * 512:(fh + 1) * 512], AluOpType.add)

        # =========== Phase E: gate + LNf + output ===========
        rwp_cm.__exit__(None, None, None)
        hy_cm.__exit__(None, None, None)
        with (
            tc.tile_pool(name="e_sc", bufs=1) as esc,
            tc.tile_pool(name="e_tmp", bufs=2) as etmp,
            tc.tile_pool(name="e_pst", bufs=3, space="PSUM") as epst,
            tc.tile_pool(name="e_psg", bufs=2, space="PSUM") as epsg,
            tc.tile_pool(name="e_psal", bufs=1, space="PSUM") as epsal,
            tc.tile_pool(name="e_out", bufs=3) as eout,
        ):
            with nc.named_scope("phaseE"):
                mask_pp = esc.tile([128, NSLOT], F32)
                h2T = esc.tile([128, HT, OWN], F32R)
                for tt in range(NSLOT):
                    for kt2 in range(2):
                        tp = epst.tile([128, 4, 128], F32, tag="tp3")
                        for q in range(4):
                            kt = kt2 * 4 + q
                            nc.tensor.transpose(
                                tp[:, q, :], h2_sb[:, tt, kt * 128:(kt + 1) * 128],
                                id_f[:])
                        nc.vector.tensor_copy(
                            h2T[:, kt2 * 4:(kt2 + 1) * 4, tt * 128:(tt + 1) * 128],
                            tp[:])
                wal1 = esc.tile([128, HT, 256], F32R)
                nc.sync.dma_start(wal1[:], wal1_d.rearrange("(kt p) c -> p kt c", p=128))
                bal1 = esc.tile([128, 2], F32)
                nc.sync.dma_start(bal1[:], bal1_d[:])
                wal2 = esc.tile([128, 2, 2], F32R)
                nc.sync.dma_start(wal2[:], wal2_d.rearrange("(m p) c -> p m c", p=128))
                gT = esc.tile([128, 2, OWN], F32R)
                for m2 in range(2):
                    for n in range(2):
                        ps = epsg.tile([128, 512], F32, tag="gps")
                        for kt in range(HT):
                            nc.tensor.matmul(ps[:],
                                             wal1[:, kt, m2 * 128:(m2 + 1) * 128],
                                             h2T[:, kt, n * 512:(n + 1) * 512],
                                             start=(kt == 0), stop=(kt == HT - 1))
                        nc.scalar.activation(gT[:, m2, n * 512:(n + 1) * 512], ps[:],
                                             AF.Gelu, bias=bal1[:, m2:m2 + 1])
                for tt in range(NSLOT):
                    ps = epsal.tile([128, 2], F32, tag="alps")
                    for m2 in range(2):
                        nc.tensor.matmul(ps[:], gT[:, m2, tt * 128:(tt + 1) * 128],
                                         wal2[:, m2, :],
                                         start=(m2 == 0), stop=(m2 == 1))
                    nc.vector.tensor_scalar(mask_pp[:, tt:tt + 1], ps[:, 0:1],
                                            thresh_sb[:, 0:1], None, AluOpType.is_gt)
                for tt in range(NSLOT):
                    stats = eout.tile([128, 2, 6], F32, tag="st3")
                    nc.vector.bn_stats(stats[:, 0, :], h2_sb[:, tt, 0:512])
                    nc.vector.bn_stats(stats[:, 1, :], h2_sb[:, tt, 512:1024])
                    mv = eout.tile([128, 2], F32, tag="mv3")
                    nc.vector.bn_aggr(mv[:], stats[:])
                    sd = eout.tile([128, 1], F32, tag="sd3")
                    nc.scalar.activation(sd[:], mv[:, 1:2], AF.Sqrt, bias=eps_sb[:])
                    rstd = eout.tile([128, 1], F32, tag="rs3")
                    nc.vector.reciprocal(rstd[:], sd[:])
                    seff = eout.tile([128, 1], F32, tag="se3")
                    nc.vector.tensor_tensor(seff[:], rstd[:], mask_pp[:, tt:tt + 1],
                                            AluOpType.mult)
                    beff = eout.tile([128, 1], F32, tag="be3")
                    nc.vector.scalar_tensor_tensor(beff[:], mv[:, 0:1], -1.0, seff[:],
                                                   AluOpType.mult, AluOpType.mult)
                    ot = eout.tile([128, H], F32, tag="ot")
                    nc.scalar.activation(ot[:], h2_sb[:, tt, :], AF.Identity,
                                         bias=beff[:], scale=seff[:])
                    nc.sync.dma_start(out_d[tt * 128:(tt + 1) * 128, :], ot[:])

        h2p_cm.__exit__(None, None, None)
        small_cm.__exit__(None, None, None)

    nc.compile()
    return nc


def _prep_host(inputs):
    f32 = np.float32
    bf16 = ml_dtypes.bfloat16
    x = np.asarray(inputs["inputs"], f32)
    ln1_g = np.asarray(inputs["ln1_g"], f32); ln1_b = np.asarray(inputs["ln1_b"], f32)
    w_qkv = np.asarray(inputs["w_qkv"], f32); b_qkv = np.asarray(inputs["b_qkv"], f32)
    w_out = np.asarray(inputs["w_out"], f32); b_out = np.asarray(inputs["b_out"], f32)
    ln2_g = np.asarray(inputs["ln2_g"], f32); ln2_b = np.asarray(inputs["ln2_b"], f32)
    w_router = np.asarray(inputs["w_router"], f32)
    b_router = np.asarray(inputs["b_router"], f32)
    w_exp = np.asarray(inputs["w_exp"], f32); b_exp = np.asarray(inputs["b_exp"], f32)
    w_al1 = np.asarray(inputs["w_al1"], f32); b_al1 = np.asarray(inputs["b_al1"], f32)
    w_al2 = np.asarray(inputs["w_al2"], f32); b_al2 = np.asarray(inputs["b_al2"], f32)

    wq_f = (ln1_g[:, None] * w_qkv).astype(bf16)
    bq_f = b_qkv + ln1_b @ w_qkv
    assert np.all(bq_f[2 * H:] == 0.0), "nonzero V bias not supported"
    bqkv_t = np.zeros((128, 16), f32)
    for j in range(16):
        bqkv_t[:, j] = bq_f[j * 128:(j + 1) * 128]
    wr_f = ln2_g[:, None] * w_router
    br_f = b_router + ln2_b @ w_router
    we_f = ln2_g[None, :, None] * w_exp
    be_f = b_exp + np.einsum("h,ehf->ef", ln2_b, w_exp)
    x_pb = x + b_out[None, None, :]

    def mk_bmask(par):
        # first-suffix-block mask per key chunk kt: key j (row), query i (col)
        # within the block; slot s0 = kt//2 has global query chunk c = 2*s0+par.
        m = np.zeros((128, 16, 2, 128), f32)
        for kt in range(16):
            c = 2 * (kt // 2) + par
            if c == kt:
                m[:, kt, :, :] = (np.arange(128)[:, None]
                                  <= np.arange(128)[None, :])[:, None, :]
            elif c > kt:
                m[:, kt, :, :] = 1.0
            # c < kt: fully masked -> zeros
        return m.reshape(128, 16 * 2 * 128).astype(bf16)
    bmasks = [mk_bmask(0), mk_bmask(1)]

    thresh = np.full((128, 1), 0.8 - float(b_al2[0]), f32)
    bal1_t = np.zeros((128, 2), f32)
    bal1_t[:, 0] = b_al1[0:128]
    bal1_t[:, 1] = b_al1[128:256]

    shared = dict(
        wq=np.ascontiguousarray(wq_f),
        bqkv=bqkv_t,
        wout=np.ascontiguousarray(w_out),
        wrout=np.ascontiguousarray(wr_f.astype(f32)),
        brout=np.tile(br_f[None, :], (128, 1)).astype(f32),
        wexp=np.ascontiguousarray(we_f.astype(f32)),
        bexp=np.ascontiguousarray(be_f.astype(f32)),
        wal1=np.ascontiguousarray(w_al1),
        bal1=bal1_t,
        wal2=np.ascontiguousarray(
            np.concatenate([w_al2, np.zeros_like(w_al2)], axis=1)),
        thresh=thresh,
    )
    per_core = []
    for c in range(N_CORES):
        b, par = c // 2, c % 2
        own_idx = np.concatenate(
            [np.arange(128) + (2 * s + par) * 128 for s in range(NSLOT)])
        m = dict(shared)
        m["x_kv"] = np.ascontiguousarray(x[b])
        m["x_ownr"] = np.ascontiguousarray(x[b][own_idx])
        m["x_own"] = np.ascontiguousarray(x_pb[b][own_idx])
        m["bmask"] = bmasks[par]
        per_core.append(m)
    return per_core


def kernel(**inputs):
    from concourse.bass_utils import run_bass_kernel_spmd

    if "prog" not in _prog_cache:
        _prog_cache["prog"] = _build_program()
    nc = _prog_cache["prog"]

    per_core = _prep_host(inputs)
    trace = bool(globals().get("TRACE", False))
    res = run_bass_kernel_spmd(nc, per_core, core_ids=list(range(N_CORES)),
                               trace=trace)
    _prog_cache["last_result"] = res

    lnf_g = np.asarray(inputs["lnf_g"], np.float32)
    lnf_b = np.asarray(inputs["lnf_b"], np.float32)
    out = np.zeros((B, S, H), np.float32)
    for c in range(N_CORES):
        b, par = c // 2, c % 2
        o = res.results[c]["out"]
        for s in range(NSLOT):
            g0 = (2 * s + par) * 128
            out[b, g0:g0 + 128, :] = o[s * 128:(s + 1) * 128, :]
    return out * lnf_g[None, None, :] + lnf_b[None, None, :]
